# revision 1
# baseline (speedup 1.0000x reference)
"""Trainium2 Bass kernel for CoreAttentionExpand (sparse local+global attention).

Sharding: tensor-parallel over heads. 16 heads / 8 cores = 2 heads per core.
Each core computes RoPE + local-block attention + pooled-global attention for
its 2 heads end-to-end (no collectives); host reassembles the full output.

Device layout: head-dim-major [D=128 partitions, T] for q/k so QK^T and PV
matmuls contract over partitions. Scores are computed transposed
(S^T = K @ Q^T -> [k-tokens, q-tokens]) so exp(S^T) feeds the PV matmul
directly as the moving operand (no P transposes). Causal/history/global
masks are applied by accumulating -1e4 ramp matmuls (bf16) into the score
PSUM before exp; exp underflows those entries to exactly 0.
"""

import sys
import math

_REPO = "/opt/trn_rl_repo"
if _REPO not in sys.path:
    sys.path.insert(0, _REPO)

import numpy as np
import ml_dtypes

# ---------------------------------------------------------------- constants
H = 16          # heads
D = 128         # head dim
T = 4096        # tokens
L = 1024        # local block size
S = 128         # global pool stride
E = 128         # local history size
PNUM = T // L   # 4 local blocks
KLEN = T // S + 1  # 33 global keys (incl. zero token)
NCORES = 8
HPC = H // NCORES  # 2 heads per core
NEGBIG = -10000.0
SCALE = 1.0 / math.sqrt(D)
CHUNK = 512     # q-chunk width

_CACHE = {}


def _apply_framework_patches(bassmod, mybir, tilemod):
    """This walrus build rejects >1 sem wait per instruction; split excess
    waits onto preceding same-engine NoOps (pure scheduling transform)."""
    if getattr(tilemod.TileContext, "_wait_split_patched", False):
        return
    TileContext = tilemod.TileContext
    ScopedClock = tilemod.ScopedClock

    orig_add = TileContext._add_instruction
    ctr = [0]

    def split_add(self, inst):
        si = inst.sync_info
        if si is not None and si.on_wait and len(si.on_wait) > 1:
            ow = list(si.on_wait)
            for w in ow[:-1]:
                ctr[0] += 1
                nop = mybir.InstNoOp(name=f"I-wsplit{ctr[0]}", engine=inst.engine)
                nop.sync_info = mybir.SyncInfo(on_wait=[w], on_update=[])
                orig_add(self, nop)
            si.on_wait = [ow[-1]]
        orig_add(self, inst)

    def drain_and_barrier(self, tick_clock, wait_clock):
        nc = self.nc
        probe = nc.sync.nop(nofuse=True, hint="waitprobe")
        wait_clock.add_sem_waits(
            probe.ins, ScopedClock({None: tick_clock.global_clock})
        )
        si = probe.ins.sync_info
        ow = list(si.on_wait) if si and si.on_wait else []
        if len(ow) > 1:
            si.on_wait = ow[:1]
            for w in ow[1:]:
                n2 = nc.sync.nop(nofuse=True, hint="waitsplit")
                n2.ins.sync_info = mybir.SyncInfo(on_wait=[w], on_update=[])
        nc.sync.drain()
        nc.all_engine_barrier()
        popped = nc._tile_sem_poison_stack.pop()
        assert popped is self._sem_poison
        nc.clear_and_free_semaphores(list(self.sems.allocated().values()))
        nc.all_engine_barrier()

    TileContext._add_instruction = split_add
    TileContext._drain_and_barrier = drain_and_barrier
    TileContext._wait_split_patched = True


# ---------------------------------------------------------------- constants (host)
def _host_constants():
    t = np.arange(T, dtype=np.float32)
    inv = (1.0 / (10000.0 ** (np.arange(0, D, 2, dtype=np.float32) / D))).astype(
        np.float32
    )  # [64]
    emb = t[:, None] * inv[None, :]          # [T, 64]
    cos64 = np.cos(emb).astype(np.float32)
    sin64 = np.sin(emb).astype(np.float32)
    # [D, T] head-dim-major tables
    cosT = np.ascontiguousarray(np.concatenate([cos64, cos64], axis=1).T)
    sinRT = np.ascontiguousarray(np.concatenate([-sin64, sin64], axis=1).T)

    bf = ml_dtypes.bfloat16
    idx = np.arange(128)
    # causal ramp: (mB^T mC)[k, q] = NEGBIG * max(k - q, 0)
    mB = (idx[:, None] <= idx[None, :]).astype(bf)              # [m, k]: m <= k
    mC = (NEGBIG * (idx[:, None] > idx[None, :])).astype(bf)    # [m, q]: m > q
    ones_bf = np.ones((128, 128), dtype=bf)
    # global stairstep: for chunk c, row j, masked cols qq < 128*(j - 4c)
    gB = np.zeros((3, 8 * KLEN), dtype=np.float32)
    for c in range(8):
        for mm in range(3):
            for j in range(KLEN):
                gB[mm, KLEN * c + j] = 1.0 if j >= 4 * c + mm + 1 else 0.0
    gB = gB.astype(bf)
    qq = np.arange(CHUNK)
    gC = np.stack(
        [NEGBIG * ((qq >= 128 * mm) & (qq < 128 * (mm + 1))) for mm in range(3)]
    ).astype(bf)                                                # [3, 512]
    ident = np.eye(128, dtype=np.float32)
    poolcol = np.full((128, 1), 1.0 / S, dtype=bf)              # pooling matmul rhs
    return {
        "cosT": cosT,
        "sinRT": sinRT,
        "mB": mB,
        "mC": mC,
        "ones_bf": ones_bf,
        "gB": gB,
        "gC": gC,
        "ident": ident,
        "poolcol": poolcol,
    }


# ---------------------------------------------------------------- device program
def _build_program():
    import concourse.bass as bass
    import concourse.mybir as mybir
    import concourse.tile as tile

    _apply_framework_patches(bass, mybir, tile)

    f32 = mybir.dt.float32
    f32r = mybir.dt.float32r
    bf16 = mybir.dt.bfloat16
    EXP = mybir.ActivationFunctionType.Exp
    MUL = mybir.AluOpType.mult
    ADD = mybir.AluOpType.add

    nc = bass.Bass()
    qT_d = nc.dram_tensor("qT", [HPC, D, T], f32, kind="ExternalInput")
    kT_d = nc.dram_tensor("kT", [HPC, D, T], f32, kind="ExternalInput")
    v_d = nc.dram_tensor("v", [HPC, T, D], f32, kind="ExternalInput")
    zk_d = nc.dram_tensor("zk", [HPC, D, 1], f32, kind="ExternalInput")
    zv_d = nc.dram_tensor("zv", [HPC, D, 1], f32, kind="ExternalInput")
    cosT_d = nc.dram_tensor("cosT", [D, T], f32, kind="ExternalInput")
    sinRT_d = nc.dram_tensor("sinRT", [D, T], f32, kind="ExternalInput")
    mB_d = nc.dram_tensor("mB", [128, 128], bf16, kind="ExternalInput")
    mC_d = nc.dram_tensor("mC", [128, 128], bf16, kind="ExternalInput")
    ones_d = nc.dram_tensor("ones_bf", [128, 128], bf16, kind="ExternalInput")
    gB_d = nc.dram_tensor("gB", [3, 8 * KLEN], bf16, kind="ExternalInput")
    gC_d = nc.dram_tensor("gC", [3, CHUNK], bf16, kind="ExternalInput")
    ident_d = nc.dram_tensor("ident", [128, 128], f32, kind="ExternalInput")
    poolcol_d = nc.dram_tensor("poolcol", [128, 1], bf16, kind="ExternalInput")
    out_d = nc.dram_tensor("outT", [HPC, D, T], f32, kind="ExternalOutput")

    NT = T // 128  # 32 token-tiles per head

    with tile.TileContext(nc) as tc:
        with (
            tc.tile_pool(name="persist", bufs=1) as persist,
            tc.tile_pool(name="vload", bufs=1) as vload,
            tc.tile_pool(name="expp", bufs=12) as expp,
            tc.tile_pool(name="small", bufs=4) as small,
            tc.tile_pool(name="combine", bufs=2) as combine,
            tc.tile_pool(name="scores", bufs=2, space="PSUM") as scores_p,
            tc.tile_pool(name="acc", bufs=1, space="PSUM") as acc_p,
        ):
            # ---- small constants (live whole kernel)
            mB = persist.tile([128, 128], bf16, tag="mB")
            mC = persist.tile([128, 128], bf16, tag="mC")
            ones_bf = persist.tile([128, 128], bf16, tag="ones")
            gB = persist.tile([3, 8 * KLEN], bf16, tag="gB")
            gC = persist.tile([3, CHUNK], bf16, tag="gC")
            ident = persist.tile([128, 128], f32, tag="ident")
            poolcol = persist.tile([128, 1], bf16, tag="poolcol")
            nc.sync.dma_start(out=mB, in_=mB_d.ap())
            nc.sync.dma_start(out=mC, in_=mC_d.ap())
            nc.sync.dma_start(out=ones_bf, in_=ones_d.ap())
            nc.sync.dma_start(out=gB, in_=gB_d.ap())
            nc.sync.dma_start(out=gC, in_=gC_d.ap())
            nc.sync.dma_start(out=ident, in_=ident_d.ap())
            nc.sync.dma_start(out=poolcol, in_=poolcol_d.ap())

            QT, KT = {}, {}
            # ================= phase 1: RoPE (tables + transients freed after)
            with (
                tc.tile_pool(name="tables", bufs=1) as tables,
                tc.tile_pool(name="rope", bufs=2) as rope,
            ):
                cosT = tables.tile([D, T], f32, tag="cosT")
                sinRT = tables.tile([D, T], f32, tag="sinRT")
                nc.sync.dma_start(out=cosT, in_=cosT_d.ap())
                nc.sync.dma_start(out=sinRT, in_=sinRT_d.ap())
                for h in range(HPC):
                    QT[h] = persist.tile([D, T], f32r, tag=f"QT{h}", name=f"QT{h}")
                    KT[h] = persist.tile([D, T], f32r, tag=f"KT{h}", name=f"KT{h}")
                    for src_d, dst in ((qT_d, QT[h]), (kT_d, KT[h])):
                        for c0 in range(0, T, 1024):
                            cs = slice(c0, c0 + 1024)
                            raw = rope.tile([D, 1024], f32, tag="raw")
                            rot = rope.tile([D, 1024], f32, tag="rot")
                            tmp = rope.tile([D, 1024], f32, tag="tmp")
                            tmp2 = rope.tile([D, 1024], f32, tag="tmp2", bufs=1)
                            nc.sync.dma_start(out=raw, in_=src_d.ap()[h, :, cs])
                            nc.sync.dma_start(
                                out=rot[0:64, :], in_=src_d.ap()[h, 64:128, cs]
                            )
                            nc.sync.dma_start(
                                out=rot[64:128, :], in_=src_d.ap()[h, 0:64, cs]
                            )
                            # tmp = rot * sinRT  (GPSIMD, SBUF-only)
                            nc.gpsimd.tensor_tensor(
                                out=tmp, in0=rot, in1=sinRT[:, cs], op=MUL
                            )
                            nc.vector.tensor_tensor(
                                out=tmp2, in0=raw, in1=cosT[:, cs], op=MUL
                            )
                            nc.vector.tensor_tensor(
                                out=dst[:, cs], in0=tmp2, in1=tmp, op=ADD
                            )

            # ================= phase 2: attention per head
            for h in range(HPC):
                # V load + bf16 cast (token-major tiles [t%128, tile, d])
                vraw = vload.tile([128, NT, D], f32, tag="vraw")
                nc.sync.dma_start(
                    out=vraw, in_=v_d.ap()[h].rearrange("(n p) d -> p n d", p=128)
                )
                vbf = persist.tile([128, NT, D], bf16, tag=f"vbf{h}")
                nc.vector.tensor_copy(out=vbf, in_=vraw)

                # global pooled K/V
                kgT = persist.tile([D, KLEN], f32r, tag=f"kgT{h}", name=f"kgT{h}")
                kgf = small.tile([D, KLEN], f32, tag="kgf")  # f32 scratch
                nc.sync.dma_start(out=kgf[:, 0:1], in_=zk_d.ap()[h])
                nc.vector.tensor_reduce(
                    out=kgf[:, 1:KLEN],
                    in_=KT[h].bitcast(f32).rearrange("p (g s) -> p g s", s=S),
                    axis=mybir.AxisListType.X,
                    op=ADD,
                )
                nc.vector.tensor_copy(out=kgT[:, 0:1], in_=kgf[:, 0:1])
                nc.vector.tensor_scalar_mul(
                    out=kgT[:, 1:KLEN], in0=kgf[:, 1:KLEN], scalar1=1.0 / S
                )
                # vg pooling via PE: vgTp[:, g+1] = V_g^T @ (1/S)
                vgTp = scores_p.tile([D, KLEN], f32, tag="s")
                for g in range(NT):
                    nc.tensor.matmul(
                        out=vgTp[:, g + 1 : g + 2],
                        lhsT=vbf[:, g, :],
                        rhs=poolcol,
                        start=(g == 0),
                        stop=(g == NT - 1),
                    )
                vgT = small.tile([D, KLEN], f32, tag="vgT")
                nc.sync.dma_start(out=vgT[:, 0:1], in_=zv_d.ap()[h])
                nc.vector.tensor_copy(out=vgT[:, 1:KLEN], in_=vgTp[:, 1:KLEN])
                # transpose -> Vg token-major [KLEN, D] bf16
                vgp = scores_p.tile([KLEN, 128], f32, tag="s")
                nc.tensor.transpose(out=vgp, in_=vgT, identity=ident)
                Vg = persist.tile([KLEN, 128], bf16, tag=f"Vg{h}")
                nc.vector.tensor_copy(out=Vg, in_=vgp)

                # blocks: global chunks then local block
                for p in range(PNUM):
                    og_norm = {}
                    # ---- global chunks 2p, 2p+1
                    for c in (2 * p, 2 * p + 1):
                        rows = min(KLEN, 4 * c + 4)
                        qs = slice(c * CHUNK, (c + 1) * CHUNK)
                        sg = scores_p.tile([128, 1024], f32, tag="s")
                        nc.tensor.matmul(
                            out=sg[0:rows, 0:CHUNK],
                            lhsT=kgT[:, 0:rows],
                            rhs=QT[h][:, qs],
                            start=True,
                            stop=False,
                        )
                        nc.tensor.matmul(
                            out=sg[0:rows, 0:CHUNK],
                            lhsT=gB[:, KLEN * c : KLEN * c + rows],
                            rhs=gC,
                            start=False,
                            stop=True,
                        )
                        eg = expp.tile([128, 1024], bf16, tag="e")
                        nc.scalar.activation(
                            out=eg[0:rows, 0:CHUNK],
                            in_=sg[0:rows, 0:CHUNK],
                            func=EXP,
                            scale=SCALE,
                        )
                        gs = acc_p.tile([128, 1024], f32, tag="sum")
                        nc.tensor.matmul(
                            out=gs[:, 0:CHUNK],
                            lhsT=ones_bf[0:rows, :],
                            rhs=eg[0:rows, 0:CHUNK],
                            start=True,
                            stop=True,
                        )
                        go = acc_p.tile([128, 1024], f32, tag="o")
                        nc.tensor.matmul(
                            out=go[:, 0:CHUNK],
                            lhsT=Vg[0:rows, :],
                            rhs=eg[0:rows, 0:CHUNK],
                            start=True,
                            stop=True,
                        )
                        rg = combine.tile([128, CHUNK], f32, tag="rg")
                        nc.vector.reciprocal(out=rg, in_=gs[:, 0:CHUNK])
                        ogn = combine.tile([128, CHUNK], f32, tag="ogn")
                        nc.vector.tensor_tensor(
                            out=ogn, in0=go[:, 0:CHUNK], in1=rg, op=MUL
                        )
                        og_norm[c] = ogn

                    # ---- local block p
                    q0 = p * L
                    ms = list(range(1, 9)) if p == 0 else list(range(0, 9))
                    expt = {}
                    sums = acc_p.tile([128, 1024], f32, tag="sum")
                    sum_started = [False, False]
                    sum_last_m = {
                        reg: max(
                            m
                            for m in ms
                            if (0 if m == 0 else 128 * (m - 1)) < 512 * (reg + 1)
                        )
                        for reg in (0, 1)
                    }
                    for m in ms:
                        start_m = 0 if m == 0 else 128 * (m - 1)
                        kcol = q0 - 128 + 128 * m  # k-token start (abs)
                        st = scores_p.tile([128, 1024], f32, tag="s")
                        # QK^T: per <=512-col PSUM region
                        for r0 in range(start_m - start_m % 512, 1024, 512):
                            c_lo = max(start_m, r0)
                            c_hi = r0 + 512
                            is_diag_reg = m >= 1 and start_m >= r0
                            nc.tensor.matmul(
                                out=st[:, c_lo:c_hi],
                                lhsT=KT[h][:, kcol : kcol + 128],
                                rhs=QT[h][:, q0 + c_lo : q0 + c_hi],
                                start=True,
                                stop=not is_diag_reg,
                            )
                            if is_diag_reg:
                                nc.tensor.matmul(
                                    out=st[:, start_m : start_m + 128],
                                    lhsT=mB,
                                    rhs=mC,
                                    start=False,
                                    stop=True,
                                )
                        et = expp.tile([128, 1024], bf16, tag="e")
                        nc.scalar.activation(
                            out=et[:, start_m:1024],
                            in_=st[:, start_m:1024],
                            func=EXP,
                            scale=SCALE,
                        )
                        expt[m] = et
                        # accumulate column sums: sums[:, c] += sum_k et[k, c]
                        for reg in (0, 1):
                            c_lo = max(start_m, reg * 512)
                            c_hi = (reg + 1) * 512
                            if c_lo >= c_hi:
                                continue
                            nc.tensor.matmul(
                                out=sums[:, c_lo:c_hi],
                                lhsT=ones_bf,
                                rhs=et[:, c_lo:c_hi],
                                start=not sum_started[reg],
                                stop=(m == sum_last_m[reg]),
                            )
                            sum_started[reg] = True
                    # PV: O^T accumulation per 512-col region
                    ot = acc_p.tile([128, 1024], f32, tag="o")
                    for reg in (0, 1):
                        valid_ms = [
                            m
                            for m in ms
                            if (0 if m == 0 else 128 * (m - 1)) < 512 * (reg + 1)
                        ]
                        for i, m in enumerate(valid_ms):
                            start_m = 0 if m == 0 else 128 * (m - 1)
                            c_lo = max(start_m, reg * 512)
                            c_hi = (reg + 1) * 512
                            vidx = 8 * p - 1 + m
                            nc.tensor.matmul(
                                out=ot[:, c_lo:c_hi],
                                lhsT=vbf[:, vidx, :],
                                rhs=expt[m][:, c_lo:c_hi],
                                start=(i == 0),
                                stop=(m == valid_ms[-1]),
                            )
                    # normalize + combine with global, write out
                    for reg in (0, 1):
                        c = 2 * p + reg
                        cols = slice(reg * 512, (reg + 1) * 512)
                        rl = combine.tile([128, CHUNK], f32, tag="rl")
                        nc.vector.reciprocal(out=rl, in_=sums[:, cols])
                        tl = combine.tile([128, CHUNK], f32, tag="tl")
                        nc.vector.tensor_tensor(
                            out=tl, in0=ot[:, cols], in1=rl, op=MUL
                        )
                        fin = combine.tile([128, CHUNK], f32, tag="fin")
                        nc.gpsimd.tensor_tensor(
                            out=fin, in0=tl, in1=og_norm[c], op=ADD
                        )
                        nc.sync.dma_start(
                            out=out_d.ap()[h, :, c * CHUNK : (c + 1) * CHUNK],
                            in_=fin,
                        )
    return nc


def _get_program():
    if "nc" not in _CACHE:
        _CACHE["nc"] = _build_program()
        _CACHE["consts"] = _host_constants()
    return _CACHE["nc"], _CACHE["consts"]


# ---------------------------------------------------------------- entry point
def kernel(q, k, v, zero_k, zero_v):
    nc, consts = _get_program()
    from concourse.bass_utils import run_bass_kernel_spmd

    q4 = np.asarray(q, dtype=np.float32).reshape(T, H, D)
    k4 = np.asarray(k, dtype=np.float32).reshape(T, H, D)
    v4 = np.asarray(v, dtype=np.float32).reshape(T, H, D)
    zk = np.asarray(zero_k, dtype=np.float32).reshape(H, D)
    zv = np.asarray(zero_v, dtype=np.float32).reshape(H, D)

    in_maps = []
    for core in range(NCORES):
        hs = slice(HPC * core, HPC * (core + 1))
        in_maps.append(
            {
                "qT": np.ascontiguousarray(q4[:, hs].transpose(1, 2, 0)),
                "kT": np.ascontiguousarray(k4[:, hs].transpose(1, 2, 0)),
                "v": np.ascontiguousarray(v4[:, hs].transpose(1, 0, 2)),
                "zk": np.ascontiguousarray(zk[hs])[:, :, None],
                "zv": np.ascontiguousarray(zv[hs])[:, :, None],
                **consts,
            }
        )

    res = run_bass_kernel_spmd(nc, in_maps, core_ids=list(range(NCORES)))
    # outT per core: [HPC, D, T] -> out[t, 0, (2*core+h)*D + d]
    arr = np.stack([res.results[i]["outT"] for i in range(NCORES)])  # [8, 2, D, T]
    out = arr.transpose(3, 0, 1, 2).reshape(T, 1, H * D)
    return np.ascontiguousarray(out.astype(np.float32))



# revision 48
# speedup vs baseline: 1.5006x; 1.5006x over previous
"""Trainium2 Bass kernel for CoreAttentionExpand (sparse local+global attention).

Sharding: tensor-parallel over heads. 16 heads / 8 cores = 2 heads per core.
Each core computes RoPE + local-block attention + pooled-global attention for
its 2 heads end-to-end (no collectives); host reassembles the full output.

v3 design (all-bf16 dataflow, DMA-latency-optimized):
- Host supplies packed bf16 inputs: per (tensor, half) a [D, raw|rot] pair
  (rot = [-x2; x1]) so one DMA feeds one RoPE chunk; cos|sin packed the same
  way; v pre-swizzled token-major so its DMA is fully contiguous; all mask /
  identity constants in one blob DMA. DMA issue order is chosen so head-0
  K/Q land first and the PE can start at ~10us.
- RoPE is 3 bf16 DVE passes per 2048-column half (rot*sin, raw*cos, add).
- Scores are computed transposed (S^T = K @ Q^T) in bf16 (1 PE cycle/column
  at any width). Causal/history/global masks accumulate -1e4 ramp matmuls
  into the score PSUM before exp; exp underflows them to 0.
- Global branch processes chunk pairs (2p, 2p+1) in one [rows,1024] PSUM
  tile with a 4-row cumulative column mask, halving ACT instruction count.
- Normalization (out/sum divides + branch add) runs on GpSimd for head 0
  and DVE for head 1 so it never queues behind head-1's RoPE.
"""

import sys
import math

_REPO = "/opt/trn_rl_repo"
if _REPO not in sys.path:
    sys.path.insert(0, _REPO)

import numpy as np
import ml_dtypes

# ---------------------------------------------------------------- constants
H = 16          # heads
D = 128         # head dim
T = 4096        # tokens
L = 1024        # local block size
S = 128         # global pool stride
E = 128         # local history size
PNUM = T // L   # 4 local blocks
KLEN = T // S + 1  # 33 global keys (incl. zero token)
NCORES = 8
HPC = H // NCORES  # 2 heads per core
NEGBIG = -10000.0
SCALE = 1.0 / math.sqrt(D)
CHUNK = 512     # q-chunk width
NT = T // 128   # 32 token-tiles per head
HALF = 2048     # RoPE chunk width

# const-blob column offsets (bf16 blob)
B_MB = 0
B_MC = 128
B_ONES = 256
B_POOL = 384
B_GB = 385
B_GC = B_GB + 8 * KLEN          # 649
B_ZK = B_GC + CHUNK             # 1161
B_W = B_ZK + HPC                # 1163

_CACHE = {}


def _apply_framework_patches(bassmod, mybir, tilemod):
    """This walrus build rejects >1 sem wait per instruction; split excess
    waits onto preceding same-engine NoOps (pure scheduling transform)."""
    if getattr(tilemod.TileContext, "_wait_split_patched", False):
        return
    TileContext = tilemod.TileContext
    ScopedClock = tilemod.ScopedClock

    orig_add = TileContext._add_instruction
    ctr = [0]

    def split_add(self, inst):
        si = inst.sync_info
        if si is not None and si.on_wait and len(si.on_wait) > 1:
            ow = list(si.on_wait)
            for w in ow[:-1]:
                ctr[0] += 1
                nop = mybir.InstNoOp(name=f"I-wsplit{ctr[0]}", engine=inst.engine)
                nop.sync_info = mybir.SyncInfo(on_wait=[w], on_update=[])
                orig_add(self, nop)
            si.on_wait = [ow[-1]]
        orig_add(self, inst)

    def drain_and_barrier(self, tick_clock, wait_clock):
        nc = self.nc
        probe = nc.sync.nop(nofuse=True, hint="waitprobe")
        wait_clock.add_sem_waits(
            probe.ins, ScopedClock({None: tick_clock.global_clock})
        )
        si = probe.ins.sync_info
        ow = list(si.on_wait) if si and si.on_wait else []
        if len(ow) > 1:
            si.on_wait = ow[:1]
            for w in ow[1:]:
                n2 = nc.sync.nop(nofuse=True, hint="waitsplit")
                n2.ins.sync_info = mybir.SyncInfo(on_wait=[w], on_update=[])
        nc.sync.drain()
        nc.all_engine_barrier()
        popped = nc._tile_sem_poison_stack.pop()
        assert popped is self._sem_poison
        nc.clear_and_free_semaphores(list(self.sems.allocated().values()))
        nc.all_engine_barrier()

    TileContext._add_instruction = split_add
    TileContext._drain_and_barrier = drain_and_barrier
    TileContext._wait_split_patched = True


# ---------------------------------------------------------------- constants (host)
def _host_constants():
    bf = ml_dtypes.bfloat16
    t = np.arange(T, dtype=np.float32)
    inv = (1.0 / (10000.0 ** (np.arange(0, D, 2, dtype=np.float32) / D))).astype(
        np.float32
    )  # [64]
    emb = t[:, None] * inv[None, :]          # [T, 64]
    cos64 = np.cos(emb).astype(np.float32)
    sin64 = np.sin(emb).astype(np.float32)
    # [D, T] head-dim-major tables (plain sin; rotation sign lives in rot)
    cosT = np.concatenate([cos64, cos64], axis=1).T
    sinT = np.concatenate([sin64, sin64], axis=1).T
    # sign-folded sin: rows 0:64 negated, so the rotate-half multiply needs
    # only a partition-swapped read of the raw tensor (no negation anywhere).
    sinN = sinT.copy()
    sinN[0 : D // 2] *= -1.0
    # packed tables: tpk[half] = cos_half | sinN_half  -> [2, D, 2*HALF]
    tpk = np.empty((2, D, 2 * HALF), dtype=np.float32)
    for half in (0, 1):
        cs = slice(half * HALF, (half + 1) * HALF)
        tpk[half, :, 0:HALF] = cosT[:, cs]
        tpk[half, :, HALF:] = sinN[:, cs]
    tpk = tpk.astype(bf)

    idx = np.arange(128)
    GMROWS = 4
    gB = np.zeros((GMROWS, 8 * KLEN), dtype=np.float32)
    for c in range(8):
        for mm in range(GMROWS):
            for j in range(KLEN):
                gB[mm, KLEN * c + j] = 1.0 if j >= 4 * c + mm + 1 else 0.0
    qq = np.arange(CHUNK)
    gC = np.stack(
        [NEGBIG * ((qq >= 128 * mm) & (qq < 128 * (mm + 1))) for mm in range(GMROWS)]
    )

    blob = np.zeros((128, B_W), dtype=np.float32)
    # causal ramp: (mB^T mC)[k, q] = NEGBIG * max(k - q, 0)
    blob[:, B_MB : B_MB + 128] = idx[:, None] <= idx[None, :]     # mB [m,k]: m<=k
    blob[:, B_MC : B_MC + 128] = NEGBIG * (idx[:, None] > idx[None, :])
    blob[:, B_ONES : B_ONES + 128] = 1.0
    blob[:, B_POOL] = 1.0 / S
    blob[0:GMROWS, B_GB : B_GB + 8 * KLEN] = gB
    blob[0:GMROWS, B_GC : B_GC + CHUNK] = gC
    # zk filled per-core in kernel()
    blob = blob.astype(bf)

    fblob = np.zeros((128, 128 + HPC), dtype=np.float32)
    fblob[:, 0:128] = np.eye(128, dtype=np.float32)
    # zv filled per-core in kernel()
    return {"tpk": tpk, "blob": blob, "fblob": fblob}


# ---------------------------------------------------------------- device program
def _build_program():
    import concourse.bass as bass
    import concourse.mybir as mybir
    import concourse.tile as tile

    _apply_framework_patches(bass, mybir, tile)

    f32 = mybir.dt.float32
    bf16 = mybir.dt.bfloat16
    EXP = mybir.ActivationFunctionType.Exp
    MUL = mybir.AluOpType.mult
    ADD = mybir.AluOpType.add
    DIV = mybir.AluOpType.divide

    nc = bass.Bass()
    qpk_d = nc.dram_tensor("qpk", [HPC, 2, D, 2 * HALF], bf16, kind="ExternalInput")
    kpk_d = nc.dram_tensor("kpk", [HPC, 2, D, 2 * HALF], bf16, kind="ExternalInput")
    vpk_d = nc.dram_tensor("vpk", [HPC, D, NT * 128], bf16, kind="ExternalInput")
    tpk_d = nc.dram_tensor("tpk", [2, D, 2 * HALF], bf16, kind="ExternalInput")
    blob_d = nc.dram_tensor("blob", [128, B_W], bf16, kind="ExternalInput")
    fblob_d = nc.dram_tensor("fblob", [128, 128 + HPC], f32, kind="ExternalInput")
    out_d = nc.dram_tensor("outT", [HPC, D, T], bf16, kind="ExternalOutput")

    with tile.TileContext(nc) as tc:
        with (
            tc.tile_pool(name="persist", bufs=1) as persist,
            tc.tile_pool(name="pk", bufs=6) as pkp,
            tc.tile_pool(name="expp", bufs=18) as expp,
            tc.tile_pool(name="small", bufs=2) as small,
            tc.tile_pool(name="combine", bufs=4) as combine,
            tc.tile_pool(name="scores", bufs=3, space="PSUM") as scores_p,
            tc.tile_pool(name="acc", bufs=1, space="PSUM") as acc_p,
        ):
            QT, KT, VBF, KGT, VG = {}, {}, {}, {}, {}
            tabs = {}

            def make_tab(half):
                tab = persist.tile([D, 2 * HALF], bf16, tag=f"tab{half}",
                                   name=f"tab{half}")
                tabs[half] = tab
                return tab

            def dma_tab(half, part, qtr=None):
                """DMA one piece of a cos|sin table. part: 'cos'|'sin';
                qtr None = whole 2048-col part, else 1024-col quarter."""
                base = 0 if part == "cos" else HALF
                cs = (
                    slice(base, base + HALF)
                    if qtr is None
                    else slice(base + qtr * 1024, base + (qtr + 1) * 1024)
                )
                nc.sync.dma_start(out=tabs[half][:, cs], in_=tpk_d.ap()[half][:, cs])

            def dma_rope_q(h, src_d, half, qtr):
                """Load raw+swap quarters ([D,1024] each) into one pk tile."""
                pk = pkp.tile([D, 2048], bf16, tag="pkq", bufs=4)
                qs = slice(qtr * 1024, (qtr + 1) * 1024)
                rs = slice(HALF + qtr * 1024, HALF + (qtr + 1) * 1024)
                nc.sync.dma_start(out=pk[:, 0:1024], in_=src_d.ap()[h, half][:, qs])
                nc.sync.dma_start(out=pk[:, 1024:2048], in_=src_d.ap()[h, half][:, rs])
                return pk

            def rope_q(pk, dst, half, qtr):
                """RoPE one 1024-col quarter: swap*sin + add on DVE, raw*cos
                on GpSimd (runs in parallel, keeps the DVE queue short)."""
                tab = tabs[half]
                co = qtr * 1024
                t1 = pkp.tile([D, 1024], bf16, tag="t1q", bufs=2)
                t2 = pkp.tile([D, 1024], bf16, tag="t2q", bufs=2)
                nc.vector.tensor_tensor(
                    out=t1, in0=pk[:, 1024:2048], in1=tab[:, HALF + co : HALF + co + 1024], op=MUL
                )
                nc.gpsimd.tensor_tensor(
                    out=t2, in0=pk[:, 0:1024], in1=tab[:, co : co + 1024], op=MUL
                )
                dc = half * HALF + co
                nc.vector.tensor_tensor(
                    out=dst[:, dc : dc + 1024], in0=t1, in1=t2, op=ADD
                )

            def emit_rope(h, src_d, dst, half):
                """RoPE one 2048-col half of one [D, T] tensor: 3 DVE passes
                (raw | plain-swapped pair loaded in one DMA)."""
                cs = slice(half * HALF, (half + 1) * HALF)
                pk = pkp.tile([D, 2 * HALF], bf16, tag="pk", bufs=2)
                nc.sync.dma_start(out=pk, in_=src_d.ap()[h, half])
                tab = tabs[half]
                t1 = pkp.tile([D, HALF], bf16, tag="t1", bufs=2)
                t2 = pkp.tile([D, HALF], bf16, tag="t2", bufs=2)
                nc.vector.tensor_tensor(
                    out=t1, in0=pk[:, HALF:], in1=tab[:, HALF:], op=MUL
                )
                nc.gpsimd.tensor_tensor(
                    out=t2, in0=pk[:, 0:HALF], in1=tab[:, 0:HALF], op=MUL
                )
                nc.vector.tensor_tensor(out=dst[:, cs], in0=t1, in1=t2, op=ADD)

            def emit_consts():
                blob = persist.tile([128, B_W], bf16, tag="blob")
                fblob = persist.tile([128, 128 + HPC], f32, tag="fblob")
                nc.sync.dma_start(out=blob, in_=blob_d.ap())
                nc.sync.dma_start(out=fblob, in_=fblob_d.ap())
                return blob, fblob

            def emit_kg(h):
                """Pooled global K for head h (after full KT RoPE)."""
                kgT = persist.tile([D, KLEN], bf16, tag=f"kgT{h}", name=f"kgT{h}")
                KGT[h] = kgT
                nc.vector.tensor_copy(out=kgT[:, 0:1], in_=blob[:, B_ZK + h : B_ZK + h + 1])
                kgf = small.tile([D, KLEN], f32, tag="kgf")
                nc.vector.tensor_reduce(
                    out=kgf[:, 1:KLEN],
                    in_=KT[h].rearrange("p (g s) -> p g s", s=S),
                    axis=mybir.AxisListType.X,
                    op=ADD,
                )
                with nc.allow_low_precision(reason="bf16 pooled keys"):
                    nc.vector.tensor_scalar_mul(
                        out=kgT[:, 1:KLEN], in0=kgf[:, 1:KLEN], scalar1=1.0 / S
                    )

            def make_vbf(h):
                VBF[h] = persist.tile([128, NT, D], bf16, tag=f"vbf{h}", name=f"vbf{h}")

            def dma_v(h, qtr=None):
                flat = VBF[h].rearrange("p n d -> p (n d)")
                if qtr is None:
                    nc.sync.dma_start(out=flat, in_=vpk_d.ap()[h])
                else:
                    cs = slice(qtr * 1024, (qtr + 1) * 1024)
                    nc.sync.dma_start(out=flat[:, cs], in_=vpk_d.ap()[h][:, cs])

            def emit_v_pool(h):
                """Pooled global V via PE: vgTp[:, g+1] = V_g^T @ (1/S)."""
                vgTp = scores_p.tile([128, 1024], f32, tag="s", name="vgTp")
                for g in range(NT):
                    nc.tensor.matmul(
                        out=vgTp[:, g + 1 : g + 2],
                        lhsT=VBF[h][:, g, :],
                        rhs=blob[:, B_POOL : B_POOL + 1],
                        start=(g == 0),
                        stop=(g == NT - 1),
                    )
                # copies on ACT so they never queue behind DVE RoPE/reduces
                vgT = small.tile([D, KLEN], f32, tag="vgT")
                nc.scalar.copy(out=vgT[:, 0:1], in_=fblob[:, 128 + h : 129 + h])
                nc.scalar.copy(out=vgT[:, 1:KLEN], in_=vgTp[:, 1:KLEN])
                # transpose -> Vg token-major [KLEN, D] bf16
                vgp = scores_p.tile([128, 1024], f32, tag="s", name="vgp")
                nc.tensor.transpose(
                    out=vgp[0:KLEN, 0:128], in_=vgT, identity=fblob[:, 0:128]
                )
                Vg = persist.tile([KLEN, 128], bf16, tag=f"Vg{h}", name=f"Vg{h}")
                nc.scalar.copy(out=Vg, in_=vgp[0:KLEN, 0:128])
                VG[h] = Vg

            ACC = {}  # (h, p) -> dict of live PSUM/SBUF tiles for deferred stages

            def emit_local(h, p):
                """Local block p for head h: scores, exp, sums, PV."""
                mB = blob[:, B_MB : B_MB + 128]
                mC = blob[:, B_MC : B_MC + 128]
                ones_bf = blob[:, B_ONES : B_ONES + 128]

                q0 = p * L
                ms = list(range(1, 9)) if p == 0 else list(range(0, 9))
                expt = {}
                sums = acc_p.tile([128, 1024], f32, tag="sum", name="sums")
                sum_started = [False, False]
                sum_last_m = {
                    reg: max(
                        m
                        for m in ms
                        if (0 if m == 0 else 128 * (m - 1)) < 512 * (reg + 1)
                    )
                    for reg in (0, 1)
                }
                for m in ms:
                    start_m = 0 if m == 0 else 128 * (m - 1)
                    kcol = q0 - 128 + 128 * m  # k-token start (abs)
                    st = scores_p.tile([128, 1024], f32, tag="s", name="st")
                    # QK^T: per <=512-col PSUM region
                    for r0 in range(start_m - start_m % 512, 1024, 512):
                        c_lo = max(start_m, r0)
                        c_hi = r0 + 512
                        is_diag_reg = m >= 1 and start_m >= r0
                        nc.tensor.matmul(
                            out=st[:, c_lo:c_hi],
                            lhsT=KT[h][:, kcol : kcol + 128],
                            rhs=QT[h][:, q0 + c_lo : q0 + c_hi],
                            start=True,
                            stop=not is_diag_reg,
                        )
                        if is_diag_reg:
                            nc.tensor.matmul(
                                out=st[:, start_m : start_m + 128],
                                lhsT=mB,
                                rhs=mC,
                                start=False,
                                stop=True,
                            )
                    et = expp.tile([128, 1024], bf16, tag="e", name="et")
                    nc.scalar.activation(
                        out=et[:, start_m:1024],
                        in_=st[:, start_m:1024],
                        func=EXP,
                        scale=SCALE,
                    )
                    expt[m] = et
                    # accumulate column sums: sums[:, c] += sum_k et[k, c]
                    for reg in (0, 1):
                        c_lo = max(start_m, reg * 512)
                        c_hi = (reg + 1) * 512
                        if c_lo >= c_hi:
                            continue
                        nc.tensor.matmul(
                            out=sums[:, c_lo:c_hi],
                            lhsT=ones_bf,
                            rhs=et[:, c_lo:c_hi],
                            start=not sum_started[reg],
                            stop=(m == sum_last_m[reg]),
                        )
                        sum_started[reg] = True
                # Normalize via recip+mult (DVE divide is not in the ISA and
                # a DVE op may read at most one PSUM operand). The recip also
                # frees the bufs=1 sums accumulator for the next block.
                rl = combine.tile([128, 1024], bf16, tag="rl", bufs=3, name="rl")
                with nc.allow_low_precision(reason="probs sum to 1"):
                    nc.vector.reciprocal(out=rl, in_=sums)
                # PV: O^T accumulation per 512-col region. ot lives in the
                # scores pool: only held through the PV phase (freed by the
                # tl multiply), freeing two PSUM banks for a third score buf.
                ot = scores_p.tile([128, 1024], f32, tag="s", name="ot")
                tl = combine.tile([128, 1024], bf16, tag="tl", name="tl")
                for reg in (0, 1):
                    valid_ms = [
                        m
                        for m in ms
                        if (0 if m == 0 else 128 * (m - 1)) < 512 * (reg + 1)
                    ]
                    for i, m in enumerate(valid_ms):
                        start_m = 0 if m == 0 else 128 * (m - 1)
                        c_lo = max(start_m, reg * 512)
                        c_hi = (reg + 1) * 512
                        vidx = 8 * p - 1 + m
                        nc.tensor.matmul(
                            out=ot[:, c_lo:c_hi],
                            lhsT=VBF[h][:, vidx, :],
                            rhs=expt[m][:, c_lo:c_hi],
                            start=(i == 0),
                            stop=(m == valid_ms[-1]),
                        )
                with nc.allow_low_precision(reason="probs sum to 1"):
                    nc.vector.tensor_tensor(out=tl, in0=ot, in1=rl, op=MUL)
                ACC.setdefault((h, p), {})["tl"] = tl

            def emit_global_scores(h, p):
                """Global chunk pair (2p, 2p+1): scores+mask+exp."""
                rows = min(KLEN, 8 * p + 8)
                sg = scores_p.tile([128, 1024], f32, tag="s", name="sg")
                for ci, c in enumerate((2 * p, 2 * p + 1)):
                    cols = slice(ci * CHUNK, (ci + 1) * CHUNK)
                    nc.tensor.matmul(
                        out=sg[0:rows, cols],
                        lhsT=KGT[h][:, 0:rows],
                        rhs=QT[h][:, c * CHUNK : (c + 1) * CHUNK],
                        start=True,
                        stop=False,
                    )
                    nc.tensor.matmul(
                        out=sg[0:rows, cols],
                        lhsT=blob[0:4, B_GB + KLEN * c : B_GB + KLEN * c + rows],
                        rhs=blob[0:4, B_GC : B_GC + CHUNK],
                        start=False,
                        stop=True,
                    )
                eg = expp.tile([128, 1024], bf16, tag="e", name="eg")
                nc.scalar.activation(
                    out=eg[0:rows, :], in_=sg[0:rows, :], func=EXP, scale=SCALE
                )
                a = ACC.setdefault((h, p), {})
                a["eg"] = eg
                a["rows"] = rows

            def emit_global_pv(h, p, split=False):
                """Global pair sums + PV matmuls."""
                ones_bf = blob[:, B_ONES : B_ONES + 128]
                eg, rows = ACC[(h, p)]["eg"], ACC[(h, p)]["rows"]
                gs = scores_p.tile([128, 1024], f32, tag="s", name="gs")
                go = scores_p.tile([128, 1024], f32, tag="s", name="go")
                for reg in (0, 1):
                    cols = slice(reg * CHUNK, (reg + 1) * CHUNK)
                    nc.tensor.matmul(
                        out=gs[:, cols],
                        lhsT=ones_bf[0:rows, :],
                        rhs=eg[0:rows, cols],
                        start=True,
                        stop=True,
                    )
                    nc.tensor.matmul(
                        out=go[:, cols],
                        lhsT=VG[h][0:rows, :],
                        rhs=eg[0:rows, cols],
                        start=True,
                        stop=True,
                    )
                # global normalize: recip (one PSUM input) then mult on DVE
                rg = combine.tile([128, 1024], bf16, tag="rg", bufs=3, name="rg")
                ogn = combine.tile([128, 1024], bf16, tag="ogn", name="ogn")
                halves = (
                    (slice(0, 512), slice(512, 1024)) if split else (slice(0, 1024),)
                )
                with nc.allow_low_precision(reason="normalized probs sum to 1"):
                    for cs in halves:
                        nc.vector.reciprocal(out=rg[:, cs], in_=gs[:, cs])
                        nc.vector.tensor_tensor(
                            out=ogn[:, cs], in0=go[:, cs], in1=rg[:, cs], op=MUL
                        )
                ACC[(h, p)]["ogn"] = ogn

            def emit_combine(h, p, split=False):
                """Branch add (SBUF-only, GpSimd) + write out. With split,
                halves go to GpSimd and DVE in parallel (tail shortening)."""
                a = ACC.pop((h, p))
                tl, ogn = a["tl"], a["ogn"]
                fin = combine.tile([128, 1024], bf16, tag="fin", name="fin")
                if split:
                    engs = [
                        (nc.gpsimd, slice(0, 512)),
                        (nc.vector, slice(512, 1024)),
                    ]
                else:
                    engs = [(nc.gpsimd, slice(0, 1024))]
                with nc.allow_low_precision(reason="normalized probs sum to 1"):
                    for add_eng, cs in engs:
                        add_eng.tensor_tensor(
                            out=fin[:, cs], in0=tl[:, cs], in1=ogn[:, cs], op=ADD
                        )
                if split:
                    for cs in (slice(0, 512), slice(512, 1024)):
                        nc.sync.dma_start(
                            out=out_d.ap()[h, :, p * L + cs.start : p * L + cs.stop],
                            in_=fin[:, cs],
                        )
                else:
                    nc.sync.dma_start(
                        out=out_d.ap()[h, :, p * L : (p + 1) * L], in_=fin
                    )

            # ---- emission order tuned for DMA latency + engine overlap ----
            QT[0] = persist.tile([D, T], bf16, tag="QT0", name="QT0")
            KT[0] = persist.tile([D, T], bf16, tag="KT0", name="KT0")
            QT[1] = persist.tile([D, T], bf16, tag="QT1", name="QT1")
            KT[1] = persist.tile([D, T], bf16, tag="KT1", name="KT1")
            make_tab(0)
            make_tab(1)
            make_vbf(0)
            make_vbf(1)
            # Startup DMA chain (single serialized DMA resource): head-0
            # K/Q land in 1024-col quarters interleaved with exactly the
            # table pieces each RoPE pass needs, so the PE's first scores
            # start at ~9.5us; V arrives in quarters just ahead of each
            # block's PV; head-1 streams in while PE chews head 0.
            dma_tab(0, "sin", 0)
            ka = dma_rope_q(0, kpk_d, 0, 0)
            dma_tab(0, "cos", 0)
            qa = dma_rope_q(0, qpk_d, 0, 0)
            blob, fblob = emit_consts()
            dma_tab(0, "sin", 1)
            kb = dma_rope_q(0, kpk_d, 0, 1)
            dma_tab(0, "cos", 1)
            qb = dma_rope_q(0, qpk_d, 0, 1)
            dma_v(0, 0)
            dma_tab(1, "sin")
            kc = dma_rope_q(0, kpk_d, 1, 0)
            dma_tab(1, "cos")
            qc = dma_rope_q(0, qpk_d, 1, 0)
            dma_v(0, 1)
            kd = dma_rope_q(0, kpk_d, 1, 1)
            qd = dma_rope_q(0, qpk_d, 1, 1)
            dma_v(0, 2)
            dma_v(0, 3)
            # DVE RoPE stream (in-order queue): head-0 quarters first; the
            # head-1 halves + kg reduces interleave into the block loop so
            # the per-block tl/ogn divides never queue behind them.
            rope_q(ka, KT[0], 0, 0)
            rope_q(qa, QT[0], 0, 0)
            rope_q(kb, KT[0], 0, 1)
            rope_q(qb, QT[0], 0, 1)
            rope_q(kc, KT[0], 1, 0)
            rope_q(qc, QT[0], 1, 0)
            rope_q(kd, KT[0], 1, 1)
            rope_q(qd, QT[0], 1, 1)
            dma_v(1)
            # Block pipeline: globals spread so sg(j) lands once kgT is
            # ready and eg exps overlap locals; gs/go + combine trail so
            # score-buffer recycling never waits on the combine divides.
            seq = [(0, p) for p in range(PNUM)] + [(1, p) for p in range(PNUM)]
            n = len(seq)
            scores_at = {2: [0], 3: [1], 4: [2], 5: [3], 6: [4, 5, 6]}
            pv_at = {4: [0], 5: [1], 6: [2, 3], 7: [4, 5, 6]}
            dve_extra = {
                0: [lambda: emit_rope(1, kpk_d, KT[1], 0)],
                1: [lambda: emit_kg(0), lambda: emit_rope(1, qpk_d, QT[1], 0)],
                2: [lambda: emit_rope(1, kpk_d, KT[1], 1)],
                3: [lambda: emit_rope(1, qpk_d, QT[1], 1), lambda: emit_kg(1)],
            }
            for i, (h, p) in enumerate(seq):
                if i == n - 1:
                    emit_global_scores(h, p)  # its exp overlaps the local
                emit_local(h, p)
                if i == 1:
                    emit_v_pool(0)
                if i == 3:
                    emit_v_pool(1)
                for fn in dve_extra.get(i, []):
                    fn()
                for j in scores_at.get(i, []):
                    emit_global_scores(*seq[j])
                for j in pv_at.get(i, []):
                    emit_global_pv(*seq[j], split=(j >= n - 3))
                    emit_combine(*seq[j], split=(j >= n - 3))
            emit_global_pv(*seq[n - 1], split=True)
            emit_combine(*seq[n - 1], split=True)
    return nc


def _get_program():
    if "nc" not in _CACHE:
        _CACHE["nc"] = _build_program()
        _CACHE["consts"] = _host_constants()
    return _CACHE["nc"], _CACHE["consts"]


# ---------------------------------------------------------------- entry point
def kernel(q, k, v, zero_k, zero_v):
    nc, consts = _get_program()
    from concourse.bass_utils import run_bass_kernel_spmd

    bf = ml_dtypes.bfloat16
    q4 = np.asarray(q, dtype=np.float32).reshape(T, H, D)
    k4 = np.asarray(k, dtype=np.float32).reshape(T, H, D)
    v4 = np.asarray(v, dtype=np.float32).reshape(T, H, D)
    zk = np.asarray(zero_k, dtype=np.float32).reshape(H, D)
    zv = np.asarray(zero_v, dtype=np.float32).reshape(H, D)

    def pack_halves(xT):  # [h, D, T] -> [h, half, D, raw|swap]
        # plain partition swap; the sign lives in the sign-folded sin table
        rot = np.concatenate([xT[:, D // 2 :], xT[:, : D // 2]], axis=1)
        pk = np.empty((HPC, 2, D, 2 * HALF), dtype=np.float32)
        for half in (0, 1):
            cs = slice(half * HALF, (half + 1) * HALF)
            pk[:, half, :, 0:HALF] = xT[:, :, cs]
            pk[:, half, :, HALF:] = rot[:, :, cs]
        return pk.astype(bf)

    in_maps = []
    for core in range(NCORES):
        hs = slice(HPC * core, HPC * (core + 1))
        qT = np.ascontiguousarray(q4[:, hs].transpose(1, 2, 0))   # [h, D, T]
        kT = np.ascontiguousarray(k4[:, hs].transpose(1, 2, 0))
        # v token-major: vpk[h, p, n*128+d] = v[n*128+p, head, d]
        vpk = np.ascontiguousarray(
            v4[:, hs].reshape(NT, 128, HPC, D).transpose(2, 1, 0, 3)
        ).reshape(HPC, 128, NT * D)
        blob = consts["blob"].copy()
        blob[:, B_ZK : B_ZK + HPC] = zk[hs].T.astype(bf)
        fblob = consts["fblob"].copy()
        fblob[:, 128 : 128 + HPC] = zv[hs].T
        in_maps.append(
            {
                "qpk": pack_halves(qT),
                "kpk": pack_halves(kT),
                "vpk": vpk.astype(bf),
                "tpk": consts["tpk"],
                "blob": blob,
                "fblob": fblob,
            }
        )

    res = run_bass_kernel_spmd(nc, in_maps, core_ids=list(range(NCORES)))
    # outT per core: [HPC, D, T] -> out[t, 0, (2*core+h)*D + d]
    arr = np.stack(
        [np.asarray(res.results[i]["outT"], dtype=np.float32) for i in range(NCORES)]
    )  # [8, 2, D, T]
    out = arr.transpose(3, 0, 1, 2).reshape(T, 1, H * D)
    return np.ascontiguousarray(out.astype(np.float32))


# revision 82
# speedup vs baseline: 1.6848x; 1.1227x over previous
"""Trainium2 Bass kernel for CoreAttentionExpand (sparse local+global attention).

Sharding: tensor-parallel over heads. 16 heads / 8 cores = 2 heads per core.
Each core computes RoPE + local-block attention + pooled-global attention for
its 2 heads end-to-end (no collectives); host reassembles the full output.

Design (all-bf16 dataflow; sim ~107.7us vs 181.5us for the f32 baseline):
- Host supplies packed bf16 inputs: per (tensor, half) a [D, raw|swap] pair
  (swap = [x2; x1]; the rotate-half sign is folded into the sin table) so
  one DMA feeds one RoPE chunk; cos|sinN packed the same way; v pre-swizzled
  token-major so its DMA is fully contiguous; all mask/identity constants in
  one blob DMA. The startup DMA chain is ordered so head-0 K/Q land in
  1024-col quarters first and the PE starts scoring at ~9.5us; head-1
  streams in (half granularity) while the PE chews head 0.
- RoPE: swap*sinN on DVE, raw*cos on GpSimd (parallel), add on DVE.
- Scores are computed transposed (S^T = K @ Q^T) in bf16 (1 PE cycle/column
  at any width). Causal/history/global masks accumulate -1e4 ramp matmuls
  into the score PSUM before exp; exp underflows them to 0. The narrow
  diagonal m-tiles pack into shared PSUM tiles (A: m4|m7|m8, B: m5|m6), so
  a block needs 6 exps / 6 score buffers instead of 9.
- Per-block software pipeline (the PE queue is in-order): block p+1's dense
  score matmuls are emitted before block p's exp-dependent sums/PV phase,
  so the PE always has ready work while ACT grinds exps. PSUM: 3 rotating
  2-bank score buffers + a 2-bank sums accumulator (ot rides the score
  pool; it only lives through the PV phase).
- Global branch processes chunk pairs (2p, 2p+1) in one [rows,1024] PSUM
  tile with a 4-row cumulative column mask (one exp per pair); its
  sums/PV lag two blocks so kgT (pooled K) is ready and eg exps overlap.
- Normalization is recip+mult on DVE (no DVE divide in the ISA; at most
  one PSUM operand per op; GpSimd cannot touch PSUM), final branch add on
  GpSimd, output DMA'd as bf16 and widened on the host.
"""

import sys
import math

_REPO = "/opt/trn_rl_repo"
if _REPO not in sys.path:
    sys.path.insert(0, _REPO)

import numpy as np
import ml_dtypes

# ---------------------------------------------------------------- constants
H = 16          # heads
D = 128         # head dim
T = 4096        # tokens
L = 1024        # local block size
S = 128         # global pool stride
E = 128         # local history size
PNUM = T // L   # 4 local blocks
KLEN = T // S + 1  # 33 global keys (incl. zero token)
NCORES = 8
HPC = H // NCORES  # 2 heads per core
NEGBIG = -10000.0
SCALE = 1.0 / math.sqrt(D)
CHUNK = 512     # q-chunk width
NT = T // 128   # 32 token-tiles per head
HALF = 2048     # RoPE chunk width

# const-blob column offsets (bf16 blob)
B_MB = 0
B_MC = 128
B_ONES = 256
B_POOL = 384
B_GB = 385
B_GC = B_GB + 8 * KLEN          # 649
B_ZK = B_GC + CHUNK             # 1161
B_W = B_ZK + HPC                # 1163

_CACHE = {}


def _apply_framework_patches(bassmod, mybir, tilemod):
    """This walrus build rejects >1 sem wait per instruction; split excess
    waits onto preceding same-engine NoOps (pure scheduling transform)."""
    if getattr(tilemod.TileContext, "_wait_split_patched", False):
        return
    TileContext = tilemod.TileContext
    ScopedClock = tilemod.ScopedClock

    orig_add = TileContext._add_instruction
    ctr = [0]

    def split_add(self, inst):
        si = inst.sync_info
        if si is not None and si.on_wait and len(si.on_wait) > 1:
            ow = list(si.on_wait)
            for w in ow[:-1]:
                ctr[0] += 1
                nop = mybir.InstNoOp(name=f"I-wsplit{ctr[0]}", engine=inst.engine)
                nop.sync_info = mybir.SyncInfo(on_wait=[w], on_update=[])
                orig_add(self, nop)
            si.on_wait = [ow[-1]]
        orig_add(self, inst)

    def drain_and_barrier(self, tick_clock, wait_clock):
        nc = self.nc
        probe = nc.sync.nop(nofuse=True, hint="waitprobe")
        wait_clock.add_sem_waits(
            probe.ins, ScopedClock({None: tick_clock.global_clock})
        )
        si = probe.ins.sync_info
        ow = list(si.on_wait) if si and si.on_wait else []
        if len(ow) > 1:
            si.on_wait = ow[:1]
            for w in ow[1:]:
                n2 = nc.sync.nop(nofuse=True, hint="waitsplit")
                n2.ins.sync_info = mybir.SyncInfo(on_wait=[w], on_update=[])
        nc.sync.drain()
        nc.all_engine_barrier()
        popped = nc._tile_sem_poison_stack.pop()
        assert popped is self._sem_poison
        nc.clear_and_free_semaphores(list(self.sems.allocated().values()))
        nc.all_engine_barrier()

    TileContext._add_instruction = split_add
    TileContext._drain_and_barrier = drain_and_barrier
    TileContext._wait_split_patched = True


# ---------------------------------------------------------------- constants (host)
def _host_constants():
    bf = ml_dtypes.bfloat16
    t = np.arange(T, dtype=np.float32)
    inv = (1.0 / (10000.0 ** (np.arange(0, D, 2, dtype=np.float32) / D))).astype(
        np.float32
    )  # [64]
    emb = t[:, None] * inv[None, :]          # [T, 64]
    cos64 = np.cos(emb).astype(np.float32)
    sin64 = np.sin(emb).astype(np.float32)
    # [D, T] head-dim-major tables (plain sin; rotation sign lives in rot)
    cosT = np.concatenate([cos64, cos64], axis=1).T
    sinT = np.concatenate([sin64, sin64], axis=1).T
    # sign-folded sin: rows 0:64 negated, so the rotate-half multiply needs
    # only a partition-swapped read of the raw tensor (no negation anywhere).
    sinN = sinT.copy()
    sinN[0 : D // 2] *= -1.0
    # packed tables: tpk[half] = cos_half | sinN_half  -> [2, D, 2*HALF]
    tpk = np.empty((2, D, 2 * HALF), dtype=np.float32)
    for half in (0, 1):
        cs = slice(half * HALF, (half + 1) * HALF)
        tpk[half, :, 0:HALF] = cosT[:, cs]
        tpk[half, :, HALF:] = sinN[:, cs]
    tpk = tpk.astype(bf)

    idx = np.arange(128)
    GMROWS = 4
    gB = np.zeros((GMROWS, 8 * KLEN), dtype=np.float32)
    for c in range(8):
        for mm in range(GMROWS):
            for j in range(KLEN):
                gB[mm, KLEN * c + j] = 1.0 if j >= 4 * c + mm + 1 else 0.0
    qq = np.arange(CHUNK)
    gC = np.stack(
        [NEGBIG * ((qq >= 128 * mm) & (qq < 128 * (mm + 1))) for mm in range(GMROWS)]
    )

    blob = np.zeros((128, B_W), dtype=np.float32)
    # causal ramp: (mB^T mC)[k, q] = NEGBIG * max(k - q, 0)
    blob[:, B_MB : B_MB + 128] = idx[:, None] <= idx[None, :]     # mB [m,k]: m<=k
    blob[:, B_MC : B_MC + 128] = NEGBIG * (idx[:, None] > idx[None, :])
    blob[:, B_ONES : B_ONES + 128] = 1.0
    blob[:, B_POOL] = 1.0 / S
    blob[0:GMROWS, B_GB : B_GB + 8 * KLEN] = gB
    blob[0:GMROWS, B_GC : B_GC + CHUNK] = gC
    # zk filled per-core in kernel()
    blob = blob.astype(bf)

    fblob = np.zeros((128, 128 + HPC), dtype=np.float32)
    fblob[:, 0:128] = np.eye(128, dtype=np.float32)
    # zv filled per-core in kernel()
    return {"tpk": tpk, "blob": blob, "fblob": fblob}


# ---------------------------------------------------------------- device program
def _build_program():
    import concourse.bass as bass
    import concourse.mybir as mybir
    import concourse.tile as tile

    _apply_framework_patches(bass, mybir, tile)

    f32 = mybir.dt.float32
    bf16 = mybir.dt.bfloat16
    EXP = mybir.ActivationFunctionType.Exp
    MUL = mybir.AluOpType.mult
    ADD = mybir.AluOpType.add
    DIV = mybir.AluOpType.divide

    nc = bass.Bass()
    qpk_d = nc.dram_tensor("qpk", [HPC, 2, D, 2 * HALF], bf16, kind="ExternalInput")
    kpk_d = nc.dram_tensor("kpk", [HPC, 2, D, 2 * HALF], bf16, kind="ExternalInput")
    vpk_d = nc.dram_tensor("vpk", [HPC, D, NT * 128], bf16, kind="ExternalInput")
    tpk_d = nc.dram_tensor("tpk", [2, D, 2 * HALF], bf16, kind="ExternalInput")
    blob_d = nc.dram_tensor("blob", [128, B_W], bf16, kind="ExternalInput")
    fblob_d = nc.dram_tensor("fblob", [128, 128 + HPC], f32, kind="ExternalInput")
    out_d = nc.dram_tensor("outT", [HPC, D, T], bf16, kind="ExternalOutput")

    with tile.TileContext(nc) as tc:
        with (
            tc.tile_pool(name="persist", bufs=1) as persist,
            tc.tile_pool(name="pk", bufs=6) as pkp,
            tc.tile_pool(name="expp", bufs=22) as expp,
            tc.tile_pool(name="small", bufs=2) as small,
            tc.tile_pool(name="combine", bufs=4) as combine,
            tc.tile_pool(name="scores", bufs=3, space="PSUM") as scores_p,
            tc.tile_pool(name="acc", bufs=1, space="PSUM") as acc_p,
        ):
            QT, KT, VBF, KGT, VG = {}, {}, {}, {}, {}
            tabs = {}

            def make_tab(half):
                tab = persist.tile([D, 2 * HALF], bf16, tag=f"tab{half}",
                                   name=f"tab{half}")
                tabs[half] = tab
                return tab

            def dma_tab(half, part, qtr=None, cols=None):
                """DMA one piece of a cos|sin table. part: 'cos'|'sin';
                qtr None = whole 2048-col part, else 1024-col quarter;
                cols=(lo,hi) overrides with an explicit column range."""
                base = 0 if part == "cos" else HALF
                if cols is not None:
                    cs = slice(base + cols[0], base + cols[1])
                elif qtr is None:
                    cs = slice(base, base + HALF)
                else:
                    cs = slice(base + qtr * 1024, base + (qtr + 1) * 1024)
                nc.sync.dma_start(out=tabs[half][:, cs], in_=tpk_d.ap()[half][:, cs])

            def dma_rope_q(h, src_d, half, qtr):
                """Load raw+swap quarters ([D,1024] each) into one pk tile."""
                pk = pkp.tile([D, 2048], bf16, tag="pkq", bufs=3)
                qs = slice(qtr * 1024, (qtr + 1) * 1024)
                rs = slice(HALF + qtr * 1024, HALF + (qtr + 1) * 1024)
                nc.sync.dma_start(out=pk[:, 0:1024], in_=src_d.ap()[h, half][:, qs])
                nc.sync.dma_start(out=pk[:, 1024:2048], in_=src_d.ap()[h, half][:, rs])
                return pk

            def dma_rope_p(h, src_d, half, p0):
                """Load a raw+swap 512-col piece into one pk tile (first-
                chunk latency: smaller transfers reach the RoPE sooner)."""
                pk = pkp.tile([D, 2048], bf16, tag="pkq", bufs=3)
                nc.sync.dma_start(
                    out=pk[:, 0:512], in_=src_d.ap()[h, half][:, p0 : p0 + 512]
                )
                nc.sync.dma_start(
                    out=pk[:, 512:1024],
                    in_=src_d.ap()[h, half][:, HALF + p0 : HALF + p0 + 512],
                )
                return pk

            def rope_p(pk, dst, half, p0):
                """RoPE one 512-col piece: 3 passes (t2 on DVE too - these
                run before the big quarters, when every ns of latency counts)."""
                tab = tabs[half]
                t1 = pkp.tile([D, 1024], bf16, tag="t1q", bufs=2)
                t2 = pkp.tile([D, 1024], bf16, tag="t2q", bufs=2)
                nc.vector.tensor_tensor(
                    out=t1[:, 0:512],
                    in0=pk[:, 512:1024],
                    in1=tab[:, HALF + p0 : HALF + p0 + 512],
                    op=MUL,
                )
                nc.vector.tensor_tensor(
                    out=t2[:, 0:512], in0=pk[:, 0:512], in1=tab[:, p0 : p0 + 512],
                    op=MUL,
                )
                dc = half * HALF + p0
                nc.vector.tensor_tensor(
                    out=dst[:, dc : dc + 512], in0=t1[:, 0:512], in1=t2[:, 0:512],
                    op=ADD,
                )

            def rope_q(pk, dst, half, qtr):
                """RoPE one 1024-col quarter: swap*sin + add on DVE, raw*cos
                on GpSimd (runs in parallel, keeps the DVE queue short)."""
                tab = tabs[half]
                co = qtr * 1024
                t1 = pkp.tile([D, 1024], bf16, tag="t1q", bufs=2)
                t2 = pkp.tile([D, 1024], bf16, tag="t2q", bufs=2)
                nc.vector.tensor_tensor(
                    out=t1, in0=pk[:, 1024:2048], in1=tab[:, HALF + co : HALF + co + 1024], op=MUL
                )
                nc.gpsimd.tensor_tensor(
                    out=t2, in0=pk[:, 0:1024], in1=tab[:, co : co + 1024], op=MUL
                )
                dc = half * HALF + co
                nc.vector.tensor_tensor(
                    out=dst[:, dc : dc + 1024], in0=t1, in1=t2, op=ADD
                )

            def dma_rope_h(h, src_d, half):
                """Load a packed raw|swap half ([D, 2*HALF]) in one DMA."""
                pk = pkp.tile([D, 2 * HALF], bf16, tag="pk", bufs=2)
                nc.sync.dma_start(out=pk, in_=src_d.ap()[h, half])
                return pk

            def rope_h(pk, dst, half):
                """RoPE one 2048-col half: swap*sin + add on DVE, raw*cos on
                GpSimd (parallel)."""
                cs = slice(half * HALF, (half + 1) * HALF)
                tab = tabs[half]
                t1 = pkp.tile([D, HALF], bf16, tag="t1", bufs=2)
                t2 = pkp.tile([D, HALF], bf16, tag="t2", bufs=2)
                nc.vector.tensor_tensor(
                    out=t1, in0=pk[:, HALF:], in1=tab[:, HALF:], op=MUL
                )
                nc.gpsimd.tensor_tensor(
                    out=t2, in0=pk[:, 0:HALF], in1=tab[:, 0:HALF], op=MUL
                )
                nc.vector.tensor_tensor(out=dst[:, cs], in0=t1, in1=t2, op=ADD)

            def emit_consts():
                blob = persist.tile([128, B_W], bf16, tag="blob")
                fblob = persist.tile([128, 128 + HPC], f32, tag="fblob")
                nc.sync.dma_start(out=blob, in_=blob_d.ap())
                nc.sync.dma_start(out=fblob, in_=fblob_d.ap())
                return blob, fblob

            def emit_kg(h):
                """Pooled global K for head h (after full KT RoPE)."""
                kgT = persist.tile([D, KLEN], bf16, tag=f"kgT{h}", name=f"kgT{h}")
                KGT[h] = kgT
                nc.vector.tensor_copy(out=kgT[:, 0:1], in_=blob[:, B_ZK + h : B_ZK + h + 1])
                kgf = small.tile([D, KLEN], f32, tag="kgf")
                nc.vector.tensor_reduce(
                    out=kgf[:, 1:KLEN],
                    in_=KT[h].rearrange("p (g s) -> p g s", s=S),
                    axis=mybir.AxisListType.X,
                    op=ADD,
                )
                with nc.allow_low_precision(reason="bf16 pooled keys"):
                    nc.vector.tensor_scalar_mul(
                        out=kgT[:, 1:KLEN], in0=kgf[:, 1:KLEN], scalar1=1.0 / S
                    )

            def make_vbf(h):
                VBF[h] = persist.tile([128, NT, D], bf16, tag=f"vbf{h}", name=f"vbf{h}")

            def dma_v(h, qtr=None):
                flat = VBF[h].rearrange("p n d -> p (n d)")
                if qtr is None:
                    nc.sync.dma_start(out=flat, in_=vpk_d.ap()[h])
                else:
                    cs = slice(qtr * 1024, (qtr + 1) * 1024)
                    nc.sync.dma_start(out=flat[:, cs], in_=vpk_d.ap()[h][:, cs])

            def emit_v_pool(h):
                """Pooled global V via PE: vgTp[:, g+1] = V_g^T @ (1/S)."""
                vgTp = scores_p.tile([128, 1024], f32, tag="s", name="vgTp")
                for g in range(NT):
                    nc.tensor.matmul(
                        out=vgTp[:, g + 1 : g + 2],
                        lhsT=VBF[h][:, g, :],
                        rhs=blob[:, B_POOL : B_POOL + 1],
                        start=(g == 0),
                        stop=(g == NT - 1),
                    )
                # copies on ACT so they never queue behind DVE RoPE/reduces
                vgT = small.tile([D, KLEN], f32, tag="vgT")
                nc.scalar.copy(out=vgT[:, 0:1], in_=fblob[:, 128 + h : 129 + h])
                nc.scalar.copy(out=vgT[:, 1:KLEN], in_=vgTp[:, 1:KLEN])
                # transpose -> Vg token-major [KLEN, D] bf16
                vgp = scores_p.tile([128, 1024], f32, tag="s", name="vgp")
                nc.tensor.transpose(
                    out=vgp[0:KLEN, 0:128], in_=vgT, identity=fblob[:, 0:128]
                )
                Vg = persist.tile([KLEN, 128], bf16, tag=f"Vg{h}", name=f"Vg{h}")
                nc.scalar.copy(out=Vg, in_=vgp[0:KLEN, 0:128])
                VG[h] = Vg

            ACC = {}  # (h, p) -> dict of live PSUM/SBUF tiles for deferred stages

            # local m-tile -> (tile_key, packed column offset). m0-m3 are
            # block-aligned in their own tiles; the narrow tails pack into
            # two shared tiles (A: m4|m7|m8, B: m5|m6), cutting exp count
            # and score-buffer churn from 9 to 6 per block.

            LOC = {
                0: (0, 0), 1: (1, 0), 2: (2, 128), 3: (3, 256),
                4: ("A", 0), 7: ("A", 640), 8: ("A", 896),
                5: ("B", 0), 6: ("B", 512),
            }
            TILE_W = {0: 1024, 1: 1024, 2: 1024, 3: 1024, "A": 1024, "B": 896}

            def emit_scores(h, p):
                """Local block p scores + exps (PE then ACT)."""
                mB = blob[:, B_MB : B_MB + 128]
                mC = blob[:, B_MC : B_MC + 128]
                q0 = p * L
                ms = list(range(1, 9)) if p == 0 else list(range(0, 9))
                tiles, expt = {}, {}
                for m in ms:
                    key, poff = LOC[m]
                    if key not in tiles:
                        tiles[key] = scores_p.tile([128, 1024], f32, tag="s",
                                                   name="st")
                    st = tiles[key]
                    start_m = 0 if m == 0 else 128 * (m - 1)
                    width = 1024 - start_m
                    kcol = q0 - 128 + 128 * m  # k-token start (abs)
                    # QK^T into packed cols [poff, poff+width), split at the
                    # 512-col PSUM bank boundaries of the tile
                    for r0 in range(poff - poff % 512, poff + width, 512):
                        c_lo = max(poff, r0)
                        c_hi = min(poff + width, r0 + 512)
                        is_diag_reg = m >= 1 and c_lo == poff
                        nc.tensor.matmul(
                            out=st[:, c_lo:c_hi],
                            lhsT=KT[h][:, kcol : kcol + 128],
                            rhs=QT[h][
                                :,
                                q0 + start_m + (c_lo - poff) : q0
                                + start_m
                                + (c_hi - poff),
                            ],
                            start=True,
                            stop=not is_diag_reg,
                        )
                        if is_diag_reg:
                            nc.tensor.matmul(
                                out=st[:, poff : poff + 128],
                                lhsT=mB,
                                rhs=mC,
                                start=False,
                                stop=True,
                            )
                # one exp per packed tile
                ets = {}
                for key, st in tiles.items():
                    # valid span of each tile
                    if key in (0, 1, 2, 3):
                        lo, hi = LOC[key][1], 1024
                    else:
                        lo, hi = 0, TILE_W[key]
                    et = expp.tile([128, 1024], bf16, tag="e", name="et")
                    nc.scalar.activation(
                        out=et[:, lo:hi], in_=st[:, lo:hi], func=EXP, scale=SCALE
                    )
                    ets[key] = et
                a = ACC.setdefault((h, p), {})
                a["ets"] = ets
                a["ms"] = ms

            def emit_sumspv(h, p):
                """Local block p sums, normalize, PV, tl."""
                ones_bf = blob[:, B_ONES : B_ONES + 128]
                a = ACC[(h, p)]
                ets, ms = a.pop("ets"), a.pop("ms")

                def et_slice(m, c_lo, c_hi):  # block cols -> packed et AP
                    key, poff = LOC[m]
                    start_m = 0 if m == 0 else 128 * (m - 1)
                    return ets[key][
                        :, poff + (c_lo - start_m) : poff + (c_hi - start_m)
                    ]

                sums = acc_p.tile([128, 1024], f32, tag="sum", name="sums")
                sum_started = [False, False]
                sum_last_m = {
                    reg: max(
                        m
                        for m in ms
                        if (0 if m == 0 else 128 * (m - 1)) < 512 * (reg + 1)
                    )
                    for reg in (0, 1)
                }
                for m in ms:
                    start_m = 0 if m == 0 else 128 * (m - 1)
                    for reg in (0, 1):
                        c_lo = max(start_m, reg * 512)
                        c_hi = (reg + 1) * 512
                        if c_lo >= c_hi:
                            continue
                        nc.tensor.matmul(
                            out=sums[:, c_lo:c_hi],
                            lhsT=ones_bf,
                            rhs=et_slice(m, c_lo, c_hi),
                            start=not sum_started[reg],
                            stop=(m == sum_last_m[reg]),
                        )
                        sum_started[reg] = True
                # Normalize via recip+mult (DVE divide is not in the ISA and
                # a DVE op may read at most one PSUM operand). The recip also
                # frees the bufs=1 sums accumulator for the next block.
                rl = combine.tile([128, 1024], bf16, tag="rl", bufs=3, name="rl")
                with nc.allow_low_precision(reason="probs sum to 1"):
                    nc.vector.reciprocal(out=rl, in_=sums)
                # PV: O^T accumulation per 512-col region. ot lives in the
                # scores pool: only held through the PV phase (freed by the
                # tl multiply), freeing two PSUM banks for a third score buf.
                ot = scores_p.tile([128, 1024], f32, tag="s", name="ot")
                tl = combine.tile([128, 1024], bf16, tag="tl", name="tl")
                for reg in (0, 1):
                    valid_ms = [
                        m
                        for m in ms
                        if (0 if m == 0 else 128 * (m - 1)) < 512 * (reg + 1)
                    ]
                    for i, m in enumerate(valid_ms):
                        start_m = 0 if m == 0 else 128 * (m - 1)
                        c_lo = max(start_m, reg * 512)
                        c_hi = (reg + 1) * 512
                        vidx = 8 * p - 1 + m
                        nc.tensor.matmul(
                            out=ot[:, c_lo:c_hi],
                            lhsT=VBF[h][:, vidx, :],
                            rhs=et_slice(m, c_lo, c_hi),
                            start=(i == 0),
                            stop=(m == valid_ms[-1]),
                        )
                with nc.allow_low_precision(reason="probs sum to 1"):
                    nc.vector.tensor_tensor(out=tl, in0=ot, in1=rl, op=MUL)
                ACC[(h, p)]["tl"] = tl

            def emit_global_scores(h, p):
                """Global chunk pair (2p, 2p+1): scores+mask+exp."""
                rows = min(KLEN, 8 * p + 8)
                sg = scores_p.tile([128, 1024], f32, tag="s", name="sg")
                for ci, c in enumerate((2 * p, 2 * p + 1)):
                    cols = slice(ci * CHUNK, (ci + 1) * CHUNK)
                    nc.tensor.matmul(
                        out=sg[0:rows, cols],
                        lhsT=KGT[h][:, 0:rows],
                        rhs=QT[h][:, c * CHUNK : (c + 1) * CHUNK],
                        start=True,
                        stop=False,
                    )
                    nc.tensor.matmul(
                        out=sg[0:rows, cols],
                        lhsT=blob[0:4, B_GB + KLEN * c : B_GB + KLEN * c + rows],
                        rhs=blob[0:4, B_GC : B_GC + CHUNK],
                        start=False,
                        stop=True,
                    )
                eg = expp.tile([128, 1024], bf16, tag="e", name="eg")
                nc.scalar.activation(
                    out=eg[0:rows, :], in_=sg[0:rows, :], func=EXP, scale=SCALE
                )
                a = ACC.setdefault((h, p), {})
                a["eg"] = eg
                a["rows"] = rows

            def emit_global_pv(h, p, split=False):
                """Global pair sums + PV matmuls."""
                ones_bf = blob[:, B_ONES : B_ONES + 128]
                eg, rows = ACC[(h, p)]["eg"], ACC[(h, p)]["rows"]
                gs = scores_p.tile([128, 1024], f32, tag="s", name="gs")
                go = scores_p.tile([128, 1024], f32, tag="s", name="go")
                for reg in (0, 1):
                    cols = slice(reg * CHUNK, (reg + 1) * CHUNK)
                    nc.tensor.matmul(
                        out=gs[:, cols],
                        lhsT=ones_bf[0:rows, :],
                        rhs=eg[0:rows, cols],
                        start=True,
                        stop=True,
                    )
                    nc.tensor.matmul(
                        out=go[:, cols],
                        lhsT=VG[h][0:rows, :],
                        rhs=eg[0:rows, cols],
                        start=True,
                        stop=True,
                    )
                # global normalize: recip (one PSUM input) then mult on DVE
                rg = combine.tile([128, 1024], bf16, tag="rg", bufs=3, name="rg")
                ogn = combine.tile([128, 1024], bf16, tag="ogn", name="ogn")
                halves = (
                    (slice(0, 512), slice(512, 1024)) if split else (slice(0, 1024),)
                )
                with nc.allow_low_precision(reason="normalized probs sum to 1"):
                    for cs in halves:
                        nc.vector.reciprocal(out=rg[:, cs], in_=gs[:, cs])
                        nc.vector.tensor_tensor(
                            out=ogn[:, cs], in0=go[:, cs], in1=rg[:, cs], op=MUL
                        )
                ACC[(h, p)]["ogn"] = ogn

            def emit_combine(h, p, split=False, fast_dma=False):
                """Branch add (SBUF-only, GpSimd) + write out. With split,
                halves go to GpSimd and DVE in parallel (tail shortening)."""
                a = ACC.pop((h, p))
                tl, ogn = a["tl"], a["ogn"]
                fin = combine.tile([128, 1024], bf16, tag="fin", name="fin")
                if split:
                    engs = [
                        (nc.gpsimd, slice(0, 512)),
                        (nc.vector, slice(512, 1024)),
                    ]
                else:
                    engs = [(nc.gpsimd, slice(0, 1024))]
                with nc.allow_low_precision(reason="normalized probs sum to 1"):
                    for add_eng, cs in engs:
                        add_eng.tensor_tensor(
                            out=fin[:, cs], in0=tl[:, cs], in1=ogn[:, cs], op=ADD
                        )
                if split:
                    for qi, cs in enumerate((slice(0, 512), slice(512, 1024))):
                        eng = nc.scalar if (fast_dma and qi == 1) else nc.sync
                        eng.dma_start(
                            out=out_d.ap()[h, :, p * L + cs.start : p * L + cs.stop],
                            in_=fin[:, cs],
                        )
                else:
                    nc.sync.dma_start(
                        out=out_d.ap()[h, :, p * L : (p + 1) * L], in_=fin
                    )

            # ---- emission order tuned for DMA latency + engine overlap ----
            QT[0] = persist.tile([D, T], bf16, tag="QT0", name="QT0")
            KT[0] = persist.tile([D, T], bf16, tag="KT0", name="KT0")
            QT[1] = persist.tile([D, T], bf16, tag="QT1", name="QT1")
            KT[1] = persist.tile([D, T], bf16, tag="KT1", name="KT1")
            make_tab(0)
            make_tab(1)
            make_vbf(0)
            make_vbf(1)
            # Startup DMA chain (single serialized DMA resource): head-0
            # K/Q land in 1024-col quarters interleaved with exactly the
            # table pieces each RoPE pass needs, so the PE's first scores
            # start at ~9.5us; V arrives in quarters just ahead of each
            # block's PV; head-1 streams in while PE chews head 0.
            dma_tab(0, "sin", 0)
            ka = dma_rope_q(0, kpk_d, 0, 0)
            dma_tab(0, "cos", 0)
            qa = dma_rope_q(0, qpk_d, 0, 0)
            blob, fblob = emit_consts()
            dma_tab(0, "sin", 1)
            kb = dma_rope_q(0, kpk_d, 0, 1)
            dma_tab(0, "cos", 1)
            qb = dma_rope_q(0, qpk_d, 0, 1)
            dma_v(0, 0)
            dma_tab(1, "sin")
            kc = dma_rope_q(0, kpk_d, 1, 0)
            dma_tab(1, "cos")
            qc = dma_rope_q(0, qpk_d, 1, 0)
            dma_v(0, 1)
            kd = dma_rope_q(0, kpk_d, 1, 1)
            qd = dma_rope_q(0, qpk_d, 1, 1)
            dma_v(0, 2)
            dma_v(0, 3)
            # DVE RoPE stream (in-order queue): head-0 quarters first; the
            # head-1 halves + kg reduces interleave into the block loop so
            # the per-block tl/ogn divides never queue behind them.
            rope_q(ka, KT[0], 0, 0)
            rope_q(qa, QT[0], 0, 0)
            rope_q(kb, KT[0], 0, 1)
            rope_q(qb, QT[0], 0, 1)
            rope_q(kc, KT[0], 1, 0)
            rope_q(qc, QT[0], 1, 0)
            rope_q(kd, KT[0], 1, 1)
            rope_q(qd, QT[0], 1, 1)
            dma_v(1)
            # Block pipeline: globals spread so sg(j) lands once kgT is
            # ready and eg exps overlap locals; gs/go + combine trail so
            # score-buffer recycling never waits on the combine divides.
            seq = [(0, p) for p in range(PNUM)] + [(1, p) for p in range(PNUM)]
            n = len(seq)
            scores_at = {2: [0], 3: [1], 4: [2], 5: [3], 6: [4, 5]}
            pv_at = {4: [0], 5: [1], 6: [2, 3], 7: [4, 5]}
            dve_extra = {
                0: [lambda: rope_h(dma_rope_h(1, kpk_d, 0), KT[1], 0)],
                1: [lambda: emit_kg(0),
                    lambda: rope_h(dma_rope_h(1, qpk_d, 0), QT[1], 0)],
                2: [lambda: rope_h(dma_rope_h(1, kpk_d, 1), KT[1], 1)],
                3: [lambda: rope_h(dma_rope_h(1, qpk_d, 1), QT[1], 1),
                    lambda: emit_kg(1)],
            }
            for i, (h, p) in enumerate(seq):
                if i == n - 1:  # last pairs' exps overlap the last local
                    emit_global_scores(*seq[n - 2])
                    emit_global_scores(*seq[n - 1])
                emit_scores(h, p)
                if i >= 1:
                    emit_sumspv(*seq[i - 1])
                if i == 1:
                    emit_v_pool(0)
                if i == 3:
                    emit_v_pool(1)
                for fn in dve_extra.get(i, []):
                    fn()
                for j in scores_at.get(i, []):
                    emit_global_scores(*seq[j])
                for j in pv_at.get(i, []):
                    emit_global_pv(*seq[j], split=(j >= n - 3))
                    emit_combine(*seq[j], split=(j >= n - 3))
            emit_sumspv(*seq[n - 1])
            for j in (n - 2, n - 1):
                emit_global_pv(*seq[j], split=True)
                emit_combine(*seq[j], split=True)
    return nc


def _get_program():
    if "nc" not in _CACHE:
        _CACHE["nc"] = _build_program()
        _CACHE["consts"] = _host_constants()
    return _CACHE["nc"], _CACHE["consts"]


# ---------------------------------------------------------------- entry point
def kernel(q, k, v, zero_k, zero_v):
    nc, consts = _get_program()
    from concourse.bass_utils import run_bass_kernel_spmd

    bf = ml_dtypes.bfloat16
    q4 = np.asarray(q, dtype=np.float32).reshape(T, H, D)
    k4 = np.asarray(k, dtype=np.float32).reshape(T, H, D)
    v4 = np.asarray(v, dtype=np.float32).reshape(T, H, D)
    zk = np.asarray(zero_k, dtype=np.float32).reshape(H, D)
    zv = np.asarray(zero_v, dtype=np.float32).reshape(H, D)

    def pack_halves(xT):  # [h, D, T] -> [h, half, D, raw|swap]
        # plain partition swap; the sign lives in the sign-folded sin table
        rot = np.concatenate([xT[:, D // 2 :], xT[:, : D // 2]], axis=1)
        pk = np.empty((HPC, 2, D, 2 * HALF), dtype=np.float32)
        for half in (0, 1):
            cs = slice(half * HALF, (half + 1) * HALF)
            pk[:, half, :, 0:HALF] = xT[:, :, cs]
            pk[:, half, :, HALF:] = rot[:, :, cs]
        return pk.astype(bf)

    in_maps = []
    for core in range(NCORES):
        hs = slice(HPC * core, HPC * (core + 1))
        qT = np.ascontiguousarray(q4[:, hs].transpose(1, 2, 0))   # [h, D, T]
        kT = np.ascontiguousarray(k4[:, hs].transpose(1, 2, 0))
        # v token-major: vpk[h, p, n*128+d] = v[n*128+p, head, d]
        vpk = np.ascontiguousarray(
            v4[:, hs].reshape(NT, 128, HPC, D).transpose(2, 1, 0, 3)
        ).reshape(HPC, 128, NT * D)
        blob = consts["blob"].copy()
        blob[:, B_ZK : B_ZK + HPC] = zk[hs].T.astype(bf)
        fblob = consts["fblob"].copy()
        fblob[:, 128 : 128 + HPC] = zv[hs].T
        in_maps.append(
            {
                "qpk": pack_halves(qT),
                "kpk": pack_halves(kT),
                "vpk": vpk.astype(bf),
                "tpk": consts["tpk"],
                "blob": blob,
                "fblob": fblob,
            }
        )

    res = run_bass_kernel_spmd(nc, in_maps, core_ids=list(range(NCORES)))
    # outT per core: [HPC, D, T] -> out[t, 0, (2*core+h)*D + d]
    arr = np.stack(
        [np.asarray(res.results[i]["outT"], dtype=np.float32) for i in range(NCORES)]
    )  # [8, 2, D, T]
    out = arr.transpose(3, 0, 1, 2).reshape(T, 1, H * D)
    return np.ascontiguousarray(out.astype(np.float32))


# revision 88
# speedup vs baseline: 1.6902x; 1.0032x over previous
"""Trainium2 Bass kernel for CoreAttentionExpand (sparse local+global attention).

Sharding: tensor-parallel over heads. 16 heads / 8 cores = 2 heads per core.
Each core computes RoPE + local-block attention + pooled-global attention for
its 2 heads end-to-end (no collectives); host reassembles the full output.

Design (all-bf16 dataflow; sim ~107.7us vs 181.5us for the f32 baseline):
- Host supplies packed bf16 inputs: per (tensor, half) a [D, raw|swap] pair
  (swap = [x2; x1]; the rotate-half sign is folded into the sin table) so
  one DMA feeds one RoPE chunk; cos|sinN packed the same way; v pre-swizzled
  token-major so its DMA is fully contiguous; all mask/identity constants in
  one blob DMA. The startup DMA chain is ordered so head-0 K/Q land in
  1024-col quarters first and the PE starts scoring at ~9.5us; head-1
  streams in (half granularity) while the PE chews head 0.
- RoPE: swap*sinN on DVE, raw*cos on GpSimd (parallel), add on DVE.
- Scores are computed transposed (S^T = K @ Q^T) in bf16 (1 PE cycle/column
  at any width). Causal/history/global masks accumulate -1e4 ramp matmuls
  into the score PSUM before exp; exp underflows them to 0. The narrow
  diagonal m-tiles pack into shared PSUM tiles (A: m4|m7|m8, B: m5|m6), so
  a block needs 6 exps / 6 score buffers instead of 9.
- Per-block software pipeline (the PE queue is in-order): block p+1's dense
  score matmuls are emitted before block p's exp-dependent sums/PV phase,
  so the PE always has ready work while ACT grinds exps. PSUM: 3 rotating
  2-bank score buffers + a 2-bank sums accumulator (ot rides the score
  pool; it only lives through the PV phase).
- Global branch processes chunk pairs (2p, 2p+1) in one [rows,1024] PSUM
  tile with a 4-row cumulative column mask (one exp per pair); its
  sums/PV lag two blocks so kgT (pooled K) is ready and eg exps overlap.
- Normalization is recip+mult on DVE (no DVE divide in the ISA; at most
  one PSUM operand per op; GpSimd cannot touch PSUM), final branch add on
  GpSimd, output DMA'd as bf16 and widened on the host.
"""

import sys
import math

_REPO = "/opt/trn_rl_repo"
if _REPO not in sys.path:
    sys.path.insert(0, _REPO)

import numpy as np
import ml_dtypes

# ---------------------------------------------------------------- constants
H = 16          # heads
D = 128         # head dim
T = 4096        # tokens
L = 1024        # local block size
S = 128         # global pool stride
E = 128         # local history size
PNUM = T // L   # 4 local blocks
KLEN = T // S + 1  # 33 global keys (incl. zero token)
NCORES = 8
HPC = H // NCORES  # 2 heads per core
NEGBIG = -10000.0
SCALE = 1.0 / math.sqrt(D)
CHUNK = 512     # q-chunk width
NT = T // 128   # 32 token-tiles per head
HALF = 2048     # RoPE chunk width

# const-blob column offsets (bf16 blob)
B_MB = 0
B_MC = 128
B_ONES = 256
B_POOL = 384
B_GB = 385
B_GC = B_GB + 8 * KLEN          # 649
B_ZK = B_GC + CHUNK             # 1161
B_W = B_ZK + HPC                # 1163

_CACHE = {}


def _apply_framework_patches(bassmod, mybir, tilemod):
    """This walrus build rejects >1 sem wait per instruction; split excess
    waits onto preceding same-engine NoOps (pure scheduling transform)."""
    if getattr(tilemod.TileContext, "_wait_split_patched", False):
        return
    TileContext = tilemod.TileContext
    ScopedClock = tilemod.ScopedClock

    orig_add = TileContext._add_instruction
    ctr = [0]

    def split_add(self, inst):
        si = inst.sync_info
        if si is not None and si.on_wait and len(si.on_wait) > 1:
            ow = list(si.on_wait)
            for w in ow[:-1]:
                ctr[0] += 1
                nop = mybir.InstNoOp(name=f"I-wsplit{ctr[0]}", engine=inst.engine)
                nop.sync_info = mybir.SyncInfo(on_wait=[w], on_update=[])
                orig_add(self, nop)
            si.on_wait = [ow[-1]]
        orig_add(self, inst)

    def drain_and_barrier(self, tick_clock, wait_clock):
        nc = self.nc
        probe = nc.sync.nop(nofuse=True, hint="waitprobe")
        wait_clock.add_sem_waits(
            probe.ins, ScopedClock({None: tick_clock.global_clock})
        )
        si = probe.ins.sync_info
        ow = list(si.on_wait) if si and si.on_wait else []
        if len(ow) > 1:
            si.on_wait = ow[:1]
            for w in ow[1:]:
                n2 = nc.sync.nop(nofuse=True, hint="waitsplit")
                n2.ins.sync_info = mybir.SyncInfo(on_wait=[w], on_update=[])
        nc.sync.drain()
        nc.all_engine_barrier()
        popped = nc._tile_sem_poison_stack.pop()
        assert popped is self._sem_poison
        nc.clear_and_free_semaphores(list(self.sems.allocated().values()))
        nc.all_engine_barrier()

    TileContext._add_instruction = split_add
    TileContext._drain_and_barrier = drain_and_barrier
    TileContext._wait_split_patched = True


# ---------------------------------------------------------------- constants (host)
def _host_constants():
    bf = ml_dtypes.bfloat16
    t = np.arange(T, dtype=np.float32)
    inv = (1.0 / (10000.0 ** (np.arange(0, D, 2, dtype=np.float32) / D))).astype(
        np.float32
    )  # [64]
    emb = t[:, None] * inv[None, :]          # [T, 64]
    cos64 = np.cos(emb).astype(np.float32)
    sin64 = np.sin(emb).astype(np.float32)
    # [D, T] head-dim-major tables (plain sin; rotation sign lives in rot)
    cosT = np.concatenate([cos64, cos64], axis=1).T
    sinT = np.concatenate([sin64, sin64], axis=1).T
    # sign-folded sin: rows 0:64 negated, so the rotate-half multiply needs
    # only a partition-swapped read of the raw tensor (no negation anywhere).
    sinN = sinT.copy()
    sinN[0 : D // 2] *= -1.0
    # packed tables: tpk[half] = cos_half | sinN_half  -> [2, D, 2*HALF]
    tpk = np.empty((2, D, 2 * HALF), dtype=np.float32)
    for half in (0, 1):
        cs = slice(half * HALF, (half + 1) * HALF)
        tpk[half, :, 0:HALF] = cosT[:, cs]
        tpk[half, :, HALF:] = sinN[:, cs]
    tpk = tpk.astype(bf)

    idx = np.arange(128)
    GMROWS = 4
    gB = np.zeros((GMROWS, 8 * KLEN), dtype=np.float32)
    for c in range(8):
        for mm in range(GMROWS):
            for j in range(KLEN):
                gB[mm, KLEN * c + j] = 1.0 if j >= 4 * c + mm + 1 else 0.0
    qq = np.arange(CHUNK)
    gC = np.stack(
        [NEGBIG * ((qq >= 128 * mm) & (qq < 128 * (mm + 1))) for mm in range(GMROWS)]
    )

    blob = np.zeros((128, B_W), dtype=np.float32)
    # causal ramp: (mB^T mC)[k, q] = NEGBIG * max(k - q, 0)
    blob[:, B_MB : B_MB + 128] = idx[:, None] <= idx[None, :]     # mB [m,k]: m<=k
    blob[:, B_MC : B_MC + 128] = NEGBIG * (idx[:, None] > idx[None, :])
    blob[:, B_ONES : B_ONES + 128] = 1.0
    blob[:, B_POOL] = 1.0 / S
    blob[0:GMROWS, B_GB : B_GB + 8 * KLEN] = gB
    blob[0:GMROWS, B_GC : B_GC + CHUNK] = gC
    # zk filled per-core in kernel()
    blob = blob.astype(bf)

    fblob = np.zeros((128, 128 + HPC), dtype=np.float32)
    fblob[:, 0:128] = np.eye(128, dtype=np.float32)
    # zv filled per-core in kernel()
    return {"tpk": tpk, "blob": blob, "fblob": fblob}


# ---------------------------------------------------------------- device program
def _build_program():
    import concourse.bass as bass
    import concourse.mybir as mybir
    import concourse.tile as tile

    _apply_framework_patches(bass, mybir, tile)

    f32 = mybir.dt.float32
    bf16 = mybir.dt.bfloat16
    EXP = mybir.ActivationFunctionType.Exp
    MUL = mybir.AluOpType.mult
    ADD = mybir.AluOpType.add
    DIV = mybir.AluOpType.divide

    nc = bass.Bass()
    qpk_d = nc.dram_tensor("qpk", [HPC, 2, D, 2 * HALF], bf16, kind="ExternalInput")
    kpk_d = nc.dram_tensor("kpk", [HPC, 2, D, 2 * HALF], bf16, kind="ExternalInput")
    vpk_d = nc.dram_tensor("vpk", [HPC, D, NT * 128], bf16, kind="ExternalInput")
    tpk_d = nc.dram_tensor("tpk", [2, D, 2 * HALF], bf16, kind="ExternalInput")
    blob_d = nc.dram_tensor("blob", [128, B_W], bf16, kind="ExternalInput")
    fblob_d = nc.dram_tensor("fblob", [128, 128 + HPC], f32, kind="ExternalInput")
    out_d = nc.dram_tensor("outT", [HPC, D, T], bf16, kind="ExternalOutput")

    with tile.TileContext(nc) as tc:
        with (
            tc.tile_pool(name="persist", bufs=1) as persist,
            tc.tile_pool(name="pk", bufs=6) as pkp,
            tc.tile_pool(name="expp", bufs=22) as expp,
            tc.tile_pool(name="small", bufs=2) as small,
            tc.tile_pool(name="combine", bufs=4) as combine,
            tc.tile_pool(name="scores", bufs=3, space="PSUM") as scores_p,
            tc.tile_pool(name="acc", bufs=1, space="PSUM") as acc_p,
        ):
            QT, KT, VBF, KGT, VG = {}, {}, {}, {}, {}
            tabs = {}

            def make_tab(half):
                tab = persist.tile([D, 2 * HALF], bf16, tag=f"tab{half}",
                                   name=f"tab{half}")
                tabs[half] = tab
                return tab

            def dma_tab(half, part, qtr=None, cols=None):
                """DMA one piece of a cos|sin table. part: 'cos'|'sin';
                qtr None = whole 2048-col part, else 1024-col quarter;
                cols=(lo,hi) overrides with an explicit column range."""
                base = 0 if part == "cos" else HALF
                if cols is not None:
                    cs = slice(base + cols[0], base + cols[1])
                elif qtr is None:
                    cs = slice(base, base + HALF)
                else:
                    cs = slice(base + qtr * 1024, base + (qtr + 1) * 1024)
                nc.sync.dma_start(out=tabs[half][:, cs], in_=tpk_d.ap()[half][:, cs])

            def dma_rope_q(h, src_d, half, qtr):
                """Load raw+swap quarters ([D,1024] each) into one pk tile."""
                pk = pkp.tile([D, 2048], bf16, tag="pkq", bufs=3)
                qs = slice(qtr * 1024, (qtr + 1) * 1024)
                rs = slice(HALF + qtr * 1024, HALF + (qtr + 1) * 1024)
                nc.sync.dma_start(out=pk[:, 0:1024], in_=src_d.ap()[h, half][:, qs])
                nc.sync.dma_start(out=pk[:, 1024:2048], in_=src_d.ap()[h, half][:, rs])
                return pk

            def dma_rope_p(h, src_d, half, p0):
                """Load a raw+swap 512-col piece into one pk tile (first-
                chunk latency: smaller transfers reach the RoPE sooner)."""
                pk = pkp.tile([D, 2048], bf16, tag="pkq", bufs=3)
                nc.sync.dma_start(
                    out=pk[:, 0:512], in_=src_d.ap()[h, half][:, p0 : p0 + 512]
                )
                nc.sync.dma_start(
                    out=pk[:, 512:1024],
                    in_=src_d.ap()[h, half][:, HALF + p0 : HALF + p0 + 512],
                )
                return pk

            def rope_p(pk, dst, half, p0):
                """RoPE one 512-col piece: 3 passes (t2 on DVE too - these
                run before the big quarters, when every ns of latency counts)."""
                tab = tabs[half]
                t1 = pkp.tile([D, 1024], bf16, tag="t1q", bufs=2)
                t2 = pkp.tile([D, 1024], bf16, tag="t2q", bufs=2)
                nc.vector.tensor_tensor(
                    out=t1[:, 0:512],
                    in0=pk[:, 512:1024],
                    in1=tab[:, HALF + p0 : HALF + p0 + 512],
                    op=MUL,
                )
                nc.vector.tensor_tensor(
                    out=t2[:, 0:512], in0=pk[:, 0:512], in1=tab[:, p0 : p0 + 512],
                    op=MUL,
                )
                dc = half * HALF + p0
                nc.vector.tensor_tensor(
                    out=dst[:, dc : dc + 512], in0=t1[:, 0:512], in1=t2[:, 0:512],
                    op=ADD,
                )

            def rope_q(pk, dst, half, qtr):
                """RoPE one 1024-col quarter: swap*sin + add on DVE, raw*cos
                on GpSimd (runs in parallel, keeps the DVE queue short)."""
                tab = tabs[half]
                co = qtr * 1024
                t1 = pkp.tile([D, 1024], bf16, tag="t1q", bufs=2)
                t2 = pkp.tile([D, 1024], bf16, tag="t2q", bufs=2)
                nc.vector.tensor_tensor(
                    out=t1, in0=pk[:, 1024:2048], in1=tab[:, HALF + co : HALF + co + 1024], op=MUL
                )
                nc.gpsimd.tensor_tensor(
                    out=t2, in0=pk[:, 0:1024], in1=tab[:, co : co + 1024], op=MUL
                )
                dc = half * HALF + co
                nc.vector.tensor_tensor(
                    out=dst[:, dc : dc + 1024], in0=t1, in1=t2, op=ADD
                )

            def dma_rope_h(h, src_d, half):
                """Load a packed raw|swap half ([D, 2*HALF]) in one DMA."""
                pk = pkp.tile([D, 2 * HALF], bf16, tag="pk", bufs=2)
                nc.sync.dma_start(out=pk, in_=src_d.ap()[h, half])
                return pk

            def rope_h(pk, dst, half):
                """RoPE one 2048-col half: swap*sin + add on DVE, raw*cos on
                GpSimd (parallel)."""
                cs = slice(half * HALF, (half + 1) * HALF)
                tab = tabs[half]
                t1 = pkp.tile([D, HALF], bf16, tag="t1", bufs=2)
                t2 = pkp.tile([D, HALF], bf16, tag="t2", bufs=2)
                nc.vector.tensor_tensor(
                    out=t1, in0=pk[:, HALF:], in1=tab[:, HALF:], op=MUL
                )
                nc.gpsimd.tensor_tensor(
                    out=t2, in0=pk[:, 0:HALF], in1=tab[:, 0:HALF], op=MUL
                )
                nc.vector.tensor_tensor(out=dst[:, cs], in0=t1, in1=t2, op=ADD)

            def emit_consts():
                blob = persist.tile([128, B_W], bf16, tag="blob")
                fblob = persist.tile([128, 128 + HPC], f32, tag="fblob")
                nc.sync.dma_start(out=blob, in_=blob_d.ap())
                nc.sync.dma_start(out=fblob, in_=fblob_d.ap())
                return blob, fblob

            def emit_kg(h):
                """Pooled global K for head h (after full KT RoPE)."""
                kgT = persist.tile([D, KLEN], bf16, tag=f"kgT{h}", name=f"kgT{h}")
                KGT[h] = kgT
                nc.vector.tensor_copy(out=kgT[:, 0:1], in_=blob[:, B_ZK + h : B_ZK + h + 1])
                kgf = small.tile([D, KLEN], f32, tag="kgf")
                nc.vector.tensor_reduce(
                    out=kgf[:, 1:KLEN],
                    in_=KT[h].rearrange("p (g s) -> p g s", s=S),
                    axis=mybir.AxisListType.X,
                    op=ADD,
                )
                with nc.allow_low_precision(reason="bf16 pooled keys"):
                    nc.vector.tensor_scalar_mul(
                        out=kgT[:, 1:KLEN], in0=kgf[:, 1:KLEN], scalar1=1.0 / S
                    )

            def make_vbf(h):
                VBF[h] = persist.tile([128, NT, D], bf16, tag=f"vbf{h}", name=f"vbf{h}")

            def dma_v(h, qtr=None):
                flat = VBF[h].rearrange("p n d -> p (n d)")
                if qtr is None:
                    nc.sync.dma_start(out=flat, in_=vpk_d.ap()[h])
                else:
                    cs = slice(qtr * 1024, (qtr + 1) * 1024)
                    nc.sync.dma_start(out=flat[:, cs], in_=vpk_d.ap()[h][:, cs])

            def emit_v_pool(h):
                """Pooled global V via PE: vgTp[:, g+1] = V_g^T @ (1/S)."""
                vgTp = scores_p.tile([128, 1024], f32, tag="s", name="vgTp")
                for g in range(NT):
                    nc.tensor.matmul(
                        out=vgTp[:, g + 1 : g + 2],
                        lhsT=VBF[h][:, g, :],
                        rhs=blob[:, B_POOL : B_POOL + 1],
                        start=(g == 0),
                        stop=(g == NT - 1),
                    )
                # copies on ACT so they never queue behind DVE RoPE/reduces
                vgT = small.tile([D, KLEN], f32, tag="vgT")
                nc.scalar.copy(out=vgT[:, 0:1], in_=fblob[:, 128 + h : 129 + h])
                nc.scalar.copy(out=vgT[:, 1:KLEN], in_=vgTp[:, 1:KLEN])
                # transpose -> Vg token-major [KLEN, D] bf16
                vgp = scores_p.tile([128, 1024], f32, tag="s", name="vgp")
                nc.tensor.transpose(
                    out=vgp[0:KLEN, 0:128], in_=vgT, identity=fblob[:, 0:128]
                )
                Vg = persist.tile([KLEN, 128], bf16, tag=f"Vg{h}", name=f"Vg{h}")
                nc.scalar.copy(out=Vg, in_=vgp[0:KLEN, 0:128])
                VG[h] = Vg

            ACC = {}  # (h, p) -> dict of live PSUM/SBUF tiles for deferred stages

            # local m-tile -> (tile_key, packed column offset). m0-m3 are
            # block-aligned in their own tiles; the narrow tails pack into
            # two shared tiles (A: m4|m7|m8, B: m5|m6), cutting exp count
            # and score-buffer churn from 9 to 6 per block.

            LOC = {
                0: (0, 0), 1: (1, 0), 2: (2, 128), 3: (3, 256),
                4: ("A", 0), 7: ("A", 640), 8: ("A", 896),
                5: ("B", 0), 6: ("B", 512),
            }
            TILE_W = {0: 1024, 1: 1024, 2: 1024, 3: 1024, "A": 1024, "B": 896}

            def emit_scores(h, p):
                """Local block p scores + exps (PE then ACT)."""
                mB = blob[:, B_MB : B_MB + 128]
                mC = blob[:, B_MC : B_MC + 128]
                q0 = p * L
                ms = list(range(1, 9)) if p == 0 else list(range(0, 9))
                tiles, expt = {}, {}
                for m in ms:
                    key, poff = LOC[m]
                    if key not in tiles:
                        tiles[key] = scores_p.tile([128, 1024], f32, tag="s",
                                                   name="st")
                    st = tiles[key]
                    start_m = 0 if m == 0 else 128 * (m - 1)
                    width = 1024 - start_m
                    kcol = q0 - 128 + 128 * m  # k-token start (abs)
                    # QK^T into packed cols [poff, poff+width), split at the
                    # 512-col PSUM bank boundaries of the tile
                    for r0 in range(poff - poff % 512, poff + width, 512):
                        c_lo = max(poff, r0)
                        c_hi = min(poff + width, r0 + 512)
                        is_diag_reg = m >= 1 and c_lo == poff
                        nc.tensor.matmul(
                            out=st[:, c_lo:c_hi],
                            lhsT=KT[h][:, kcol : kcol + 128],
                            rhs=QT[h][
                                :,
                                q0 + start_m + (c_lo - poff) : q0
                                + start_m
                                + (c_hi - poff),
                            ],
                            start=True,
                            stop=not is_diag_reg,
                        )
                        if is_diag_reg:
                            nc.tensor.matmul(
                                out=st[:, poff : poff + 128],
                                lhsT=mB,
                                rhs=mC,
                                start=False,
                                stop=True,
                            )
                # one exp per packed tile
                ets = {}
                for key, st in tiles.items():
                    # valid span of each tile
                    if key in (0, 1, 2, 3):
                        lo, hi = LOC[key][1], 1024
                    else:
                        lo, hi = 0, TILE_W[key]
                    et = expp.tile([128, 1024], bf16, tag="e", name="et")
                    nc.scalar.activation(
                        out=et[:, lo:hi], in_=st[:, lo:hi], func=EXP, scale=SCALE
                    )
                    ets[key] = et
                a = ACC.setdefault((h, p), {})
                a["ets"] = ets
                a["ms"] = ms

            def emit_sumspv(h, p):
                """Local block p sums, normalize, PV, tl."""
                ones_bf = blob[:, B_ONES : B_ONES + 128]
                a = ACC[(h, p)]
                ets, ms = a.pop("ets"), a.pop("ms")

                def et_slice(m, c_lo, c_hi):  # block cols -> packed et AP
                    key, poff = LOC[m]
                    start_m = 0 if m == 0 else 128 * (m - 1)
                    return ets[key][
                        :, poff + (c_lo - start_m) : poff + (c_hi - start_m)
                    ]

                sums = acc_p.tile([128, 1024], f32, tag="sum", name="sums")
                sum_started = [False, False]
                sum_last_m = {
                    reg: max(
                        m
                        for m in ms
                        if (0 if m == 0 else 128 * (m - 1)) < 512 * (reg + 1)
                    )
                    for reg in (0, 1)
                }
                for m in ms:
                    start_m = 0 if m == 0 else 128 * (m - 1)
                    for reg in (0, 1):
                        c_lo = max(start_m, reg * 512)
                        c_hi = (reg + 1) * 512
                        if c_lo >= c_hi:
                            continue
                        nc.tensor.matmul(
                            out=sums[:, c_lo:c_hi],
                            lhsT=ones_bf,
                            rhs=et_slice(m, c_lo, c_hi),
                            start=not sum_started[reg],
                            stop=(m == sum_last_m[reg]),
                        )
                        sum_started[reg] = True
                # Normalize via recip+mult (DVE divide is not in the ISA and
                # a DVE op may read at most one PSUM operand). The recip also
                # frees the bufs=1 sums accumulator for the next block.
                rl = combine.tile([128, 1024], bf16, tag="rl", bufs=3, name="rl")
                with nc.allow_low_precision(reason="probs sum to 1"):
                    nc.vector.reciprocal(out=rl, in_=sums)
                # PV: O^T accumulation per 512-col region. ot lives in the
                # scores pool: only held through the PV phase (freed by the
                # tl multiply), freeing two PSUM banks for a third score buf.
                ot = scores_p.tile([128, 1024], f32, tag="s", name="ot")
                tl = combine.tile([128, 1024], bf16, tag="tl", name="tl")
                for reg in (0, 1):
                    valid_ms = [
                        m
                        for m in ms
                        if (0 if m == 0 else 128 * (m - 1)) < 512 * (reg + 1)
                    ]
                    for i, m in enumerate(valid_ms):
                        start_m = 0 if m == 0 else 128 * (m - 1)
                        c_lo = max(start_m, reg * 512)
                        c_hi = (reg + 1) * 512
                        vidx = 8 * p - 1 + m
                        nc.tensor.matmul(
                            out=ot[:, c_lo:c_hi],
                            lhsT=VBF[h][:, vidx, :],
                            rhs=et_slice(m, c_lo, c_hi),
                            start=(i == 0),
                            stop=(m == valid_ms[-1]),
                        )
                with nc.allow_low_precision(reason="probs sum to 1"):
                    nc.vector.tensor_tensor(out=tl, in0=ot, in1=rl, op=MUL)
                ACC[(h, p)]["tl"] = tl

            def emit_global_scores(h, p):
                """Global chunk pair (2p, 2p+1): scores+mask+exp."""
                rows = min(KLEN, 8 * p + 8)
                sg = scores_p.tile([128, 1024], f32, tag="s", name="sg")
                for ci, c in enumerate((2 * p, 2 * p + 1)):
                    cols = slice(ci * CHUNK, (ci + 1) * CHUNK)
                    nc.tensor.matmul(
                        out=sg[0:rows, cols],
                        lhsT=KGT[h][:, 0:rows],
                        rhs=QT[h][:, c * CHUNK : (c + 1) * CHUNK],
                        start=True,
                        stop=False,
                    )
                    nc.tensor.matmul(
                        out=sg[0:rows, cols],
                        lhsT=blob[0:4, B_GB + KLEN * c : B_GB + KLEN * c + rows],
                        rhs=blob[0:4, B_GC : B_GC + CHUNK],
                        start=False,
                        stop=True,
                    )
                eg = expp.tile([128, 1024], bf16, tag="e", name="eg")
                nc.scalar.activation(
                    out=eg[0:rows, :], in_=sg[0:rows, :], func=EXP, scale=SCALE
                )
                a = ACC.setdefault((h, p), {})
                a["eg"] = eg
                a["rows"] = rows

            def emit_global_pv(h, p, split=False):
                """Global pair sums + PV matmuls."""
                ones_bf = blob[:, B_ONES : B_ONES + 128]
                eg, rows = ACC[(h, p)]["eg"], ACC[(h, p)]["rows"]
                gs = scores_p.tile([128, 1024], f32, tag="s", name="gs")
                go = scores_p.tile([128, 1024], f32, tag="s", name="go")
                for reg in (0, 1):
                    cols = slice(reg * CHUNK, (reg + 1) * CHUNK)
                    nc.tensor.matmul(
                        out=gs[:, cols],
                        lhsT=ones_bf[0:rows, :],
                        rhs=eg[0:rows, cols],
                        start=True,
                        stop=True,
                    )
                    nc.tensor.matmul(
                        out=go[:, cols],
                        lhsT=VG[h][0:rows, :],
                        rhs=eg[0:rows, cols],
                        start=True,
                        stop=True,
                    )
                # global normalize: recip (one PSUM input) then mult on DVE
                rg = combine.tile([128, 1024], bf16, tag="rg", bufs=3, name="rg")
                ogn = combine.tile([128, 1024], bf16, tag="ogn", name="ogn")
                halves = (
                    (slice(0, 512), slice(512, 1024)) if split else (slice(0, 1024),)
                )
                with nc.allow_low_precision(reason="normalized probs sum to 1"):
                    for cs in halves:
                        nc.vector.reciprocal(out=rg[:, cs], in_=gs[:, cs])
                        nc.vector.tensor_tensor(
                            out=ogn[:, cs], in0=go[:, cs], in1=rg[:, cs], op=MUL
                        )
                ACC[(h, p)]["ogn"] = ogn

            def emit_combine(h, p, split=False, fast_dma=False):
                """Branch add (SBUF-only, GpSimd) + write out. With split,
                halves go to GpSimd and DVE in parallel (tail shortening)."""
                a = ACC.pop((h, p))
                tl, ogn = a["tl"], a["ogn"]
                fin = combine.tile([128, 1024], bf16, tag="fin", name="fin")
                if split:
                    engs = [
                        (nc.gpsimd, slice(0, 512)),
                        (nc.vector, slice(512, 1024)),
                    ]
                else:
                    engs = [(nc.gpsimd, slice(0, 1024))]
                with nc.allow_low_precision(reason="normalized probs sum to 1"):
                    for add_eng, cs in engs:
                        add_eng.tensor_tensor(
                            out=fin[:, cs], in0=tl[:, cs], in1=ogn[:, cs], op=ADD
                        )
                if split:
                    for qi, cs in enumerate((slice(0, 512), slice(512, 1024))):
                        eng = nc.scalar if (fast_dma and qi == 1) else nc.sync
                        eng.dma_start(
                            out=out_d.ap()[h, :, p * L + cs.start : p * L + cs.stop],
                            in_=fin[:, cs],
                        )
                else:
                    nc.sync.dma_start(
                        out=out_d.ap()[h, :, p * L : (p + 1) * L], in_=fin
                    )

            # ---- emission order tuned for DMA latency + engine overlap ----
            QT[0] = persist.tile([D, T], bf16, tag="QT0", name="QT0")
            KT[0] = persist.tile([D, T], bf16, tag="KT0", name="KT0")
            QT[1] = persist.tile([D, T], bf16, tag="QT1", name="QT1")
            KT[1] = persist.tile([D, T], bf16, tag="KT1", name="KT1")
            make_tab(0)
            make_tab(1)
            make_vbf(0)
            make_vbf(1)
            # Startup DMA chain (single serialized DMA resource): head-0
            # K/Q land in 1024-col quarters interleaved with exactly the
            # table pieces each RoPE pass needs, so the PE's first scores
            # start at ~9.5us; V arrives in quarters just ahead of each
            # block's PV; head-1 streams in while PE chews head 0.
            dma_tab(0, "sin", 0)
            ka = dma_rope_q(0, kpk_d, 0, 0)
            dma_tab(0, "cos", 0)
            qa = dma_rope_q(0, qpk_d, 0, 0)
            # PE warmup: dummy matmuls on the (already landed) sin table keep
            # the PE busy through the DMA startup bubble so the p-state /
            # HAM clock-gate reaches full speed before the first real scores
            # (outputs are never read; the buffer recycles on write-done).
            warm = scores_p.tile([128, 1024], f32, tag="s", name="warm")
            for _ in range(8):
                nc.tensor.matmul(
                    out=warm[:, 0:512],
                    lhsT=tabs[0][:, HALF : HALF + 128],
                    rhs=tabs[0][:, HALF : HALF + 512],
                    start=True,
                    stop=True,
                )
            blob, fblob = emit_consts()
            dma_tab(0, "sin", 1)
            kb = dma_rope_q(0, kpk_d, 0, 1)
            dma_tab(0, "cos", 1)
            qb = dma_rope_q(0, qpk_d, 0, 1)
            dma_v(0, 0)
            dma_tab(1, "sin")
            kc = dma_rope_q(0, kpk_d, 1, 0)
            dma_tab(1, "cos")
            qc = dma_rope_q(0, qpk_d, 1, 0)
            dma_v(0, 1)
            kd = dma_rope_q(0, kpk_d, 1, 1)
            qd = dma_rope_q(0, qpk_d, 1, 1)
            dma_v(0, 2)
            dma_v(0, 3)
            # DVE RoPE stream (in-order queue): head-0 quarters first; the
            # head-1 halves + kg reduces interleave into the block loop so
            # the per-block tl/ogn divides never queue behind them.
            rope_q(ka, KT[0], 0, 0)
            rope_q(qa, QT[0], 0, 0)
            rope_q(kb, KT[0], 0, 1)
            rope_q(qb, QT[0], 0, 1)
            rope_q(kc, KT[0], 1, 0)
            rope_q(qc, QT[0], 1, 0)
            rope_q(kd, KT[0], 1, 1)
            rope_q(qd, QT[0], 1, 1)
            dma_v(1)
            # Block pipeline: globals spread so sg(j) lands once kgT is
            # ready and eg exps overlap locals; gs/go + combine trail so
            # score-buffer recycling never waits on the combine divides.
            seq = [(0, p) for p in range(PNUM)] + [(1, p) for p in range(PNUM)]
            n = len(seq)
            scores_at = {2: [0], 3: [1], 4: [2], 5: [3], 6: [4, 5]}
            pv_at = {4: [0], 5: [1], 6: [2, 3], 7: [4, 5]}
            dve_extra = {
                0: [lambda: rope_h(dma_rope_h(1, kpk_d, 0), KT[1], 0)],
                1: [lambda: emit_kg(0),
                    lambda: rope_h(dma_rope_h(1, qpk_d, 0), QT[1], 0)],
                2: [lambda: rope_h(dma_rope_h(1, kpk_d, 1), KT[1], 1)],
                3: [lambda: rope_h(dma_rope_h(1, qpk_d, 1), QT[1], 1),
                    lambda: emit_kg(1)],
            }
            for i, (h, p) in enumerate(seq):
                if i == n - 1:  # last pairs' exps overlap the last local
                    emit_global_scores(*seq[n - 2])
                    emit_global_scores(*seq[n - 1])
                emit_scores(h, p)
                if i >= 1:
                    emit_sumspv(*seq[i - 1])
                if i == 1:
                    emit_v_pool(0)
                if i == 3:
                    emit_v_pool(1)
                for fn in dve_extra.get(i, []):
                    fn()
                for j in scores_at.get(i, []):
                    emit_global_scores(*seq[j])
                for j in pv_at.get(i, []):
                    emit_global_pv(*seq[j], split=(j >= n - 3))
                    emit_combine(*seq[j], split=(j >= n - 3))
            emit_sumspv(*seq[n - 1])
            for j in (n - 2, n - 1):
                emit_global_pv(*seq[j], split=True)
                emit_combine(*seq[j], split=True)
    return nc


def _get_program():
    if "nc" not in _CACHE:
        _CACHE["nc"] = _build_program()
        _CACHE["consts"] = _host_constants()
    return _CACHE["nc"], _CACHE["consts"]


# ---------------------------------------------------------------- entry point
def kernel(q, k, v, zero_k, zero_v):
    nc, consts = _get_program()
    from concourse.bass_utils import run_bass_kernel_spmd

    bf = ml_dtypes.bfloat16
    q4 = np.asarray(q, dtype=np.float32).reshape(T, H, D)
    k4 = np.asarray(k, dtype=np.float32).reshape(T, H, D)
    v4 = np.asarray(v, dtype=np.float32).reshape(T, H, D)
    zk = np.asarray(zero_k, dtype=np.float32).reshape(H, D)
    zv = np.asarray(zero_v, dtype=np.float32).reshape(H, D)

    def pack_halves(xT):  # [h, D, T] -> [h, half, D, raw|swap]
        # plain partition swap; the sign lives in the sign-folded sin table
        rot = np.concatenate([xT[:, D // 2 :], xT[:, : D // 2]], axis=1)
        pk = np.empty((HPC, 2, D, 2 * HALF), dtype=np.float32)
        for half in (0, 1):
            cs = slice(half * HALF, (half + 1) * HALF)
            pk[:, half, :, 0:HALF] = xT[:, :, cs]
            pk[:, half, :, HALF:] = rot[:, :, cs]
        return pk.astype(bf)

    in_maps = []
    for core in range(NCORES):
        hs = slice(HPC * core, HPC * (core + 1))
        qT = np.ascontiguousarray(q4[:, hs].transpose(1, 2, 0))   # [h, D, T]
        kT = np.ascontiguousarray(k4[:, hs].transpose(1, 2, 0))
        # v token-major: vpk[h, p, n*128+d] = v[n*128+p, head, d]
        vpk = np.ascontiguousarray(
            v4[:, hs].reshape(NT, 128, HPC, D).transpose(2, 1, 0, 3)
        ).reshape(HPC, 128, NT * D)
        blob = consts["blob"].copy()
        blob[:, B_ZK : B_ZK + HPC] = zk[hs].T.astype(bf)
        fblob = consts["fblob"].copy()
        fblob[:, 128 : 128 + HPC] = zv[hs].T
        in_maps.append(
            {
                "qpk": pack_halves(qT),
                "kpk": pack_halves(kT),
                "vpk": vpk.astype(bf),
                "tpk": consts["tpk"],
                "blob": blob,
                "fblob": fblob,
            }
        )

    res = run_bass_kernel_spmd(nc, in_maps, core_ids=list(range(NCORES)))
    # outT per core: [HPC, D, T] -> out[t, 0, (2*core+h)*D + d]
    arr = np.stack(
        [np.asarray(res.results[i]["outT"], dtype=np.float32) for i in range(NCORES)]
    )  # [8, 2, D, T]
    out = arr.transpose(3, 0, 1, 2).reshape(T, 1, H * D)
    return np.ascontiguousarray(out.astype(np.float32))


# revision 94
# speedup vs baseline: 1.7129x; 1.0134x over previous
"""Trainium2 Bass kernel for CoreAttentionExpand (sparse local+global attention).

Sharding: tensor-parallel over heads. 16 heads / 8 cores = 2 heads per core.
Each core computes RoPE + local-block attention + pooled-global attention for
its 2 heads end-to-end (no collectives); host reassembles the full output.

Design (all-bf16 dataflow; sim ~107.7us vs 181.5us for the f32 baseline):
- Host supplies packed bf16 inputs: per (tensor, half) a [D, raw|swap] pair
  (swap = [x2; x1]; the rotate-half sign is folded into the sin table) so
  one DMA feeds one RoPE chunk; cos|sinN packed the same way; v pre-swizzled
  token-major so its DMA is fully contiguous; all mask/identity constants in
  one blob DMA. The startup DMA chain is ordered so head-0 K/Q land in
  1024-col quarters first and the PE starts scoring at ~9.5us; head-1
  streams in (half granularity) while the PE chews head 0.
- RoPE: swap*sinN on DVE, raw*cos on GpSimd (parallel), add on DVE.
- Scores are computed transposed (S^T = K @ Q^T) in bf16 (1 PE cycle/column
  at any width). Causal/history/global masks accumulate -1e4 ramp matmuls
  into the score PSUM before exp; exp underflows them to 0. The narrow
  diagonal m-tiles pack into shared PSUM tiles (A: m4|m7|m8, B: m5|m6), so
  a block needs 6 exps / 6 score buffers instead of 9.
- Per-block software pipeline (the PE queue is in-order): block p+1's dense
  score matmuls are emitted before block p's exp-dependent sums/PV phase,
  so the PE always has ready work while ACT grinds exps. PSUM: 3 rotating
  2-bank score buffers + a 2-bank sums accumulator (ot rides the score
  pool; it only lives through the PV phase).
- Global branch processes chunk pairs (2p, 2p+1) in one [rows,1024] PSUM
  tile with a 4-row cumulative column mask (one exp per pair); its
  sums/PV lag two blocks so kgT (pooled K) is ready and eg exps overlap.
- Normalization is recip+mult on DVE (no DVE divide in the ISA; at most
  one PSUM operand per op; GpSimd cannot touch PSUM), final branch add on
  GpSimd, output DMA'd as bf16 and widened on the host.
"""

import sys
import math

_REPO = "/opt/trn_rl_repo"
if _REPO not in sys.path:
    sys.path.insert(0, _REPO)

import numpy as np
import ml_dtypes

# ---------------------------------------------------------------- constants
H = 16          # heads
D = 128         # head dim
T = 4096        # tokens
L = 1024        # local block size
S = 128         # global pool stride
E = 128         # local history size
PNUM = T // L   # 4 local blocks
KLEN = T // S + 1  # 33 global keys (incl. zero token)
NCORES = 8
HPC = H // NCORES  # 2 heads per core
NEGBIG = -10000.0
SCALE = 1.0 / math.sqrt(D)
CHUNK = 512     # q-chunk width
NT = T // 128   # 32 token-tiles per head
HALF = 2048     # RoPE chunk width

# const-blob column offsets (bf16 blob)
B_MB = 0
B_MC = 128
B_ONES = 256
B_POOL = 384
B_GB = 385
B_GC = B_GB + 8 * KLEN          # 649
B_ZK = B_GC + CHUNK             # 1161
B_W = B_ZK + HPC                # 1163

_CACHE = {}


def _apply_framework_patches(bassmod, mybir, tilemod):
    """This walrus build rejects >1 sem wait per instruction; split excess
    waits onto preceding same-engine NoOps (pure scheduling transform)."""
    if getattr(tilemod.TileContext, "_wait_split_patched", False):
        return
    TileContext = tilemod.TileContext
    ScopedClock = tilemod.ScopedClock

    orig_add = TileContext._add_instruction
    ctr = [0]

    def split_add(self, inst):
        si = inst.sync_info
        if si is not None and si.on_wait and len(si.on_wait) > 1:
            ow = list(si.on_wait)
            for w in ow[:-1]:
                ctr[0] += 1
                nop = mybir.InstNoOp(name=f"I-wsplit{ctr[0]}", engine=inst.engine)
                nop.sync_info = mybir.SyncInfo(on_wait=[w], on_update=[])
                orig_add(self, nop)
            si.on_wait = [ow[-1]]
        orig_add(self, inst)

    def drain_and_barrier(self, tick_clock, wait_clock):
        nc = self.nc
        probe = nc.sync.nop(nofuse=True, hint="waitprobe")
        wait_clock.add_sem_waits(
            probe.ins, ScopedClock({None: tick_clock.global_clock})
        )
        si = probe.ins.sync_info
        ow = list(si.on_wait) if si and si.on_wait else []
        if len(ow) > 1:
            si.on_wait = ow[:1]
            for w in ow[1:]:
                n2 = nc.sync.nop(nofuse=True, hint="waitsplit")
                n2.ins.sync_info = mybir.SyncInfo(on_wait=[w], on_update=[])
        nc.sync.drain()
        nc.all_engine_barrier()
        popped = nc._tile_sem_poison_stack.pop()
        assert popped is self._sem_poison
        nc.clear_and_free_semaphores(list(self.sems.allocated().values()))
        nc.all_engine_barrier()

    TileContext._add_instruction = split_add
    TileContext._drain_and_barrier = drain_and_barrier
    TileContext._wait_split_patched = True


# ---------------------------------------------------------------- constants (host)
def _host_constants():
    bf = ml_dtypes.bfloat16
    t = np.arange(T, dtype=np.float32)
    inv = (1.0 / (10000.0 ** (np.arange(0, D, 2, dtype=np.float32) / D))).astype(
        np.float32
    )  # [64]
    emb = t[:, None] * inv[None, :]          # [T, 64]
    cos64 = np.cos(emb).astype(np.float32)
    sin64 = np.sin(emb).astype(np.float32)
    # [D, T] head-dim-major tables (plain sin; rotation sign lives in rot)
    cosT = np.concatenate([cos64, cos64], axis=1).T
    sinT = np.concatenate([sin64, sin64], axis=1).T
    # sign-folded sin: rows 0:64 negated, so the rotate-half multiply needs
    # only a partition-swapped read of the raw tensor (no negation anywhere).
    sinN = sinT.copy()
    sinN[0 : D // 2] *= -1.0
    # packed tables: tpk[half] = cos_half | sinN_half  -> [2, D, 2*HALF]
    tpk = np.empty((2, D, 2 * HALF), dtype=np.float32)
    for half in (0, 1):
        cs = slice(half * HALF, (half + 1) * HALF)
        tpk[half, :, 0:HALF] = cosT[:, cs]
        tpk[half, :, HALF:] = sinN[:, cs]
    tpk = tpk.astype(bf)

    idx = np.arange(128)
    GMROWS = 4
    gB = np.zeros((GMROWS, 8 * KLEN), dtype=np.float32)
    for c in range(8):
        for mm in range(GMROWS):
            for j in range(KLEN):
                gB[mm, KLEN * c + j] = 1.0 if j >= 4 * c + mm + 1 else 0.0
    qq = np.arange(CHUNK)
    gC = np.stack(
        [NEGBIG * ((qq >= 128 * mm) & (qq < 128 * (mm + 1))) for mm in range(GMROWS)]
    )

    blob = np.zeros((128, B_W), dtype=np.float32)
    # causal ramp: (mB^T mC)[k, q] = NEGBIG * max(k - q, 0)
    blob[:, B_MB : B_MB + 128] = idx[:, None] <= idx[None, :]     # mB [m,k]: m<=k
    blob[:, B_MC : B_MC + 128] = NEGBIG * (idx[:, None] > idx[None, :])
    blob[:, B_ONES : B_ONES + 128] = 1.0
    blob[:, B_POOL] = 1.0 / S
    blob[0:GMROWS, B_GB : B_GB + 8 * KLEN] = gB
    blob[0:GMROWS, B_GC : B_GC + CHUNK] = gC
    # zk filled per-core in kernel()
    blob = blob.astype(bf)

    fblob = np.zeros((128, 128 + HPC), dtype=np.float32)
    fblob[:, 0:128] = np.eye(128, dtype=np.float32)
    # zv filled per-core in kernel()
    return {"tpk": tpk, "blob": blob, "fblob": fblob}


# ---------------------------------------------------------------- device program
def _build_program():
    import concourse.bass as bass
    import concourse.mybir as mybir
    import concourse.tile as tile

    _apply_framework_patches(bass, mybir, tile)

    f32 = mybir.dt.float32
    bf16 = mybir.dt.bfloat16
    EXP = mybir.ActivationFunctionType.Exp
    MUL = mybir.AluOpType.mult
    ADD = mybir.AluOpType.add
    DIV = mybir.AluOpType.divide

    nc = bass.Bass()
    qpk_d = nc.dram_tensor("qpk", [HPC, 2, D, 2 * HALF], bf16, kind="ExternalInput")
    kpk_d = nc.dram_tensor("kpk", [HPC, 2, D, 2 * HALF], bf16, kind="ExternalInput")
    vpk_d = nc.dram_tensor("vpk", [HPC, D, NT * 128], bf16, kind="ExternalInput")
    tpk_d = nc.dram_tensor("tpk", [2, D, 2 * HALF], bf16, kind="ExternalInput")
    blob_d = nc.dram_tensor("blob", [128, B_W], bf16, kind="ExternalInput")
    fblob_d = nc.dram_tensor("fblob", [128, 128 + HPC], f32, kind="ExternalInput")
    out_d = nc.dram_tensor("outT", [HPC, D, T], bf16, kind="ExternalOutput")

    with tile.TileContext(nc) as tc:
        with (
            tc.tile_pool(name="persist", bufs=1) as persist,
            tc.tile_pool(name="pk", bufs=6) as pkp,
            tc.tile_pool(name="expp", bufs=22) as expp,
            tc.tile_pool(name="small", bufs=2) as small,
            tc.tile_pool(name="combine", bufs=4) as combine,
            tc.tile_pool(name="scores", bufs=3, space="PSUM") as scores_p,
            tc.tile_pool(name="acc", bufs=1, space="PSUM") as acc_p,
        ):
            QT, KT, VBF, KGT, VG = {}, {}, {}, {}, {}
            tabs = {}

            def make_tab(half):
                tab = persist.tile([D, 2 * HALF], bf16, tag=f"tab{half}",
                                   name=f"tab{half}")
                tabs[half] = tab
                return tab

            def dma_tab(half, part, qtr=None, cols=None):
                """DMA one piece of a cos|sin table. part: 'cos'|'sin';
                qtr None = whole 2048-col part, else 1024-col quarter;
                cols=(lo,hi) overrides with an explicit column range."""
                base = 0 if part == "cos" else HALF
                if cols is not None:
                    cs = slice(base + cols[0], base + cols[1])
                elif qtr is None:
                    cs = slice(base, base + HALF)
                else:
                    cs = slice(base + qtr * 1024, base + (qtr + 1) * 1024)
                nc.sync.dma_start(out=tabs[half][:, cs], in_=tpk_d.ap()[half][:, cs])

            def dma_rope_q(h, src_d, half, qtr):
                """Load raw+swap quarters ([D,1024] each) into one pk tile."""
                pk = pkp.tile([D, 2048], bf16, tag="pkq", bufs=3)
                qs = slice(qtr * 1024, (qtr + 1) * 1024)
                rs = slice(HALF + qtr * 1024, HALF + (qtr + 1) * 1024)
                nc.sync.dma_start(out=pk[:, 0:1024], in_=src_d.ap()[h, half][:, qs])
                nc.sync.dma_start(out=pk[:, 1024:2048], in_=src_d.ap()[h, half][:, rs])
                return pk

            def dma_rope_p(h, src_d, half, p0):
                """Load a raw+swap 512-col piece into one pk tile (first-
                chunk latency: smaller transfers reach the RoPE sooner)."""
                pk = pkp.tile([D, 2048], bf16, tag="pkq", bufs=3)
                nc.sync.dma_start(
                    out=pk[:, 0:512], in_=src_d.ap()[h, half][:, p0 : p0 + 512]
                )
                nc.sync.dma_start(
                    out=pk[:, 512:1024],
                    in_=src_d.ap()[h, half][:, HALF + p0 : HALF + p0 + 512],
                )
                return pk

            def rope_p(pk, dst, half, p0):
                """RoPE one 512-col piece: 3 passes (t2 on DVE too - these
                run before the big quarters, when every ns of latency counts)."""
                tab = tabs[half]
                t1 = pkp.tile([D, 1024], bf16, tag="t1q", bufs=2)
                t2 = pkp.tile([D, 1024], bf16, tag="t2q", bufs=2)
                nc.vector.tensor_tensor(
                    out=t1[:, 0:512],
                    in0=pk[:, 512:1024],
                    in1=tab[:, HALF + p0 : HALF + p0 + 512],
                    op=MUL,
                )
                nc.vector.tensor_tensor(
                    out=t2[:, 0:512], in0=pk[:, 0:512], in1=tab[:, p0 : p0 + 512],
                    op=MUL,
                )
                dc = half * HALF + p0
                nc.vector.tensor_tensor(
                    out=dst[:, dc : dc + 512], in0=t1[:, 0:512], in1=t2[:, 0:512],
                    op=ADD,
                )

            def rope_q(pk, dst, half, qtr, t2_eng=None):
                """RoPE one 1024-col quarter: swap*sin + add on DVE, raw*cos
                on GpSimd (runs in parallel, keeps the DVE queue short) --
                except the startup-critical first quarters, where the slow
                GpSimd pass would gate the PE start (t2_eng=DVE there)."""
                tab = tabs[half]
                co = qtr * 1024
                t1 = pkp.tile([D, 1024], bf16, tag="t1q", bufs=2)
                t2 = pkp.tile([D, 1024], bf16, tag="t2q", bufs=2)
                nc.vector.tensor_tensor(
                    out=t1, in0=pk[:, 1024:2048], in1=tab[:, HALF + co : HALF + co + 1024], op=MUL
                )
                (t2_eng or nc.gpsimd).tensor_tensor(
                    out=t2, in0=pk[:, 0:1024], in1=tab[:, co : co + 1024], op=MUL
                )
                dc = half * HALF + co
                nc.vector.tensor_tensor(
                    out=dst[:, dc : dc + 1024], in0=t1, in1=t2, op=ADD
                )

            def dma_rope_h(h, src_d, half):
                """Load a packed raw|swap half ([D, 2*HALF]) in one DMA."""
                pk = pkp.tile([D, 2 * HALF], bf16, tag="pk", bufs=2)
                nc.sync.dma_start(out=pk, in_=src_d.ap()[h, half])
                return pk

            def rope_h(pk, dst, half):
                """RoPE one 2048-col half: swap*sin + add on DVE, raw*cos on
                GpSimd (parallel)."""
                cs = slice(half * HALF, (half + 1) * HALF)
                tab = tabs[half]
                t1 = pkp.tile([D, HALF], bf16, tag="t1", bufs=2)
                t2 = pkp.tile([D, HALF], bf16, tag="t2", bufs=2)
                nc.vector.tensor_tensor(
                    out=t1, in0=pk[:, HALF:], in1=tab[:, HALF:], op=MUL
                )
                nc.gpsimd.tensor_tensor(
                    out=t2, in0=pk[:, 0:HALF], in1=tab[:, 0:HALF], op=MUL
                )
                nc.vector.tensor_tensor(out=dst[:, cs], in0=t1, in1=t2, op=ADD)

            def emit_consts():
                blob = persist.tile([128, B_W], bf16, tag="blob")
                fblob = persist.tile([128, 128 + HPC], f32, tag="fblob")
                nc.sync.dma_start(out=blob, in_=blob_d.ap())
                nc.sync.dma_start(out=fblob, in_=fblob_d.ap())
                return blob, fblob

            def emit_kg(h):
                """Pooled global K for head h (after full KT RoPE)."""
                kgT = persist.tile([D, KLEN], bf16, tag=f"kgT{h}", name=f"kgT{h}")
                KGT[h] = kgT
                nc.vector.tensor_copy(out=kgT[:, 0:1], in_=blob[:, B_ZK + h : B_ZK + h + 1])
                kgf = small.tile([D, KLEN], f32, tag="kgf")
                nc.vector.tensor_reduce(
                    out=kgf[:, 1:KLEN],
                    in_=KT[h].rearrange("p (g s) -> p g s", s=S),
                    axis=mybir.AxisListType.X,
                    op=ADD,
                )
                with nc.allow_low_precision(reason="bf16 pooled keys"):
                    nc.vector.tensor_scalar_mul(
                        out=kgT[:, 1:KLEN], in0=kgf[:, 1:KLEN], scalar1=1.0 / S
                    )

            def make_vbf(h):
                VBF[h] = persist.tile([128, NT, D], bf16, tag=f"vbf{h}", name=f"vbf{h}")

            def dma_v(h, qtr=None):
                flat = VBF[h].rearrange("p n d -> p (n d)")
                if qtr is None:
                    nc.sync.dma_start(out=flat, in_=vpk_d.ap()[h])
                else:
                    cs = slice(qtr * 1024, (qtr + 1) * 1024)
                    nc.sync.dma_start(out=flat[:, cs], in_=vpk_d.ap()[h][:, cs])

            def emit_v_pool(h):
                """Pooled global V via PE: vgTp[:, g+1] = V_g^T @ (1/S)."""
                vgTp = scores_p.tile([128, 1024], f32, tag="s", name="vgTp")
                for g in range(NT):
                    nc.tensor.matmul(
                        out=vgTp[:, g + 1 : g + 2],
                        lhsT=VBF[h][:, g, :],
                        rhs=blob[:, B_POOL : B_POOL + 1],
                        start=(g == 0),
                        stop=(g == NT - 1),
                    )
                # copies on ACT so they never queue behind DVE RoPE/reduces
                vgT = small.tile([D, KLEN], f32, tag="vgT")
                nc.scalar.copy(out=vgT[:, 0:1], in_=fblob[:, 128 + h : 129 + h])
                nc.scalar.copy(out=vgT[:, 1:KLEN], in_=vgTp[:, 1:KLEN])
                # transpose -> Vg token-major [KLEN, D] bf16
                vgp = scores_p.tile([128, 1024], f32, tag="s", name="vgp")
                nc.tensor.transpose(
                    out=vgp[0:KLEN, 0:128], in_=vgT, identity=fblob[:, 0:128]
                )
                Vg = persist.tile([KLEN, 128], bf16, tag=f"Vg{h}", name=f"Vg{h}")
                nc.scalar.copy(out=Vg, in_=vgp[0:KLEN, 0:128])
                VG[h] = Vg

            ACC = {}  # (h, p) -> dict of live PSUM/SBUF tiles for deferred stages

            # local m-tile -> (tile_key, packed column offset). m0-m3 are
            # block-aligned in their own tiles; the narrow tails pack into
            # two shared tiles (A: m4|m7|m8, B: m5|m6), cutting exp count
            # and score-buffer churn from 9 to 6 per block.

            LOC = {
                0: (0, 0), 1: (1, 0), 2: (2, 128), 3: (3, 256),
                4: ("A", 0), 7: ("A", 640), 8: ("A", 896),
                5: ("B", 0), 6: ("B", 512),
            }
            TILE_W = {0: 1024, 1: 1024, 2: 1024, 3: 1024, "A": 1024, "B": 896}

            def emit_scores(h, p):
                """Local block p scores + exps (PE then ACT)."""
                mB = blob[:, B_MB : B_MB + 128]
                mC = blob[:, B_MC : B_MC + 128]
                q0 = p * L
                ms = list(range(1, 9)) if p == 0 else list(range(0, 9))
                tiles, expt = {}, {}
                for m in ms:
                    key, poff = LOC[m]
                    if key not in tiles:
                        tiles[key] = scores_p.tile([128, 1024], f32, tag="s",
                                                   name="st")
                    st = tiles[key]
                    start_m = 0 if m == 0 else 128 * (m - 1)
                    width = 1024 - start_m
                    kcol = q0 - 128 + 128 * m  # k-token start (abs)
                    # QK^T into packed cols [poff, poff+width), split at the
                    # 512-col PSUM bank boundaries of the tile
                    for r0 in range(poff - poff % 512, poff + width, 512):
                        c_lo = max(poff, r0)
                        c_hi = min(poff + width, r0 + 512)
                        is_diag_reg = m >= 1 and c_lo == poff
                        nc.tensor.matmul(
                            out=st[:, c_lo:c_hi],
                            lhsT=KT[h][:, kcol : kcol + 128],
                            rhs=QT[h][
                                :,
                                q0 + start_m + (c_lo - poff) : q0
                                + start_m
                                + (c_hi - poff),
                            ],
                            start=True,
                            stop=not is_diag_reg,
                        )
                        if is_diag_reg:
                            nc.tensor.matmul(
                                out=st[:, poff : poff + 128],
                                lhsT=mB,
                                rhs=mC,
                                start=False,
                                stop=True,
                            )
                # one exp per packed tile
                ets = {}
                for key, st in tiles.items():
                    # valid span of each tile
                    if key in (0, 1, 2, 3):
                        lo, hi = LOC[key][1], 1024
                    else:
                        lo, hi = 0, TILE_W[key]
                    et = expp.tile([128, 1024], bf16, tag="e", name="et")
                    nc.scalar.activation(
                        out=et[:, lo:hi], in_=st[:, lo:hi], func=EXP, scale=SCALE
                    )
                    ets[key] = et
                a = ACC.setdefault((h, p), {})
                a["ets"] = ets
                a["ms"] = ms

            def emit_sumspv(h, p):
                """Local block p sums, normalize, PV, tl."""
                ones_bf = blob[:, B_ONES : B_ONES + 128]
                a = ACC[(h, p)]
                ets, ms = a.pop("ets"), a.pop("ms")

                def et_slice(m, c_lo, c_hi):  # block cols -> packed et AP
                    key, poff = LOC[m]
                    start_m = 0 if m == 0 else 128 * (m - 1)
                    return ets[key][
                        :, poff + (c_lo - start_m) : poff + (c_hi - start_m)
                    ]

                sums = acc_p.tile([128, 1024], f32, tag="sum", name="sums")
                sum_started = [False, False]
                sum_last_m = {
                    reg: max(
                        m
                        for m in ms
                        if (0 if m == 0 else 128 * (m - 1)) < 512 * (reg + 1)
                    )
                    for reg in (0, 1)
                }
                for m in ms:
                    start_m = 0 if m == 0 else 128 * (m - 1)
                    for reg in (0, 1):
                        c_lo = max(start_m, reg * 512)
                        c_hi = (reg + 1) * 512
                        if c_lo >= c_hi:
                            continue
                        nc.tensor.matmul(
                            out=sums[:, c_lo:c_hi],
                            lhsT=ones_bf,
                            rhs=et_slice(m, c_lo, c_hi),
                            start=not sum_started[reg],
                            stop=(m == sum_last_m[reg]),
                        )
                        sum_started[reg] = True
                # Normalize via recip+mult (DVE divide is not in the ISA and
                # a DVE op may read at most one PSUM operand). The recip also
                # frees the bufs=1 sums accumulator for the next block.
                rl = combine.tile([128, 1024], bf16, tag="rl", bufs=3, name="rl")
                with nc.allow_low_precision(reason="probs sum to 1"):
                    nc.vector.reciprocal(out=rl, in_=sums)
                # PV: O^T accumulation per 512-col region. ot lives in the
                # scores pool: only held through the PV phase (freed by the
                # tl multiply), freeing two PSUM banks for a third score buf.
                ot = scores_p.tile([128, 1024], f32, tag="s", name="ot")
                tl = combine.tile([128, 1024], bf16, tag="tl", name="tl")
                for reg in (0, 1):
                    valid_ms = [
                        m
                        for m in ms
                        if (0 if m == 0 else 128 * (m - 1)) < 512 * (reg + 1)
                    ]
                    for i, m in enumerate(valid_ms):
                        start_m = 0 if m == 0 else 128 * (m - 1)
                        c_lo = max(start_m, reg * 512)
                        c_hi = (reg + 1) * 512
                        vidx = 8 * p - 1 + m
                        nc.tensor.matmul(
                            out=ot[:, c_lo:c_hi],
                            lhsT=VBF[h][:, vidx, :],
                            rhs=et_slice(m, c_lo, c_hi),
                            start=(i == 0),
                            stop=(m == valid_ms[-1]),
                        )
                with nc.allow_low_precision(reason="probs sum to 1"):
                    nc.vector.tensor_tensor(out=tl, in0=ot, in1=rl, op=MUL)
                ACC[(h, p)]["tl"] = tl

            def emit_global_scores(h, p):
                """Global chunk pair (2p, 2p+1): scores+mask+exp."""
                rows = min(KLEN, 8 * p + 8)
                sg = scores_p.tile([128, 1024], f32, tag="s", name="sg")
                for ci, c in enumerate((2 * p, 2 * p + 1)):
                    cols = slice(ci * CHUNK, (ci + 1) * CHUNK)
                    nc.tensor.matmul(
                        out=sg[0:rows, cols],
                        lhsT=KGT[h][:, 0:rows],
                        rhs=QT[h][:, c * CHUNK : (c + 1) * CHUNK],
                        start=True,
                        stop=False,
                    )
                    nc.tensor.matmul(
                        out=sg[0:rows, cols],
                        lhsT=blob[0:4, B_GB + KLEN * c : B_GB + KLEN * c + rows],
                        rhs=blob[0:4, B_GC : B_GC + CHUNK],
                        start=False,
                        stop=True,
                    )
                eg = expp.tile([128, 1024], bf16, tag="e", name="eg")
                nc.scalar.activation(
                    out=eg[0:rows, :], in_=sg[0:rows, :], func=EXP, scale=SCALE
                )
                a = ACC.setdefault((h, p), {})
                a["eg"] = eg
                a["rows"] = rows

            def emit_global_pv(h, p, split=False):
                """Global pair sums + PV matmuls."""
                ones_bf = blob[:, B_ONES : B_ONES + 128]
                eg, rows = ACC[(h, p)]["eg"], ACC[(h, p)]["rows"]
                gs = scores_p.tile([128, 1024], f32, tag="s", name="gs")
                go = scores_p.tile([128, 1024], f32, tag="s", name="go")
                for reg in (0, 1):
                    cols = slice(reg * CHUNK, (reg + 1) * CHUNK)
                    nc.tensor.matmul(
                        out=gs[:, cols],
                        lhsT=ones_bf[0:rows, :],
                        rhs=eg[0:rows, cols],
                        start=True,
                        stop=True,
                    )
                    nc.tensor.matmul(
                        out=go[:, cols],
                        lhsT=VG[h][0:rows, :],
                        rhs=eg[0:rows, cols],
                        start=True,
                        stop=True,
                    )
                # global normalize: recip (one PSUM input) then mult on DVE
                rg = combine.tile([128, 1024], bf16, tag="rg", bufs=3, name="rg")
                ogn = combine.tile([128, 1024], bf16, tag="ogn", name="ogn")
                halves = (
                    (slice(0, 512), slice(512, 1024)) if split else (slice(0, 1024),)
                )
                with nc.allow_low_precision(reason="normalized probs sum to 1"):
                    for cs in halves:
                        nc.vector.reciprocal(out=rg[:, cs], in_=gs[:, cs])
                        nc.vector.tensor_tensor(
                            out=ogn[:, cs], in0=go[:, cs], in1=rg[:, cs], op=MUL
                        )
                ACC[(h, p)]["ogn"] = ogn

            def emit_combine(h, p, split=False, fast_dma=False):
                """Branch add (SBUF-only, GpSimd) + write out. With split,
                halves go to GpSimd and DVE in parallel (tail shortening)."""
                a = ACC.pop((h, p))
                tl, ogn = a["tl"], a["ogn"]
                fin = combine.tile([128, 1024], bf16, tag="fin", name="fin")
                if split:
                    engs = [
                        (nc.gpsimd, slice(0, 512)),
                        (nc.vector, slice(512, 1024)),
                    ]
                else:
                    engs = [(nc.gpsimd, slice(0, 1024))]
                with nc.allow_low_precision(reason="normalized probs sum to 1"):
                    for add_eng, cs in engs:
                        add_eng.tensor_tensor(
                            out=fin[:, cs], in0=tl[:, cs], in1=ogn[:, cs], op=ADD
                        )
                if split:
                    for qi, cs in enumerate((slice(0, 512), slice(512, 1024))):
                        eng = nc.scalar if (fast_dma and qi == 1) else nc.sync
                        eng.dma_start(
                            out=out_d.ap()[h, :, p * L + cs.start : p * L + cs.stop],
                            in_=fin[:, cs],
                        )
                else:
                    nc.sync.dma_start(
                        out=out_d.ap()[h, :, p * L : (p + 1) * L], in_=fin
                    )

            # ---- emission order tuned for DMA latency + engine overlap ----
            QT[0] = persist.tile([D, T], bf16, tag="QT0", name="QT0")
            KT[0] = persist.tile([D, T], bf16, tag="KT0", name="KT0")
            QT[1] = persist.tile([D, T], bf16, tag="QT1", name="QT1")
            KT[1] = persist.tile([D, T], bf16, tag="KT1", name="KT1")
            make_tab(0)
            make_tab(1)
            make_vbf(0)
            make_vbf(1)
            # Startup DMA chain (single serialized DMA resource): head-0
            # K/Q land in 1024-col quarters interleaved with exactly the
            # table pieces each RoPE pass needs, so the PE's first scores
            # start at ~9.5us; V arrives in quarters just ahead of each
            # block's PV; head-1 streams in while PE chews head 0.
            dma_tab(0, "sin", 0)
            ka = dma_rope_q(0, kpk_d, 0, 0)
            dma_tab(0, "cos", 0)
            qa = dma_rope_q(0, qpk_d, 0, 0)
            # PE warmup: dummy matmuls on the (already landed) sin table keep
            # the PE busy through the DMA startup bubble so the p-state /
            # HAM clock-gate reaches full speed before the first real scores
            # (outputs are never read; the buffer recycles on write-done).
            warm = scores_p.tile([128, 1024], f32, tag="s", name="warm")
            for _ in range(8):
                nc.tensor.matmul(
                    out=warm[:, 0:512],
                    lhsT=tabs[0][:, HALF : HALF + 128],
                    rhs=tabs[0][:, HALF : HALF + 512],
                    start=True,
                    stop=True,
                )
            blob, fblob = emit_consts()
            dma_tab(0, "sin", 1)
            kb = dma_rope_q(0, kpk_d, 0, 1)
            dma_tab(0, "cos", 1)
            qb = dma_rope_q(0, qpk_d, 0, 1)
            dma_v(0, 0)
            dma_tab(1, "sin")
            kc = dma_rope_q(0, kpk_d, 1, 0)
            dma_tab(1, "cos")
            qc = dma_rope_q(0, qpk_d, 1, 0)
            dma_v(0, 1)
            kd = dma_rope_q(0, kpk_d, 1, 1)
            qd = dma_rope_q(0, qpk_d, 1, 1)
            dma_v(0, 2)
            dma_v(0, 3)
            # DVE RoPE stream (in-order queue): head-0 quarters first; the
            # head-1 halves + kg reduces interleave into the block loop so
            # the per-block tl/ogn divides never queue behind them.
            rope_q(ka, KT[0], 0, 0, t2_eng=nc.vector)
            rope_q(qa, QT[0], 0, 0, t2_eng=nc.vector)
            rope_q(kb, KT[0], 0, 1, t2_eng=nc.vector)
            rope_q(qb, QT[0], 0, 1, t2_eng=nc.vector)
            rope_q(kc, KT[0], 1, 0, t2_eng=nc.vector)
            rope_q(qc, QT[0], 1, 0, t2_eng=nc.vector)
            rope_q(kd, KT[0], 1, 1)
            rope_q(qd, QT[0], 1, 1)
            dma_v(1)
            # Block pipeline: globals spread so sg(j) lands once kgT is
            # ready and eg exps overlap locals; gs/go + combine trail so
            # score-buffer recycling never waits on the combine divides.
            seq = [(0, p) for p in range(PNUM)] + [(1, p) for p in range(PNUM)]
            n = len(seq)
            scores_at = {2: [0], 3: [1], 4: [2], 5: [3], 6: [4, 5]}
            pv_at = {4: [0], 5: [1], 6: [2, 3], 7: [4, 5]}
            dve_extra = {
                0: [lambda: rope_h(dma_rope_h(1, kpk_d, 0), KT[1], 0)],
                1: [lambda: emit_kg(0),
                    lambda: rope_h(dma_rope_h(1, qpk_d, 0), QT[1], 0)],
                2: [lambda: rope_h(dma_rope_h(1, kpk_d, 1), KT[1], 1)],
                3: [lambda: rope_h(dma_rope_h(1, qpk_d, 1), QT[1], 1),
                    lambda: emit_kg(1)],
            }
            for i, (h, p) in enumerate(seq):
                if i == n - 1:  # last pairs' exps overlap the last local
                    emit_global_scores(*seq[n - 2])
                    emit_global_scores(*seq[n - 1])
                emit_scores(h, p)
                if i >= 1:
                    emit_sumspv(*seq[i - 1])
                if i == 1:
                    emit_v_pool(0)
                if i == 3:
                    emit_v_pool(1)
                for fn in dve_extra.get(i, []):
                    fn()
                for j in scores_at.get(i, []):
                    emit_global_scores(*seq[j])
                for j in pv_at.get(i, []):
                    emit_global_pv(*seq[j], split=(j >= n - 3))
                    emit_combine(*seq[j], split=(j >= n - 3))
            emit_sumspv(*seq[n - 1])
            for j in (n - 2, n - 1):
                emit_global_pv(*seq[j], split=True)
                emit_combine(*seq[j], split=True)
    return nc


def _get_program():
    if "nc" not in _CACHE:
        _CACHE["nc"] = _build_program()
        _CACHE["consts"] = _host_constants()
    return _CACHE["nc"], _CACHE["consts"]


# ---------------------------------------------------------------- entry point
def kernel(q, k, v, zero_k, zero_v):
    nc, consts = _get_program()
    from concourse.bass_utils import run_bass_kernel_spmd

    bf = ml_dtypes.bfloat16
    q4 = np.asarray(q, dtype=np.float32).reshape(T, H, D)
    k4 = np.asarray(k, dtype=np.float32).reshape(T, H, D)
    v4 = np.asarray(v, dtype=np.float32).reshape(T, H, D)
    zk = np.asarray(zero_k, dtype=np.float32).reshape(H, D)
    zv = np.asarray(zero_v, dtype=np.float32).reshape(H, D)

    def pack_halves(xT):  # [h, D, T] -> [h, half, D, raw|swap]
        # plain partition swap; the sign lives in the sign-folded sin table
        rot = np.concatenate([xT[:, D // 2 :], xT[:, : D // 2]], axis=1)
        pk = np.empty((HPC, 2, D, 2 * HALF), dtype=np.float32)
        for half in (0, 1):
            cs = slice(half * HALF, (half + 1) * HALF)
            pk[:, half, :, 0:HALF] = xT[:, :, cs]
            pk[:, half, :, HALF:] = rot[:, :, cs]
        return pk.astype(bf)

    in_maps = []
    for core in range(NCORES):
        hs = slice(HPC * core, HPC * (core + 1))
        qT = np.ascontiguousarray(q4[:, hs].transpose(1, 2, 0))   # [h, D, T]
        kT = np.ascontiguousarray(k4[:, hs].transpose(1, 2, 0))
        # v token-major: vpk[h, p, n*128+d] = v[n*128+p, head, d]
        vpk = np.ascontiguousarray(
            v4[:, hs].reshape(NT, 128, HPC, D).transpose(2, 1, 0, 3)
        ).reshape(HPC, 128, NT * D)
        blob = consts["blob"].copy()
        blob[:, B_ZK : B_ZK + HPC] = zk[hs].T.astype(bf)
        fblob = consts["fblob"].copy()
        fblob[:, 128 : 128 + HPC] = zv[hs].T
        in_maps.append(
            {
                "qpk": pack_halves(qT),
                "kpk": pack_halves(kT),
                "vpk": vpk.astype(bf),
                "tpk": consts["tpk"],
                "blob": blob,
                "fblob": fblob,
            }
        )

    res = run_bass_kernel_spmd(nc, in_maps, core_ids=list(range(NCORES)))
    # outT per core: [HPC, D, T] -> out[t, 0, (2*core+h)*D + d]
    arr = np.stack(
        [np.asarray(res.results[i]["outT"], dtype=np.float32) for i in range(NCORES)]
    )  # [8, 2, D, T]
    out = arr.transpose(3, 0, 1, 2).reshape(T, 1, H * D)
    return np.ascontiguousarray(out.astype(np.float32))


# revision 97
# speedup vs baseline: 1.7536x; 1.0237x over previous
"""Trainium2 Bass kernel for CoreAttentionExpand (sparse local+global attention).

Sharding: tensor-parallel over heads. 16 heads / 8 cores = 2 heads per core.
Each core computes RoPE + local-block attention + pooled-global attention for
its 2 heads end-to-end (no collectives); host reassembles the full output.

Design (all-bf16 dataflow; sim ~107.7us vs 181.5us for the f32 baseline):
- Host supplies packed bf16 inputs: per (tensor, half) a [D, raw|swap] pair
  (swap = [x2; x1]; the rotate-half sign is folded into the sin table) so
  one DMA feeds one RoPE chunk; cos|sinN packed the same way; v pre-swizzled
  token-major so its DMA is fully contiguous; all mask/identity constants in
  one blob DMA. The startup DMA chain is ordered so head-0 K/Q land in
  1024-col quarters first and the PE starts scoring at ~9.5us; head-1
  streams in (half granularity) while the PE chews head 0.
- RoPE: swap*sinN on DVE, raw*cos on GpSimd (parallel), add on DVE.
- Scores are computed transposed (S^T = K @ Q^T) in bf16 (1 PE cycle/column
  at any width). Causal/history/global masks accumulate -1e4 ramp matmuls
  into the score PSUM before exp; exp underflows them to 0. The narrow
  diagonal m-tiles pack into shared PSUM tiles (A: m4|m7|m8, B: m5|m6), so
  a block needs 6 exps / 6 score buffers instead of 9.
- Per-block software pipeline (the PE queue is in-order): block p+1's dense
  score matmuls are emitted before block p's exp-dependent sums/PV phase,
  so the PE always has ready work while ACT grinds exps. PSUM: 3 rotating
  2-bank score buffers + a 2-bank sums accumulator (ot rides the score
  pool; it only lives through the PV phase).
- Global branch processes chunk pairs (2p, 2p+1) in one [rows,1024] PSUM
  tile with a 4-row cumulative column mask (one exp per pair); its
  sums/PV lag two blocks so kgT (pooled K) is ready and eg exps overlap.
- Normalization is recip+mult on DVE (no DVE divide in the ISA; at most
  one PSUM operand per op; GpSimd cannot touch PSUM), final branch add on
  GpSimd, output DMA'd as bf16 and widened on the host.
"""

import sys
import math

_REPO = "/opt/trn_rl_repo"
if _REPO not in sys.path:
    sys.path.insert(0, _REPO)

import numpy as np
import ml_dtypes

# ---------------------------------------------------------------- constants
H = 16          # heads
D = 128         # head dim
T = 4096        # tokens
L = 1024        # local block size
S = 128         # global pool stride
E = 128         # local history size
PNUM = T // L   # 4 local blocks
KLEN = T // S + 1  # 33 global keys (incl. zero token)
NCORES = 8
HPC = H // NCORES  # 2 heads per core
NEGBIG = -10000.0
SCALE = 1.0 / math.sqrt(D)
CHUNK = 512     # q-chunk width
NT = T // 128   # 32 token-tiles per head
HALF = 2048     # RoPE chunk width

# const-blob column offsets (bf16 blob)
B_MB = 0
B_MC = 128
B_ONES = 256
B_POOL = 384
B_GB = 385
B_GC = B_GB + 8 * KLEN          # 649
B_ZK = B_GC + CHUNK             # 1161
B_W = B_ZK + HPC                # 1163

_CACHE = {}


def _apply_framework_patches(bassmod, mybir, tilemod):
    """This walrus build rejects >1 sem wait per instruction; split excess
    waits onto preceding same-engine NoOps (pure scheduling transform)."""
    if getattr(tilemod.TileContext, "_wait_split_patched", False):
        return
    TileContext = tilemod.TileContext
    ScopedClock = tilemod.ScopedClock

    orig_add = TileContext._add_instruction
    ctr = [0]

    def split_add(self, inst):
        si = inst.sync_info
        if si is not None and si.on_wait and len(si.on_wait) > 1:
            ow = list(si.on_wait)
            for w in ow[:-1]:
                ctr[0] += 1
                nop = mybir.InstNoOp(name=f"I-wsplit{ctr[0]}", engine=inst.engine)
                nop.sync_info = mybir.SyncInfo(on_wait=[w], on_update=[])
                orig_add(self, nop)
            si.on_wait = [ow[-1]]
        orig_add(self, inst)

    def drain_and_barrier(self, tick_clock, wait_clock):
        nc = self.nc
        probe = nc.sync.nop(nofuse=True, hint="waitprobe")
        wait_clock.add_sem_waits(
            probe.ins, ScopedClock({None: tick_clock.global_clock})
        )
        si = probe.ins.sync_info
        ow = list(si.on_wait) if si and si.on_wait else []
        if len(ow) > 1:
            si.on_wait = ow[:1]
            for w in ow[1:]:
                n2 = nc.sync.nop(nofuse=True, hint="waitsplit")
                n2.ins.sync_info = mybir.SyncInfo(on_wait=[w], on_update=[])
        nc.sync.drain()
        nc.all_engine_barrier()
        popped = nc._tile_sem_poison_stack.pop()
        assert popped is self._sem_poison
        nc.clear_and_free_semaphores(list(self.sems.allocated().values()))
        nc.all_engine_barrier()

    TileContext._add_instruction = split_add
    TileContext._drain_and_barrier = drain_and_barrier
    TileContext._wait_split_patched = True


# ---------------------------------------------------------------- constants (host)
def _host_constants():
    bf = ml_dtypes.bfloat16
    t = np.arange(T, dtype=np.float32)
    inv = (1.0 / (10000.0 ** (np.arange(0, D, 2, dtype=np.float32) / D))).astype(
        np.float32
    )  # [64]
    emb = t[:, None] * inv[None, :]          # [T, 64]
    cos64 = np.cos(emb).astype(np.float32)
    sin64 = np.sin(emb).astype(np.float32)
    # [D, T] head-dim-major tables (plain sin; rotation sign lives in rot)
    cosT = np.concatenate([cos64, cos64], axis=1).T
    sinT = np.concatenate([sin64, sin64], axis=1).T
    # sign-folded sin: rows 0:64 negated, so the rotate-half multiply needs
    # only a partition-swapped read of the raw tensor (no negation anywhere).
    sinN = sinT.copy()
    sinN[0 : D // 2] *= -1.0
    # packed tables: tpk[half] = cos_half | sinN_half  -> [2, D, 2*HALF]
    tpk = np.empty((2, D, 2 * HALF), dtype=np.float32)
    for half in (0, 1):
        cs = slice(half * HALF, (half + 1) * HALF)
        tpk[half, :, 0:HALF] = cosT[:, cs]
        tpk[half, :, HALF:] = sinN[:, cs]
    tpk = tpk.astype(bf)

    idx = np.arange(128)
    GMROWS = 4
    gB = np.zeros((GMROWS, 8 * KLEN), dtype=np.float32)
    for c in range(8):
        for mm in range(GMROWS):
            for j in range(KLEN):
                gB[mm, KLEN * c + j] = 1.0 if j >= 4 * c + mm + 1 else 0.0
    qq = np.arange(CHUNK)
    gC = np.stack(
        [NEGBIG * ((qq >= 128 * mm) & (qq < 128 * (mm + 1))) for mm in range(GMROWS)]
    )

    blob = np.zeros((128, B_W), dtype=np.float32)
    # causal ramp: (mB^T mC)[k, q] = NEGBIG * max(k - q, 0)
    blob[:, B_MB : B_MB + 128] = idx[:, None] <= idx[None, :]     # mB [m,k]: m<=k
    blob[:, B_MC : B_MC + 128] = NEGBIG * (idx[:, None] > idx[None, :])
    blob[:, B_ONES : B_ONES + 128] = 1.0
    blob[:, B_POOL] = 1.0 / S
    blob[0:GMROWS, B_GB : B_GB + 8 * KLEN] = gB
    blob[0:GMROWS, B_GC : B_GC + CHUNK] = gC
    # zk filled per-core in kernel()
    blob = blob.astype(bf)

    fblob = np.zeros((128, 128 + HPC), dtype=np.float32)
    fblob[:, 0:128] = np.eye(128, dtype=np.float32)
    # zv filled per-core in kernel()
    return {"tpk": tpk, "blob": blob, "fblob": fblob}


# ---------------------------------------------------------------- device program
def _build_program():
    import concourse.bass as bass
    import concourse.mybir as mybir
    import concourse.tile as tile

    _apply_framework_patches(bass, mybir, tile)

    f32 = mybir.dt.float32
    bf16 = mybir.dt.bfloat16
    EXP = mybir.ActivationFunctionType.Exp
    MUL = mybir.AluOpType.mult
    ADD = mybir.AluOpType.add
    DIV = mybir.AluOpType.divide

    nc = bass.Bass()
    qpk_d = nc.dram_tensor("qpk", [HPC, 2, D, 2 * HALF], bf16, kind="ExternalInput")
    kpk_d = nc.dram_tensor("kpk", [HPC, 2, D, 2 * HALF], bf16, kind="ExternalInput")
    vpk_d = nc.dram_tensor("vpk", [HPC, D, NT * 128], bf16, kind="ExternalInput")
    tpk_d = nc.dram_tensor("tpk", [2, D, 2 * HALF], bf16, kind="ExternalInput")
    blob_d = nc.dram_tensor("blob", [128, B_W], bf16, kind="ExternalInput")
    fblob_d = nc.dram_tensor("fblob", [128, 128 + HPC], f32, kind="ExternalInput")
    out_d = nc.dram_tensor("outT", [HPC, D, T], bf16, kind="ExternalOutput")

    with tile.TileContext(nc) as tc:
        with (
            tc.tile_pool(name="persist", bufs=1) as persist,
            tc.tile_pool(name="pk", bufs=6) as pkp,
            tc.tile_pool(name="expp", bufs=22) as expp,
            tc.tile_pool(name="small", bufs=2) as small,
            tc.tile_pool(name="combine", bufs=4) as combine,
            tc.tile_pool(name="scores", bufs=3, space="PSUM") as scores_p,
            tc.tile_pool(name="acc", bufs=1, space="PSUM") as acc_p,
        ):
            QT, KT, VBF, KGT, VG = {}, {}, {}, {}, {}
            tabs = {}

            def make_tab(half):
                tab = persist.tile([D, 2 * HALF], bf16, tag=f"tab{half}",
                                   name=f"tab{half}")
                tabs[half] = tab
                return tab

            def dma_tab(half, part, qtr=None, cols=None):
                """DMA one piece of a cos|sin table. part: 'cos'|'sin';
                qtr None = whole 2048-col part, else 1024-col quarter;
                cols=(lo,hi) overrides with an explicit column range."""
                base = 0 if part == "cos" else HALF
                if cols is not None:
                    cs = slice(base + cols[0], base + cols[1])
                elif qtr is None:
                    cs = slice(base, base + HALF)
                else:
                    cs = slice(base + qtr * 1024, base + (qtr + 1) * 1024)
                nc.sync.dma_start(out=tabs[half][:, cs], in_=tpk_d.ap()[half][:, cs])

            def dma_rope_q(h, src_d, half, qtr):
                """Load raw+swap quarters ([D,1024] each) into one pk tile."""
                pk = pkp.tile([D, 2048], bf16, tag="pkq", bufs=3)
                qs = slice(qtr * 1024, (qtr + 1) * 1024)
                rs = slice(HALF + qtr * 1024, HALF + (qtr + 1) * 1024)
                nc.sync.dma_start(out=pk[:, 0:1024], in_=src_d.ap()[h, half][:, qs])
                nc.sync.dma_start(out=pk[:, 1024:2048], in_=src_d.ap()[h, half][:, rs])
                return pk

            def dma_rope_p(h, src_d, half, p0):
                """Load a raw+swap 512-col piece into one pk tile (first-
                chunk latency: smaller transfers reach the RoPE sooner)."""
                pk = pkp.tile([D, 2048], bf16, tag="pkq", bufs=3)
                nc.sync.dma_start(
                    out=pk[:, 0:512], in_=src_d.ap()[h, half][:, p0 : p0 + 512]
                )
                nc.sync.dma_start(
                    out=pk[:, 512:1024],
                    in_=src_d.ap()[h, half][:, HALF + p0 : HALF + p0 + 512],
                )
                return pk

            def rope_p(pk, dst, half, p0):
                """RoPE one 512-col piece: 3 passes (t2 on DVE too - these
                run before the big quarters, when every ns of latency counts)."""
                tab = tabs[half]
                t1 = pkp.tile([D, 1024], bf16, tag="t1q", bufs=2)
                t2 = pkp.tile([D, 1024], bf16, tag="t2q", bufs=2)
                nc.vector.tensor_tensor(
                    out=t1[:, 0:512],
                    in0=pk[:, 512:1024],
                    in1=tab[:, HALF + p0 : HALF + p0 + 512],
                    op=MUL,
                )
                nc.vector.tensor_tensor(
                    out=t2[:, 0:512], in0=pk[:, 0:512], in1=tab[:, p0 : p0 + 512],
                    op=MUL,
                )
                dc = half * HALF + p0
                nc.vector.tensor_tensor(
                    out=dst[:, dc : dc + 512], in0=t1[:, 0:512], in1=t2[:, 0:512],
                    op=ADD,
                )

            def rope_q(pk, dst, half, qtr, t2_eng=None):
                """RoPE one 1024-col quarter: swap*sin + add on DVE, raw*cos
                on GpSimd (runs in parallel, keeps the DVE queue short) --
                except the startup-critical first quarters, where the slow
                GpSimd pass would gate the PE start (t2_eng=DVE there)."""
                tab = tabs[half]
                co = qtr * 1024
                t1 = pkp.tile([D, 1024], bf16, tag="t1q", bufs=2)
                t2 = pkp.tile([D, 1024], bf16, tag="t2q", bufs=2)
                nc.vector.tensor_tensor(
                    out=t1, in0=pk[:, 1024:2048], in1=tab[:, HALF + co : HALF + co + 1024], op=MUL
                )
                (t2_eng or nc.gpsimd).tensor_tensor(
                    out=t2, in0=pk[:, 0:1024], in1=tab[:, co : co + 1024], op=MUL
                )
                dc = half * HALF + co
                nc.vector.tensor_tensor(
                    out=dst[:, dc : dc + 1024], in0=t1, in1=t2, op=ADD
                )

            def dma_rope_h(h, src_d, half):
                """Load a packed raw|swap half ([D, 2*HALF]) in one DMA."""
                pk = pkp.tile([D, 2 * HALF], bf16, tag="pk", bufs=2)
                nc.sync.dma_start(out=pk, in_=src_d.ap()[h, half])
                return pk

            def rope_h(pk, dst, half, t2_eng=None):
                """RoPE one 2048-col half: swap*sin + add on DVE, raw*cos on
                GpSimd (parallel) unless t2_eng overrides."""
                cs = slice(half * HALF, (half + 1) * HALF)
                tab = tabs[half]
                t1 = pkp.tile([D, HALF], bf16, tag="t1", bufs=2)
                t2 = pkp.tile([D, HALF], bf16, tag="t2", bufs=2)
                nc.vector.tensor_tensor(
                    out=t1, in0=pk[:, HALF:], in1=tab[:, HALF:], op=MUL
                )
                (t2_eng or nc.gpsimd).tensor_tensor(
                    out=t2, in0=pk[:, 0:HALF], in1=tab[:, 0:HALF], op=MUL
                )
                nc.vector.tensor_tensor(out=dst[:, cs], in0=t1, in1=t2, op=ADD)

            def emit_consts():
                blob = persist.tile([128, B_W], bf16, tag="blob")
                fblob = persist.tile([128, 128 + HPC], f32, tag="fblob")
                nc.sync.dma_start(out=blob, in_=blob_d.ap())
                nc.sync.dma_start(out=fblob, in_=fblob_d.ap())
                return blob, fblob

            def emit_kg(h):
                """Pooled global K for head h (after full KT RoPE)."""
                kgT = persist.tile([D, KLEN], bf16, tag=f"kgT{h}", name=f"kgT{h}")
                KGT[h] = kgT
                nc.vector.tensor_copy(out=kgT[:, 0:1], in_=blob[:, B_ZK + h : B_ZK + h + 1])
                kgf = small.tile([D, KLEN], f32, tag="kgf")
                nc.vector.tensor_reduce(
                    out=kgf[:, 1:KLEN],
                    in_=KT[h].rearrange("p (g s) -> p g s", s=S),
                    axis=mybir.AxisListType.X,
                    op=ADD,
                )
                with nc.allow_low_precision(reason="bf16 pooled keys"):
                    nc.vector.tensor_scalar_mul(
                        out=kgT[:, 1:KLEN], in0=kgf[:, 1:KLEN], scalar1=1.0 / S
                    )

            def make_vbf(h):
                VBF[h] = persist.tile([128, NT, D], bf16, tag=f"vbf{h}", name=f"vbf{h}")

            def dma_v(h, qtr=None):
                flat = VBF[h].rearrange("p n d -> p (n d)")
                if qtr is None:
                    nc.sync.dma_start(out=flat, in_=vpk_d.ap()[h])
                else:
                    cs = slice(qtr * 1024, (qtr + 1) * 1024)
                    nc.sync.dma_start(out=flat[:, cs], in_=vpk_d.ap()[h][:, cs])

            def emit_v_pool(h):
                """Pooled global V via PE: vgTp[:, g+1] = V_g^T @ (1/S)."""
                vgTp = scores_p.tile([128, 1024], f32, tag="s", name="vgTp")
                for g in range(NT):
                    nc.tensor.matmul(
                        out=vgTp[:, g + 1 : g + 2],
                        lhsT=VBF[h][:, g, :],
                        rhs=blob[:, B_POOL : B_POOL + 1],
                        start=(g == 0),
                        stop=(g == NT - 1),
                    )
                # copies on ACT so they never queue behind DVE RoPE/reduces
                vgT = small.tile([D, KLEN], f32, tag="vgT")
                nc.scalar.copy(out=vgT[:, 0:1], in_=fblob[:, 128 + h : 129 + h])
                nc.scalar.copy(out=vgT[:, 1:KLEN], in_=vgTp[:, 1:KLEN])
                # transpose -> Vg token-major [KLEN, D] bf16
                vgp = scores_p.tile([128, 1024], f32, tag="s", name="vgp")
                nc.tensor.transpose(
                    out=vgp[0:KLEN, 0:128], in_=vgT, identity=fblob[:, 0:128]
                )
                Vg = persist.tile([KLEN, 128], bf16, tag=f"Vg{h}", name=f"Vg{h}")
                nc.scalar.copy(out=Vg, in_=vgp[0:KLEN, 0:128])
                VG[h] = Vg

            ACC = {}  # (h, p) -> dict of live PSUM/SBUF tiles for deferred stages

            # local m-tile -> (tile_key, packed column offset). m0-m3 are
            # block-aligned in their own tiles; the narrow tails pack into
            # two shared tiles (A: m4|m7|m8, B: m5|m6), cutting exp count
            # and score-buffer churn from 9 to 6 per block.

            LOC = {
                0: (0, 0), 1: (1, 0), 2: (2, 128), 3: (3, 256),
                4: ("A", 0), 7: ("A", 640), 8: ("A", 896),
                5: ("B", 0), 6: ("B", 512),
            }
            TILE_W = {0: 1024, 1: 1024, 2: 1024, 3: 1024, "A": 1024, "B": 896}

            def emit_scores(h, p):
                """Local block p scores + exps (PE then ACT)."""
                mB = blob[:, B_MB : B_MB + 128]
                mC = blob[:, B_MC : B_MC + 128]
                q0 = p * L
                ms = list(range(1, 9)) if p == 0 else list(range(0, 9))
                tiles, expt = {}, {}
                for m in ms:
                    key, poff = LOC[m]
                    if key not in tiles:
                        tiles[key] = scores_p.tile([128, 1024], f32, tag="s",
                                                   name="st")
                    st = tiles[key]
                    start_m = 0 if m == 0 else 128 * (m - 1)
                    width = 1024 - start_m
                    kcol = q0 - 128 + 128 * m  # k-token start (abs)
                    # QK^T into packed cols [poff, poff+width), split at the
                    # 512-col PSUM bank boundaries of the tile
                    for r0 in range(poff - poff % 512, poff + width, 512):
                        c_lo = max(poff, r0)
                        c_hi = min(poff + width, r0 + 512)
                        is_diag_reg = m >= 1 and c_lo == poff
                        nc.tensor.matmul(
                            out=st[:, c_lo:c_hi],
                            lhsT=KT[h][:, kcol : kcol + 128],
                            rhs=QT[h][
                                :,
                                q0 + start_m + (c_lo - poff) : q0
                                + start_m
                                + (c_hi - poff),
                            ],
                            start=True,
                            stop=not is_diag_reg,
                        )
                        if is_diag_reg:
                            nc.tensor.matmul(
                                out=st[:, poff : poff + 128],
                                lhsT=mB,
                                rhs=mC,
                                start=False,
                                stop=True,
                            )
                # one exp per packed tile
                ets = {}
                for key, st in tiles.items():
                    # valid span of each tile
                    if key in (0, 1, 2, 3):
                        lo, hi = LOC[key][1], 1024
                    else:
                        lo, hi = 0, TILE_W[key]
                    et = expp.tile([128, 1024], bf16, tag="e", name="et")
                    nc.scalar.activation(
                        out=et[:, lo:hi], in_=st[:, lo:hi], func=EXP, scale=SCALE
                    )
                    ets[key] = et
                a = ACC.setdefault((h, p), {})
                a["ets"] = ets
                a["ms"] = ms

            def emit_sumspv(h, p):
                """Local block p sums, normalize, PV, tl."""
                ones_bf = blob[:, B_ONES : B_ONES + 128]
                a = ACC[(h, p)]
                ets, ms = a.pop("ets"), a.pop("ms")

                def et_slice(m, c_lo, c_hi):  # block cols -> packed et AP
                    key, poff = LOC[m]
                    start_m = 0 if m == 0 else 128 * (m - 1)
                    return ets[key][
                        :, poff + (c_lo - start_m) : poff + (c_hi - start_m)
                    ]

                sums = acc_p.tile([128, 1024], f32, tag="sum", name="sums")
                sum_started = [False, False]
                sum_last_m = {
                    reg: max(
                        m
                        for m in ms
                        if (0 if m == 0 else 128 * (m - 1)) < 512 * (reg + 1)
                    )
                    for reg in (0, 1)
                }
                for m in ms:
                    start_m = 0 if m == 0 else 128 * (m - 1)
                    for reg in (0, 1):
                        c_lo = max(start_m, reg * 512)
                        c_hi = (reg + 1) * 512
                        if c_lo >= c_hi:
                            continue
                        nc.tensor.matmul(
                            out=sums[:, c_lo:c_hi],
                            lhsT=ones_bf,
                            rhs=et_slice(m, c_lo, c_hi),
                            start=not sum_started[reg],
                            stop=(m == sum_last_m[reg]),
                        )
                        sum_started[reg] = True
                # Normalize via recip+mult (DVE divide is not in the ISA and
                # a DVE op may read at most one PSUM operand). The recip also
                # frees the bufs=1 sums accumulator for the next block.
                rl = combine.tile([128, 1024], bf16, tag="rl", bufs=3, name="rl")
                with nc.allow_low_precision(reason="probs sum to 1"):
                    nc.vector.reciprocal(out=rl, in_=sums)
                # PV: O^T accumulation per 512-col region. ot lives in the
                # scores pool: only held through the PV phase (freed by the
                # tl multiply), freeing two PSUM banks for a third score buf.
                ot = scores_p.tile([128, 1024], f32, tag="s", name="ot")
                tl = combine.tile([128, 1024], bf16, tag="tl", name="tl")
                for reg in (0, 1):
                    valid_ms = [
                        m
                        for m in ms
                        if (0 if m == 0 else 128 * (m - 1)) < 512 * (reg + 1)
                    ]
                    for i, m in enumerate(valid_ms):
                        start_m = 0 if m == 0 else 128 * (m - 1)
                        c_lo = max(start_m, reg * 512)
                        c_hi = (reg + 1) * 512
                        vidx = 8 * p - 1 + m
                        nc.tensor.matmul(
                            out=ot[:, c_lo:c_hi],
                            lhsT=VBF[h][:, vidx, :],
                            rhs=et_slice(m, c_lo, c_hi),
                            start=(i == 0),
                            stop=(m == valid_ms[-1]),
                        )
                with nc.allow_low_precision(reason="probs sum to 1"):
                    nc.vector.tensor_tensor(out=tl, in0=ot, in1=rl, op=MUL)
                ACC[(h, p)]["tl"] = tl

            def emit_global_scores(h, p):
                """Global chunk pair (2p, 2p+1): scores+mask+exp."""
                rows = min(KLEN, 8 * p + 8)
                sg = scores_p.tile([128, 1024], f32, tag="s", name="sg")
                for ci, c in enumerate((2 * p, 2 * p + 1)):
                    cols = slice(ci * CHUNK, (ci + 1) * CHUNK)
                    nc.tensor.matmul(
                        out=sg[0:rows, cols],
                        lhsT=KGT[h][:, 0:rows],
                        rhs=QT[h][:, c * CHUNK : (c + 1) * CHUNK],
                        start=True,
                        stop=False,
                    )
                    nc.tensor.matmul(
                        out=sg[0:rows, cols],
                        lhsT=blob[0:4, B_GB + KLEN * c : B_GB + KLEN * c + rows],
                        rhs=blob[0:4, B_GC : B_GC + CHUNK],
                        start=False,
                        stop=True,
                    )
                eg = expp.tile([128, 1024], bf16, tag="e", name="eg")
                nc.scalar.activation(
                    out=eg[0:rows, :], in_=sg[0:rows, :], func=EXP, scale=SCALE
                )
                a = ACC.setdefault((h, p), {})
                a["eg"] = eg
                a["rows"] = rows

            def emit_global_pv(h, p, split=False):
                """Global pair sums + PV matmuls."""
                ones_bf = blob[:, B_ONES : B_ONES + 128]
                eg, rows = ACC[(h, p)]["eg"], ACC[(h, p)]["rows"]
                gs = scores_p.tile([128, 1024], f32, tag="s", name="gs")
                go = scores_p.tile([128, 1024], f32, tag="s", name="go")
                for reg in (0, 1):
                    cols = slice(reg * CHUNK, (reg + 1) * CHUNK)
                    nc.tensor.matmul(
                        out=gs[:, cols],
                        lhsT=ones_bf[0:rows, :],
                        rhs=eg[0:rows, cols],
                        start=True,
                        stop=True,
                    )
                    nc.tensor.matmul(
                        out=go[:, cols],
                        lhsT=VG[h][0:rows, :],
                        rhs=eg[0:rows, cols],
                        start=True,
                        stop=True,
                    )
                # global normalize: recip (one PSUM input) then mult on DVE
                rg = combine.tile([128, 1024], bf16, tag="rg", bufs=3, name="rg")
                ogn = combine.tile([128, 1024], bf16, tag="ogn", name="ogn")
                halves = (
                    (slice(0, 512), slice(512, 1024)) if split else (slice(0, 1024),)
                )
                with nc.allow_low_precision(reason="normalized probs sum to 1"):
                    for cs in halves:
                        nc.vector.reciprocal(out=rg[:, cs], in_=gs[:, cs])
                        nc.vector.tensor_tensor(
                            out=ogn[:, cs], in0=go[:, cs], in1=rg[:, cs], op=MUL
                        )
                ACC[(h, p)]["ogn"] = ogn

            def emit_combine(h, p, split=False, fast_dma=False):
                """Branch add (SBUF-only, GpSimd) + write out. With split,
                halves go to GpSimd and DVE in parallel (tail shortening)."""
                a = ACC.pop((h, p))
                tl, ogn = a["tl"], a["ogn"]
                fin = combine.tile([128, 1024], bf16, tag="fin", name="fin")
                if split:
                    engs = [
                        (nc.gpsimd, slice(0, 512)),
                        (nc.vector, slice(512, 1024)),
                    ]
                else:
                    engs = [(nc.gpsimd, slice(0, 1024))]
                with nc.allow_low_precision(reason="normalized probs sum to 1"):
                    for add_eng, cs in engs:
                        add_eng.tensor_tensor(
                            out=fin[:, cs], in0=tl[:, cs], in1=ogn[:, cs], op=ADD
                        )
                if split:
                    for qi, cs in enumerate((slice(0, 512), slice(512, 1024))):
                        eng = nc.scalar if (fast_dma and qi == 1) else nc.sync
                        eng.dma_start(
                            out=out_d.ap()[h, :, p * L + cs.start : p * L + cs.stop],
                            in_=fin[:, cs],
                        )
                else:
                    nc.sync.dma_start(
                        out=out_d.ap()[h, :, p * L : (p + 1) * L], in_=fin
                    )

            # ---- emission order tuned for DMA latency + engine overlap ----
            QT[0] = persist.tile([D, T], bf16, tag="QT0", name="QT0")
            KT[0] = persist.tile([D, T], bf16, tag="KT0", name="KT0")
            QT[1] = persist.tile([D, T], bf16, tag="QT1", name="QT1")
            KT[1] = persist.tile([D, T], bf16, tag="KT1", name="KT1")
            make_tab(0)
            make_tab(1)
            make_vbf(0)
            make_vbf(1)
            # Startup DMA chain (single serialized DMA resource): head-0
            # K/Q land in 1024-col quarters interleaved with exactly the
            # table pieces each RoPE pass needs, so the PE's first scores
            # start at ~9.5us; V arrives in quarters just ahead of each
            # block's PV; head-1 streams in while PE chews head 0.
            dma_tab(0, "sin", 0)
            ka = dma_rope_q(0, kpk_d, 0, 0)
            dma_tab(0, "cos", 0)
            qa = dma_rope_q(0, qpk_d, 0, 0)
            # PE warmup: dummy matmuls on the (already landed) sin table keep
            # the PE busy through the DMA startup bubble so the p-state /
            # HAM clock-gate reaches full speed before the first real scores
            # (outputs are never read; the buffer recycles on write-done).
            warm = scores_p.tile([128, 1024], f32, tag="s", name="warm")
            for _ in range(8):
                nc.tensor.matmul(
                    out=warm[:, 0:512],
                    lhsT=tabs[0][:, HALF : HALF + 128],
                    rhs=tabs[0][:, HALF : HALF + 512],
                    start=True,
                    stop=True,
                )
            blob, fblob = emit_consts()
            dma_tab(0, "sin", 1)
            kb = dma_rope_q(0, kpk_d, 0, 1)
            dma_tab(0, "cos", 1)
            qb = dma_rope_q(0, qpk_d, 0, 1)
            dma_v(0, 0)
            dma_tab(1, "sin")
            kc = dma_rope_q(0, kpk_d, 1, 0)
            dma_tab(1, "cos")
            qc = dma_rope_q(0, qpk_d, 1, 0)
            dma_v(0, 1)
            kd = dma_rope_q(0, kpk_d, 1, 1)
            qd = dma_rope_q(0, qpk_d, 1, 1)
            dma_v(0, 2)
            dma_v(0, 3)
            # DVE RoPE stream (in-order queue): head-0 quarters first; the
            # head-1 halves + kg reduces interleave into the block loop so
            # the per-block tl/ogn divides never queue behind them.
            rope_q(ka, KT[0], 0, 0, t2_eng=nc.vector)
            rope_q(qa, QT[0], 0, 0, t2_eng=nc.vector)
            rope_q(kb, KT[0], 0, 1, t2_eng=nc.vector)
            rope_q(qb, QT[0], 0, 1, t2_eng=nc.vector)
            rope_q(kc, KT[0], 1, 0, t2_eng=nc.vector)
            rope_q(qc, QT[0], 1, 0, t2_eng=nc.vector)
            rope_q(kd, KT[0], 1, 1)
            rope_q(qd, QT[0], 1, 1)
            dma_v(1)
            # Block pipeline: globals spread so sg(j) lands once kgT is
            # ready and eg exps overlap locals; gs/go + combine trail so
            # score-buffer recycling never waits on the combine divides.
            seq = [(0, p) for p in range(PNUM)] + [(1, p) for p in range(PNUM)]
            n = len(seq)
            scores_at = {2: [0], 3: [1], 4: [2], 5: [3], 6: [4, 5]}
            pv_at = {4: [0], 5: [1], 6: [2, 3], 7: [4, 5]}
            dve_extra = {
                0: [lambda: rope_h(dma_rope_h(1, kpk_d, 0), KT[1], 0, t2_eng=nc.vector)],
                1: [lambda: emit_kg(0),
                    lambda: rope_h(dma_rope_h(1, qpk_d, 0), QT[1], 0, t2_eng=nc.vector)],
                2: [lambda: rope_h(dma_rope_h(1, kpk_d, 1), KT[1], 1)],
                3: [lambda: rope_h(dma_rope_h(1, qpk_d, 1), QT[1], 1),
                    lambda: emit_kg(1)],
            }
            for i, (h, p) in enumerate(seq):
                if i == n - 1:  # last pairs' exps overlap the last local
                    emit_global_scores(*seq[n - 2])
                    emit_global_scores(*seq[n - 1])
                emit_scores(h, p)
                if i >= 1:
                    emit_sumspv(*seq[i - 1])
                if i == 1:
                    emit_v_pool(0)
                if i == 3:
                    emit_v_pool(1)
                for fn in dve_extra.get(i, []):
                    fn()
                for j in scores_at.get(i, []):
                    emit_global_scores(*seq[j])
                for j in pv_at.get(i, []):
                    emit_global_pv(*seq[j], split=(j >= n - 3))
                    emit_combine(*seq[j], split=(j >= n - 3))
            emit_sumspv(*seq[n - 1])
            for j in (n - 2, n - 1):
                emit_global_pv(*seq[j], split=True)
                emit_combine(*seq[j], split=True)
    return nc


def _get_program():
    if "nc" not in _CACHE:
        _CACHE["nc"] = _build_program()
        _CACHE["consts"] = _host_constants()
    return _CACHE["nc"], _CACHE["consts"]


# ---------------------------------------------------------------- entry point
def kernel(q, k, v, zero_k, zero_v):
    nc, consts = _get_program()
    from concourse.bass_utils import run_bass_kernel_spmd

    bf = ml_dtypes.bfloat16
    q4 = np.asarray(q, dtype=np.float32).reshape(T, H, D)
    k4 = np.asarray(k, dtype=np.float32).reshape(T, H, D)
    v4 = np.asarray(v, dtype=np.float32).reshape(T, H, D)
    zk = np.asarray(zero_k, dtype=np.float32).reshape(H, D)
    zv = np.asarray(zero_v, dtype=np.float32).reshape(H, D)

    def pack_halves(xT):  # [h, D, T] -> [h, half, D, raw|swap]
        # plain partition swap; the sign lives in the sign-folded sin table
        rot = np.concatenate([xT[:, D // 2 :], xT[:, : D // 2]], axis=1)
        pk = np.empty((HPC, 2, D, 2 * HALF), dtype=np.float32)
        for half in (0, 1):
            cs = slice(half * HALF, (half + 1) * HALF)
            pk[:, half, :, 0:HALF] = xT[:, :, cs]
            pk[:, half, :, HALF:] = rot[:, :, cs]
        return pk.astype(bf)

    in_maps = []
    for core in range(NCORES):
        hs = slice(HPC * core, HPC * (core + 1))
        qT = np.ascontiguousarray(q4[:, hs].transpose(1, 2, 0))   # [h, D, T]
        kT = np.ascontiguousarray(k4[:, hs].transpose(1, 2, 0))
        # v token-major: vpk[h, p, n*128+d] = v[n*128+p, head, d]
        vpk = np.ascontiguousarray(
            v4[:, hs].reshape(NT, 128, HPC, D).transpose(2, 1, 0, 3)
        ).reshape(HPC, 128, NT * D)
        blob = consts["blob"].copy()
        blob[:, B_ZK : B_ZK + HPC] = zk[hs].T.astype(bf)
        fblob = consts["fblob"].copy()
        fblob[:, 128 : 128 + HPC] = zv[hs].T
        in_maps.append(
            {
                "qpk": pack_halves(qT),
                "kpk": pack_halves(kT),
                "vpk": vpk.astype(bf),
                "tpk": consts["tpk"],
                "blob": blob,
                "fblob": fblob,
            }
        )

    res = run_bass_kernel_spmd(nc, in_maps, core_ids=list(range(NCORES)))
    # outT per core: [HPC, D, T] -> out[t, 0, (2*core+h)*D + d]
    arr = np.stack(
        [np.asarray(res.results[i]["outT"], dtype=np.float32) for i in range(NCORES)]
    )  # [8, 2, D, T]
    out = arr.transpose(3, 0, 1, 2).reshape(T, 1, H * D)
    return np.ascontiguousarray(out.astype(np.float32))


# revision 106
# speedup vs baseline: 1.7692x; 1.0089x over previous
"""Trainium2 Bass kernel for CoreAttentionExpand (sparse local+global attention).

Sharding: tensor-parallel over heads. 16 heads / 8 cores = 2 heads per core.
Each core computes RoPE + local-block attention + pooled-global attention for
its 2 heads end-to-end (no collectives); host reassembles the full output.

Design (all-bf16 dataflow; sim ~107.7us vs 181.5us for the f32 baseline):
- Host supplies packed bf16 inputs: per (tensor, half) a [D, raw|swap] pair
  (swap = [x2; x1]; the rotate-half sign is folded into the sin table) so
  one DMA feeds one RoPE chunk; cos|sinN packed the same way; v pre-swizzled
  token-major so its DMA is fully contiguous; all mask/identity constants in
  one blob DMA. The startup DMA chain is ordered so head-0 K/Q land in
  1024-col quarters first and the PE starts scoring at ~9.5us; head-1
  streams in (half granularity) while the PE chews head 0.
- RoPE: swap*sinN on DVE, raw*cos on GpSimd (parallel), add on DVE.
- Scores are computed transposed (S^T = K @ Q^T) in bf16 (1 PE cycle/column
  at any width). Causal/history/global masks accumulate -1e4 ramp matmuls
  into the score PSUM before exp; exp underflows them to 0. The narrow
  diagonal m-tiles pack into shared PSUM tiles (A: m4|m7|m8, B: m5|m6), so
  a block needs 6 exps / 6 score buffers instead of 9.
- Per-block software pipeline (the PE queue is in-order): block p+1's dense
  score matmuls are emitted before block p's exp-dependent sums/PV phase,
  so the PE always has ready work while ACT grinds exps. PSUM: 3 rotating
  2-bank score buffers + a 2-bank sums accumulator (ot rides the score
  pool; it only lives through the PV phase).
- Global branch processes chunk pairs (2p, 2p+1) in one [rows,1024] PSUM
  tile with a 4-row cumulative column mask (one exp per pair); its
  sums/PV lag two blocks so kgT (pooled K) is ready and eg exps overlap.
- Normalization is recip+mult on DVE (no DVE divide in the ISA; at most
  one PSUM operand per op; GpSimd cannot touch PSUM), final branch add on
  GpSimd, output DMA'd as bf16 and widened on the host.
"""

import sys
import math

_REPO = "/opt/trn_rl_repo"
if _REPO not in sys.path:
    sys.path.insert(0, _REPO)

import numpy as np
import ml_dtypes

# ---------------------------------------------------------------- constants
H = 16          # heads
D = 128         # head dim
T = 4096        # tokens
L = 1024        # local block size
S = 128         # global pool stride
E = 128         # local history size
PNUM = T // L   # 4 local blocks
KLEN = T // S + 1  # 33 global keys (incl. zero token)
NCORES = 8
HPC = H // NCORES  # 2 heads per core
NEGBIG = -10000.0
SCALE = 1.0 / math.sqrt(D)
CHUNK = 512     # q-chunk width
NT = T // 128   # 32 token-tiles per head
HALF = 2048     # RoPE chunk width

# const-blob column offsets (bf16 blob)
B_MB = 0
B_MC = 128
B_ONES = 256
B_POOL = 384
B_GB = 385
B_GC = B_GB + 8 * KLEN          # 649
B_ZK = B_GC + CHUNK             # 1161
B_W = B_ZK + HPC                # 1163

_CACHE = {}


def _apply_framework_patches(bassmod, mybir, tilemod):
    """This walrus build rejects >1 sem wait per instruction; split excess
    waits onto preceding same-engine NoOps (pure scheduling transform)."""
    if getattr(tilemod.TileContext, "_wait_split_patched", False):
        return
    TileContext = tilemod.TileContext
    ScopedClock = tilemod.ScopedClock

    orig_add = TileContext._add_instruction
    ctr = [0]

    def split_add(self, inst):
        si = inst.sync_info
        if si is not None and si.on_wait and len(si.on_wait) > 1:
            ow = list(si.on_wait)
            for w in ow[:-1]:
                ctr[0] += 1
                nop = mybir.InstNoOp(name=f"I-wsplit{ctr[0]}", engine=inst.engine)
                nop.sync_info = mybir.SyncInfo(on_wait=[w], on_update=[])
                orig_add(self, nop)
            si.on_wait = [ow[-1]]
        orig_add(self, inst)

    def drain_and_barrier(self, tick_clock, wait_clock):
        nc = self.nc
        probe = nc.sync.nop(nofuse=True, hint="waitprobe")
        wait_clock.add_sem_waits(
            probe.ins, ScopedClock({None: tick_clock.global_clock})
        )
        si = probe.ins.sync_info
        ow = list(si.on_wait) if si and si.on_wait else []
        if len(ow) > 1:
            si.on_wait = ow[:1]
            for w in ow[1:]:
                n2 = nc.sync.nop(nofuse=True, hint="waitsplit")
                n2.ins.sync_info = mybir.SyncInfo(on_wait=[w], on_update=[])
        nc.sync.drain()
        nc.all_engine_barrier()
        popped = nc._tile_sem_poison_stack.pop()
        assert popped is self._sem_poison
        nc.clear_and_free_semaphores(list(self.sems.allocated().values()))
        nc.all_engine_barrier()

    TileContext._add_instruction = split_add
    TileContext._drain_and_barrier = drain_and_barrier
    TileContext._wait_split_patched = True


# ---------------------------------------------------------------- constants (host)
def _host_constants():
    bf = ml_dtypes.bfloat16
    t = np.arange(T, dtype=np.float32)
    inv = (1.0 / (10000.0 ** (np.arange(0, D, 2, dtype=np.float32) / D))).astype(
        np.float32
    )  # [64]
    emb = t[:, None] * inv[None, :]          # [T, 64]
    cos64 = np.cos(emb).astype(np.float32)
    sin64 = np.sin(emb).astype(np.float32)
    # [D, T] head-dim-major tables (plain sin; rotation sign lives in rot)
    cosT = np.concatenate([cos64, cos64], axis=1).T
    sinT = np.concatenate([sin64, sin64], axis=1).T
    # sign-folded sin: rows 0:64 negated, so the rotate-half multiply needs
    # only a partition-swapped read of the raw tensor (no negation anywhere).
    sinN = sinT.copy()
    sinN[0 : D // 2] *= -1.0
    # packed tables: tpk[half] = cos_half | sinN_half  -> [2, D, 2*HALF]
    tpk = np.empty((2, D, 2 * HALF), dtype=np.float32)
    for half in (0, 1):
        cs = slice(half * HALF, (half + 1) * HALF)
        tpk[half, :, 0:HALF] = cosT[:, cs]
        tpk[half, :, HALF:] = sinN[:, cs]
    tpk = tpk.astype(bf)

    idx = np.arange(128)
    GMROWS = 4
    gB = np.zeros((GMROWS, 8 * KLEN), dtype=np.float32)
    for c in range(8):
        for mm in range(GMROWS):
            for j in range(KLEN):
                gB[mm, KLEN * c + j] = 1.0 if j >= 4 * c + mm + 1 else 0.0
    qq = np.arange(CHUNK)
    gC = np.stack(
        [NEGBIG * ((qq >= 128 * mm) & (qq < 128 * (mm + 1))) for mm in range(GMROWS)]
    )

    blob = np.zeros((128, B_W), dtype=np.float32)
    # causal ramp: (mB^T mC)[k, q] = NEGBIG * max(k - q, 0)
    blob[:, B_MB : B_MB + 128] = idx[:, None] <= idx[None, :]     # mB [m,k]: m<=k
    blob[:, B_MC : B_MC + 128] = NEGBIG * (idx[:, None] > idx[None, :])
    blob[:, B_ONES : B_ONES + 128] = 1.0
    blob[:, B_POOL] = 1.0 / S
    blob[0:GMROWS, B_GB : B_GB + 8 * KLEN] = gB
    blob[0:GMROWS, B_GC : B_GC + CHUNK] = gC
    # zk filled per-core in kernel()
    blob = blob.astype(bf)

    fblob = np.zeros((128, 128 + HPC), dtype=np.float32)
    fblob[:, 0:128] = np.eye(128, dtype=np.float32)
    # zv filled per-core in kernel()
    return {"tpk": tpk, "blob": blob, "fblob": fblob}


# ---------------------------------------------------------------- device program
def _build_program():
    import concourse.bass as bass
    import concourse.mybir as mybir
    import concourse.tile as tile

    _apply_framework_patches(bass, mybir, tile)

    f32 = mybir.dt.float32
    bf16 = mybir.dt.bfloat16
    EXP = mybir.ActivationFunctionType.Exp
    MUL = mybir.AluOpType.mult
    ADD = mybir.AluOpType.add
    DIV = mybir.AluOpType.divide

    nc = bass.Bass()
    qpk_d = nc.dram_tensor("qpk", [HPC, 2, D, 2 * HALF], bf16, kind="ExternalInput")
    kpk_d = nc.dram_tensor("kpk", [HPC, 2, D, 2 * HALF], bf16, kind="ExternalInput")
    vpk_d = nc.dram_tensor("vpk", [HPC, D, NT * 128], bf16, kind="ExternalInput")
    tpk_d = nc.dram_tensor("tpk", [2, D, 2 * HALF], bf16, kind="ExternalInput")
    blob_d = nc.dram_tensor("blob", [128, B_W], bf16, kind="ExternalInput")
    fblob_d = nc.dram_tensor("fblob", [128, 128 + HPC], f32, kind="ExternalInput")
    out_d = nc.dram_tensor("outT", [HPC, D, T], bf16, kind="ExternalOutput")

    with tile.TileContext(nc) as tc:
        with (
            tc.tile_pool(name="persist", bufs=1) as persist,
            tc.tile_pool(name="pk", bufs=6) as pkp,
            tc.tile_pool(name="expp", bufs=22) as expp,
            tc.tile_pool(name="small", bufs=2) as small,
            tc.tile_pool(name="combine", bufs=4) as combine,
            tc.tile_pool(name="scores", bufs=3, space="PSUM") as scores_p,
            tc.tile_pool(name="acc", bufs=1, space="PSUM") as acc_p,
        ):
            QT, KT, VBF, KGT, VG = {}, {}, {}, {}, {}
            tabs = {}

            def make_tab(half):
                tab = persist.tile([D, 2 * HALF], bf16, tag=f"tab{half}",
                                   name=f"tab{half}")
                tabs[half] = tab
                return tab

            def dma_tab(half, part, qtr=None, cols=None):
                """DMA one piece of a cos|sin table. part: 'cos'|'sin';
                qtr None = whole 2048-col part, else 1024-col quarter;
                cols=(lo,hi) overrides with an explicit column range."""
                base = 0 if part == "cos" else HALF
                if cols is not None:
                    cs = slice(base + cols[0], base + cols[1])
                elif qtr is None:
                    cs = slice(base, base + HALF)
                else:
                    cs = slice(base + qtr * 1024, base + (qtr + 1) * 1024)
                nc.sync.dma_start(out=tabs[half][:, cs], in_=tpk_d.ap()[half][:, cs])

            def dma_rope_q(h, src_d, half, qtr):
                """Load raw+swap quarters ([D,1024] each) into one pk tile."""
                pk = pkp.tile([D, 2048], bf16, tag="pkq", bufs=3)
                qs = slice(qtr * 1024, (qtr + 1) * 1024)
                rs = slice(HALF + qtr * 1024, HALF + (qtr + 1) * 1024)
                nc.sync.dma_start(out=pk[:, 0:1024], in_=src_d.ap()[h, half][:, qs])
                nc.sync.dma_start(out=pk[:, 1024:2048], in_=src_d.ap()[h, half][:, rs])
                return pk

            def dma_rope_p(h, src_d, half, p0):
                """Load a raw+swap 512-col piece into one pk tile (first-
                chunk latency: smaller transfers reach the RoPE sooner)."""
                pk = pkp.tile([D, 2048], bf16, tag="pkq", bufs=3)
                nc.sync.dma_start(
                    out=pk[:, 0:512], in_=src_d.ap()[h, half][:, p0 : p0 + 512]
                )
                nc.sync.dma_start(
                    out=pk[:, 512:1024],
                    in_=src_d.ap()[h, half][:, HALF + p0 : HALF + p0 + 512],
                )
                return pk

            def rope_p(pk, dst, half, p0):
                """RoPE one 512-col piece: 3 passes (t2 on DVE too - these
                run before the big quarters, when every ns of latency counts)."""
                tab = tabs[half]
                t1 = pkp.tile([D, 1024], bf16, tag="t1q", bufs=2)
                t2 = pkp.tile([D, 1024], bf16, tag="t2q", bufs=2)
                nc.vector.tensor_tensor(
                    out=t1[:, 0:512],
                    in0=pk[:, 512:1024],
                    in1=tab[:, HALF + p0 : HALF + p0 + 512],
                    op=MUL,
                )
                nc.vector.tensor_tensor(
                    out=t2[:, 0:512], in0=pk[:, 0:512], in1=tab[:, p0 : p0 + 512],
                    op=MUL,
                )
                dc = half * HALF + p0
                nc.vector.tensor_tensor(
                    out=dst[:, dc : dc + 512], in0=t1[:, 0:512], in1=t2[:, 0:512],
                    op=ADD,
                )

            def rope_q(pk, dst, half, qtr, t2_eng=None):
                """RoPE one 1024-col quarter: swap*sin + add on DVE, raw*cos
                on GpSimd (runs in parallel, keeps the DVE queue short) --
                except the startup-critical first quarters, where the slow
                GpSimd pass would gate the PE start (t2_eng=DVE there)."""
                tab = tabs[half]
                co = qtr * 1024
                t1 = pkp.tile([D, 1024], bf16, tag="t1q", bufs=2)
                t2 = pkp.tile([D, 1024], bf16, tag="t2q", bufs=2)
                nc.vector.tensor_tensor(
                    out=t1, in0=pk[:, 1024:2048], in1=tab[:, HALF + co : HALF + co + 1024], op=MUL
                )
                (t2_eng or nc.gpsimd).tensor_tensor(
                    out=t2, in0=pk[:, 0:1024], in1=tab[:, co : co + 1024], op=MUL
                )
                dc = half * HALF + co
                nc.vector.tensor_tensor(
                    out=dst[:, dc : dc + 1024], in0=t1, in1=t2, op=ADD
                )

            def dma_rope_h(h, src_d, half):
                """Load a packed raw|swap half ([D, 2*HALF]) in one DMA."""
                pk = pkp.tile([D, 2 * HALF], bf16, tag="pk", bufs=2)
                nc.sync.dma_start(out=pk, in_=src_d.ap()[h, half])
                return pk

            def rope_h(pk, dst, half, t2_eng=None):
                """RoPE one 2048-col half: swap*sin + add on DVE, raw*cos on
                GpSimd (parallel) unless t2_eng overrides."""
                cs = slice(half * HALF, (half + 1) * HALF)
                tab = tabs[half]
                t1 = pkp.tile([D, HALF], bf16, tag="t1", bufs=2)
                t2 = pkp.tile([D, HALF], bf16, tag="t2", bufs=2)
                nc.vector.tensor_tensor(
                    out=t1, in0=pk[:, HALF:], in1=tab[:, HALF:], op=MUL
                )
                (t2_eng or nc.gpsimd).tensor_tensor(
                    out=t2, in0=pk[:, 0:HALF], in1=tab[:, 0:HALF], op=MUL
                )
                nc.vector.tensor_tensor(out=dst[:, cs], in0=t1, in1=t2, op=ADD)

            def emit_consts():
                blob = persist.tile([128, B_W], bf16, tag="blob")
                fblob = persist.tile([128, 128 + HPC], f32, tag="fblob")
                nc.sync.dma_start(out=blob, in_=blob_d.ap())
                nc.sync.dma_start(out=fblob, in_=fblob_d.ap())
                return blob, fblob

            def emit_kg(h):
                """Pooled global K for head h (after full KT RoPE)."""
                kgT = persist.tile([D, KLEN], bf16, tag=f"kgT{h}", name=f"kgT{h}")
                KGT[h] = kgT
                nc.vector.tensor_copy(out=kgT[:, 0:1], in_=blob[:, B_ZK + h : B_ZK + h + 1])
                kgf = small.tile([D, KLEN], f32, tag="kgf")
                nc.vector.tensor_reduce(
                    out=kgf[:, 1:KLEN],
                    in_=KT[h].rearrange("p (g s) -> p g s", s=S),
                    axis=mybir.AxisListType.X,
                    op=ADD,
                )
                with nc.allow_low_precision(reason="bf16 pooled keys"):
                    nc.vector.tensor_scalar_mul(
                        out=kgT[:, 1:KLEN], in0=kgf[:, 1:KLEN], scalar1=1.0 / S
                    )

            def make_vbf(h):
                VBF[h] = persist.tile([128, NT, D], bf16, tag=f"vbf{h}", name=f"vbf{h}")

            def dma_v(h, qtr=None):
                flat = VBF[h].rearrange("p n d -> p (n d)")
                if qtr is None:
                    nc.sync.dma_start(out=flat, in_=vpk_d.ap()[h])
                else:
                    cs = slice(qtr * 1024, (qtr + 1) * 1024)
                    nc.sync.dma_start(out=flat[:, cs], in_=vpk_d.ap()[h][:, cs])

            def emit_v_pool(h):
                """Pooled global V via PE: vgTp[:, g+1] = V_g^T @ (1/S)."""
                vgTp = scores_p.tile([128, 1024], f32, tag="s", name="vgTp")
                for g in range(NT):
                    nc.tensor.matmul(
                        out=vgTp[:, g + 1 : g + 2],
                        lhsT=VBF[h][:, g, :],
                        rhs=blob[:, B_POOL : B_POOL + 1],
                        start=(g == 0),
                        stop=(g == NT - 1),
                    )
                # copies on ACT so they never queue behind DVE RoPE/reduces
                vgT = small.tile([D, KLEN], f32, tag="vgT")
                nc.scalar.copy(out=vgT[:, 0:1], in_=fblob[:, 128 + h : 129 + h])
                nc.scalar.copy(out=vgT[:, 1:KLEN], in_=vgTp[:, 1:KLEN])
                # transpose -> Vg token-major [KLEN, D] bf16
                vgp = scores_p.tile([128, 1024], f32, tag="s", name="vgp")
                nc.tensor.transpose(
                    out=vgp[0:KLEN, 0:128], in_=vgT, identity=fblob[:, 0:128]
                )
                Vg = persist.tile([KLEN, 128], bf16, tag=f"Vg{h}", name=f"Vg{h}")
                nc.scalar.copy(out=Vg, in_=vgp[0:KLEN, 0:128])
                VG[h] = Vg

            ACC = {}  # (h, p) -> dict of live PSUM/SBUF tiles for deferred stages

            # local m-tile -> (tile_key, packed column offset). m0-m3 are
            # block-aligned in their own tiles; the narrow tails pack into
            # two shared tiles (A: m4|m7|m8, B: m5|m6), cutting exp count
            # and score-buffer churn from 9 to 6 per block.

            LOC = {
                0: (0, 0), 1: (1, 0), 2: (2, 128), 3: (3, 256),
                4: ("A", 0), 7: ("A", 640), 8: ("A", 896),
                5: ("B", 0), 6: ("B", 512),
            }
            TILE_W = {0: 1024, 1: 1024, 2: 1024, 3: 1024, "A": 1024, "B": 896}

            def emit_scores(h, p):
                """Local block p scores + exps (PE then ACT)."""
                mB = blob[:, B_MB : B_MB + 128]
                mC = blob[:, B_MC : B_MC + 128]
                q0 = p * L
                ms = list(range(1, 9)) if p == 0 else list(range(0, 9))
                tiles, expt = {}, {}
                for m in ms:
                    key, poff = LOC[m]
                    if key not in tiles:
                        tiles[key] = scores_p.tile([128, 1024], f32, tag="s",
                                                   name="st")
                    st = tiles[key]
                    start_m = 0 if m == 0 else 128 * (m - 1)
                    width = 1024 - start_m
                    kcol = q0 - 128 + 128 * m  # k-token start (abs)
                    # QK^T into packed cols [poff, poff+width), split at the
                    # 512-col PSUM bank boundaries of the tile
                    for r0 in range(poff - poff % 512, poff + width, 512):
                        c_lo = max(poff, r0)
                        c_hi = min(poff + width, r0 + 512)
                        is_diag_reg = m >= 1 and c_lo == poff
                        nc.tensor.matmul(
                            out=st[:, c_lo:c_hi],
                            lhsT=KT[h][:, kcol : kcol + 128],
                            rhs=QT[h][
                                :,
                                q0 + start_m + (c_lo - poff) : q0
                                + start_m
                                + (c_hi - poff),
                            ],
                            start=True,
                            stop=not is_diag_reg,
                        )
                        if is_diag_reg:
                            nc.tensor.matmul(
                                out=st[:, poff : poff + 128],
                                lhsT=mB,
                                rhs=mC,
                                start=False,
                                stop=True,
                            )
                # one exp per packed tile
                ets = {}
                for key, st in tiles.items():
                    # valid span of each tile
                    if key in (0, 1, 2, 3):
                        lo, hi = LOC[key][1], 1024
                    else:
                        lo, hi = 0, TILE_W[key]
                    et = expp.tile([128, 1024], bf16, tag="e", name="et")
                    nc.scalar.activation(
                        out=et[:, lo:hi], in_=st[:, lo:hi], func=EXP, scale=SCALE
                    )
                    ets[key] = et
                a = ACC.setdefault((h, p), {})
                a["ets"] = ets
                a["ms"] = ms

            def emit_sumspv(h, p):
                """Local block p sums, normalize, PV, tl."""
                ones_bf = blob[:, B_ONES : B_ONES + 128]
                a = ACC[(h, p)]
                ets, ms = a.pop("ets"), a.pop("ms")

                def et_slice(m, c_lo, c_hi):  # block cols -> packed et AP
                    key, poff = LOC[m]
                    start_m = 0 if m == 0 else 128 * (m - 1)
                    return ets[key][
                        :, poff + (c_lo - start_m) : poff + (c_hi - start_m)
                    ]

                sums = acc_p.tile([128, 1024], f32, tag="sum", name="sums")
                sum_started = [False, False]
                sum_last_m = {
                    reg: max(
                        m
                        for m in ms
                        if (0 if m == 0 else 128 * (m - 1)) < 512 * (reg + 1)
                    )
                    for reg in (0, 1)
                }
                for m in ms:
                    start_m = 0 if m == 0 else 128 * (m - 1)
                    for reg in (0, 1):
                        c_lo = max(start_m, reg * 512)
                        c_hi = (reg + 1) * 512
                        if c_lo >= c_hi:
                            continue
                        nc.tensor.matmul(
                            out=sums[:, c_lo:c_hi],
                            lhsT=ones_bf,
                            rhs=et_slice(m, c_lo, c_hi),
                            start=not sum_started[reg],
                            stop=(m == sum_last_m[reg]),
                        )
                        sum_started[reg] = True
                # Normalize via recip+mult (DVE divide is not in the ISA and
                # a DVE op may read at most one PSUM operand). The recip also
                # frees the bufs=1 sums accumulator for the next block.
                rl = combine.tile([128, 1024], bf16, tag="rl", bufs=3, name="rl")
                with nc.allow_low_precision(reason="probs sum to 1"):
                    nc.vector.reciprocal(out=rl, in_=sums)
                # PV: O^T accumulation per 512-col region. ot lives in the
                # scores pool: only held through the PV phase (freed by the
                # tl multiply), freeing two PSUM banks for a third score buf.
                ot = scores_p.tile([128, 1024], f32, tag="s", name="ot")
                tl = combine.tile([128, 1024], bf16, tag="tl", name="tl")
                for reg in (0, 1):
                    valid_ms = [
                        m
                        for m in ms
                        if (0 if m == 0 else 128 * (m - 1)) < 512 * (reg + 1)
                    ]
                    for i, m in enumerate(valid_ms):
                        start_m = 0 if m == 0 else 128 * (m - 1)
                        c_lo = max(start_m, reg * 512)
                        c_hi = (reg + 1) * 512
                        vidx = 8 * p - 1 + m
                        nc.tensor.matmul(
                            out=ot[:, c_lo:c_hi],
                            lhsT=VBF[h][:, vidx, :],
                            rhs=et_slice(m, c_lo, c_hi),
                            start=(i == 0),
                            stop=(m == valid_ms[-1]),
                        )
                with nc.allow_low_precision(reason="probs sum to 1"):
                    nc.vector.tensor_tensor(out=tl, in0=ot, in1=rl, op=MUL)
                ACC[(h, p)]["tl"] = tl

            def emit_global_scores(h, p):
                """Global chunk pair (2p, 2p+1): scores+mask+exp."""
                rows = min(KLEN, 8 * p + 8)
                sg = scores_p.tile([128, 1024], f32, tag="s", name="sg")
                for ci, c in enumerate((2 * p, 2 * p + 1)):
                    cols = slice(ci * CHUNK, (ci + 1) * CHUNK)
                    nc.tensor.matmul(
                        out=sg[0:rows, cols],
                        lhsT=KGT[h][:, 0:rows],
                        rhs=QT[h][:, c * CHUNK : (c + 1) * CHUNK],
                        start=True,
                        stop=False,
                    )
                    nc.tensor.matmul(
                        out=sg[0:rows, cols],
                        lhsT=blob[0:4, B_GB + KLEN * c : B_GB + KLEN * c + rows],
                        rhs=blob[0:4, B_GC : B_GC + CHUNK],
                        start=False,
                        stop=True,
                    )
                eg = expp.tile([128, 1024], bf16, tag="e", name="eg")
                nc.scalar.activation(
                    out=eg[0:rows, :], in_=sg[0:rows, :], func=EXP, scale=SCALE
                )
                a = ACC.setdefault((h, p), {})
                a["eg"] = eg
                a["rows"] = rows

            def emit_global_pv(h, p, split=False):
                """Global pair sums + PV matmuls."""
                ones_bf = blob[:, B_ONES : B_ONES + 128]
                eg, rows = ACC[(h, p)]["eg"], ACC[(h, p)]["rows"]
                gs = scores_p.tile([128, 1024], f32, tag="s", name="gs")
                go = scores_p.tile([128, 1024], f32, tag="s", name="go")
                for reg in (0, 1):
                    cols = slice(reg * CHUNK, (reg + 1) * CHUNK)
                    nc.tensor.matmul(
                        out=gs[:, cols],
                        lhsT=ones_bf[0:rows, :],
                        rhs=eg[0:rows, cols],
                        start=True,
                        stop=True,
                    )
                    nc.tensor.matmul(
                        out=go[:, cols],
                        lhsT=VG[h][0:rows, :],
                        rhs=eg[0:rows, cols],
                        start=True,
                        stop=True,
                    )
                # global normalize: recip (one PSUM input) then mult on DVE
                rg = combine.tile([128, 1024], bf16, tag="rg", bufs=3, name="rg")
                ogn = combine.tile([128, 1024], bf16, tag="ogn", name="ogn")
                halves = (
                    (slice(0, 512), slice(512, 1024)) if split else (slice(0, 1024),)
                )
                with nc.allow_low_precision(reason="normalized probs sum to 1"):
                    for cs in halves:
                        nc.vector.reciprocal(out=rg[:, cs], in_=gs[:, cs])
                        nc.vector.tensor_tensor(
                            out=ogn[:, cs], in0=go[:, cs], in1=rg[:, cs], op=MUL
                        )
                ACC[(h, p)]["ogn"] = ogn

            def emit_combine(h, p, split=False, fast_dma=False):
                """Branch add (SBUF-only, GpSimd) + write out. With split,
                halves go to GpSimd and DVE in parallel (tail shortening)."""
                a = ACC.pop((h, p))
                tl, ogn = a["tl"], a["ogn"]
                fin = combine.tile([128, 1024], bf16, tag="fin", name="fin")
                if split:
                    engs = [
                        (nc.gpsimd, slice(0, 512)),
                        (nc.vector, slice(512, 1024)),
                    ]
                else:
                    engs = [(nc.gpsimd, slice(0, 1024))]
                with nc.allow_low_precision(reason="normalized probs sum to 1"):
                    for add_eng, cs in engs:
                        add_eng.tensor_tensor(
                            out=fin[:, cs], in0=tl[:, cs], in1=ogn[:, cs], op=ADD
                        )
                if split:
                    for qi, cs in enumerate((slice(0, 512), slice(512, 1024))):
                        eng = nc.scalar if (fast_dma and qi == 1) else nc.sync
                        eng.dma_start(
                            out=out_d.ap()[h, :, p * L + cs.start : p * L + cs.stop],
                            in_=fin[:, cs],
                        )
                else:
                    nc.sync.dma_start(
                        out=out_d.ap()[h, :, p * L : (p + 1) * L], in_=fin
                    )

            # ---- emission order tuned for DMA latency + engine overlap ----
            QT[0] = persist.tile([D, T], bf16, tag="QT0", name="QT0")
            KT[0] = persist.tile([D, T], bf16, tag="KT0", name="KT0")
            QT[1] = persist.tile([D, T], bf16, tag="QT1", name="QT1")
            KT[1] = persist.tile([D, T], bf16, tag="KT1", name="KT1")
            make_tab(0)
            make_tab(1)
            make_vbf(0)
            make_vbf(1)
            # Startup DMA chain (single serialized DMA resource): head-0
            # K/Q land in 1024-col quarters interleaved with exactly the
            # table pieces each RoPE pass needs, so the PE's first scores
            # start at ~9.5us; V arrives in quarters just ahead of each
            # block's PV; head-1 streams in while PE chews head 0.
            dma_tab(0, "sin", 0)
            ka = dma_rope_q(0, kpk_d, 0, 0)
            dma_tab(0, "cos", 0)
            qa = dma_rope_q(0, qpk_d, 0, 0)
            # PE warmup: dummy matmuls on the (already landed) sin table keep
            # the PE busy through the DMA startup bubble so the p-state /
            # HAM clock-gate reaches full speed before the first real scores
            # (outputs are never read; the buffer recycles on write-done).
            warm = scores_p.tile([128, 1024], f32, tag="s", name="warm")
            for _ in range(8):
                nc.tensor.matmul(
                    out=warm[:, 0:512],
                    lhsT=tabs[0][:, HALF : HALF + 128],
                    rhs=tabs[0][:, HALF : HALF + 512],
                    start=True,
                    stop=True,
                )
            blob, fblob = emit_consts()
            dma_tab(0, "sin", 1)
            kb = dma_rope_q(0, kpk_d, 0, 1)
            dma_tab(0, "cos", 1)
            qb = dma_rope_q(0, qpk_d, 0, 1)
            dma_v(0, 0)
            dma_tab(1, "sin")
            kc = dma_rope_q(0, kpk_d, 1, 0)
            dma_tab(1, "cos")
            qc = dma_rope_q(0, qpk_d, 1, 0)
            dma_v(0, 1)
            kd = dma_rope_q(0, kpk_d, 1, 1)
            qd = dma_rope_q(0, qpk_d, 1, 1)
            dma_v(0, 2)
            dma_v(0, 3)
            # DVE RoPE stream (in-order queue): head-0 quarters first; the
            # head-1 halves + kg reduces interleave into the block loop so
            # the per-block tl/ogn divides never queue behind them.
            rope_q(ka, KT[0], 0, 0, t2_eng=nc.vector)
            rope_q(qa, QT[0], 0, 0, t2_eng=nc.vector)
            rope_q(kb, KT[0], 0, 1, t2_eng=nc.vector)
            rope_q(qb, QT[0], 0, 1, t2_eng=nc.vector)
            rope_q(kc, KT[0], 1, 0, t2_eng=nc.vector)
            rope_q(qc, QT[0], 1, 0, t2_eng=nc.vector)
            rope_q(kd, KT[0], 1, 1)
            rope_q(qd, QT[0], 1, 1)
            dma_v(1)
            # Block pipeline: globals spread so sg(j) lands once kgT is
            # ready and eg exps overlap locals; gs/go + combine trail so
            # score-buffer recycling never waits on the combine divides.
            seq = [(0, p) for p in range(PNUM)] + [(1, p) for p in range(PNUM)]
            n = len(seq)
            scores_at = {2: [0], 3: [1], 4: [2], 5: [3], 6: [4, 5, 6, 7]}
            pv_at = {4: [0], 5: [1], 6: [2, 3], 7: [4, 5]}
            dve_extra = {
                0: [lambda: rope_h(dma_rope_h(1, kpk_d, 0), KT[1], 0, t2_eng=nc.vector)],
                1: [lambda: emit_kg(0),
                    lambda: rope_h(dma_rope_h(1, qpk_d, 0), QT[1], 0, t2_eng=nc.vector)],
                2: [lambda: rope_h(dma_rope_h(1, kpk_d, 1), KT[1], 1)],
                3: [lambda: rope_h(dma_rope_h(1, qpk_d, 1), QT[1], 1),
                    lambda: emit_kg(1)],
            }
            for i, (h, p) in enumerate(seq):
                emit_scores(h, p)
                if i >= 1:
                    emit_sumspv(*seq[i - 1])
                if i == 1:
                    emit_v_pool(0)
                if i == 3:
                    emit_v_pool(1)
                for fn in dve_extra.get(i, []):
                    fn()
                for j in scores_at.get(i, []):
                    emit_global_scores(*seq[j])
                for j in pv_at.get(i, []):
                    emit_global_pv(*seq[j], split=(j >= n - 3))
                    emit_combine(*seq[j], split=(j >= n - 3))
            emit_sumspv(*seq[n - 1])
            for j in (n - 2, n - 1):
                emit_global_pv(*seq[j], split=True)
                emit_combine(*seq[j], split=True)
    return nc


def _get_program():
    if "nc" not in _CACHE:
        _CACHE["nc"] = _build_program()
        _CACHE["consts"] = _host_constants()
    return _CACHE["nc"], _CACHE["consts"]


# ---------------------------------------------------------------- entry point
def kernel(q, k, v, zero_k, zero_v):
    nc, consts = _get_program()
    from concourse.bass_utils import run_bass_kernel_spmd

    bf = ml_dtypes.bfloat16
    q4 = np.asarray(q, dtype=np.float32).reshape(T, H, D)
    k4 = np.asarray(k, dtype=np.float32).reshape(T, H, D)
    v4 = np.asarray(v, dtype=np.float32).reshape(T, H, D)
    zk = np.asarray(zero_k, dtype=np.float32).reshape(H, D)
    zv = np.asarray(zero_v, dtype=np.float32).reshape(H, D)

    def pack_halves(xT):  # [h, D, T] -> [h, half, D, raw|swap]
        # plain partition swap; the sign lives in the sign-folded sin table
        rot = np.concatenate([xT[:, D // 2 :], xT[:, : D // 2]], axis=1)
        pk = np.empty((HPC, 2, D, 2 * HALF), dtype=np.float32)
        for half in (0, 1):
            cs = slice(half * HALF, (half + 1) * HALF)
            pk[:, half, :, 0:HALF] = xT[:, :, cs]
            pk[:, half, :, HALF:] = rot[:, :, cs]
        return pk.astype(bf)

    in_maps = []
    for core in range(NCORES):
        hs = slice(HPC * core, HPC * (core + 1))
        qT = np.ascontiguousarray(q4[:, hs].transpose(1, 2, 0))   # [h, D, T]
        kT = np.ascontiguousarray(k4[:, hs].transpose(1, 2, 0))
        # v token-major: vpk[h, p, n*128+d] = v[n*128+p, head, d]
        vpk = np.ascontiguousarray(
            v4[:, hs].reshape(NT, 128, HPC, D).transpose(2, 1, 0, 3)
        ).reshape(HPC, 128, NT * D)
        blob = consts["blob"].copy()
        blob[:, B_ZK : B_ZK + HPC] = zk[hs].T.astype(bf)
        fblob = consts["fblob"].copy()
        fblob[:, 128 : 128 + HPC] = zv[hs].T
        in_maps.append(
            {
                "qpk": pack_halves(qT),
                "kpk": pack_halves(kT),
                "vpk": vpk.astype(bf),
                "tpk": consts["tpk"],
                "blob": blob,
                "fblob": fblob,
            }
        )

    res = run_bass_kernel_spmd(nc, in_maps, core_ids=list(range(NCORES)))
    # outT per core: [HPC, D, T] -> out[t, 0, (2*core+h)*D + d]
    arr = np.stack(
        [np.asarray(res.results[i]["outT"], dtype=np.float32) for i in range(NCORES)]
    )  # [8, 2, D, T]
    out = arr.transpose(3, 0, 1, 2).reshape(T, 1, H * D)
    return np.ascontiguousarray(out.astype(np.float32))
